# revision 26
# baseline (speedup 1.0000x reference)
"""Trainium2 Bass kernel for nn_CustomModel_7378753814838.

Math (reference):
    a = x1.reshape(N,R,F); b = x2.reshape(N,R,F)
    d2[k,n,i,j] = ||a[n,i] - b[n,j] - m_k||^2
    kv = exp(-d2 / (2*sigma_k^2));  out = sum_k w_k * softmax_j(exp(kv))
    with w = softmax(1/sigma_params^2)

Fast path (single surviving kernel k, |sc_k * d2| small -- true for the
staged data, where w is one-hot and sigma ~ -108):
    softmax_j(exp(exp(x))) is invariant to positive scaling of exp(exp(x)),
    and over the actual x = sc*d2 range (|x| < 0.04) a monic quadratic
    (x+h)^2 + g fits exp(exp(x)) to ~1e-6 relative.  Undoing the sc scale,
    p = (d2 + h/sc)^2 + g/sc^2, so the device needs NO transcendentals and
    no per-element scale at all:

    - host: quantize -2(a-m) and b to fp8, transposed to [F, n, i]; compute
      v = rowA + h/sc (split hi/lo bf16) and colB (bf16) from the QUANTIZED
      values so d2 is exact for the quantized inputs
    - PE: per sample, one fp8 128^3 matmul (-2 dot) plus one contraction-3
      bf16 matmul adding v_hi[i] + v_lo[i] + colB[j]; PSUM then holds
      u = d2 + h/sc
    - ACT: per sample one Square: P = u^2 (bf16); samples use one PSUM
      bank each (8 rotating banks) so the PE pipelines 2-matmul chains
    - DVE: per group row-sum of P; per 8 samples a tiny chain
      rec = 1/(S/128 + g') = 128/(S + 128 g'), gr1 = g'*rec - 1; per sample
      one tensor_scalar: delta = P*rec + gr1  (= 128*softmax - 1, bf16)
    - host: out = (delta + 1) / 128

    DMA: input chunks spread across the SP / Activation / Pool queues;
    finals split across DVE and Pool; last output sample exits via the
    Pool queue to shorten the tail.

Sharding: data-parallel over N across 8 cores (16 samples each).
Fallback path (multiple kernels or large |x|): exp/exp via ACT, correct for
any parameters.
"""

import numpy as np

N, R, F, K = 128, 128, 128, 4
NCORES = 8
NP = N // NCORES  # samples per core
GS = 4            # samples per PSUM group (one 2KB psum bank)
NG = NP // GS


def _mld():
    import ml_dtypes

    return ml_dtypes


def _fit_quad(xlo):
    """Least-squares quadratic fit of exp(exp(x)) on [xlo, 0], normalized to
    monic form p(x) = (x+h)^2 + g (softmax is invariant to the scale)."""
    xs = np.linspace(xlo, 0.0, 4001)
    p = np.exp(np.exp(xs))
    M = np.stack([xs * xs, xs, np.ones_like(xs)], 1)
    (a2, a1, a0), *_ = np.linalg.lstsq(M, p, rcond=None)
    h = a1 / (2.0 * a2)
    g = a0 / a2 - h * h
    return float(h), float(g)


def _plan(x1, x2, sigmas, means, sigma_params):
    mld = _mld()
    f8 = mld.float8_e4m3
    bf16 = mld.bfloat16

    sig = np.asarray(sigmas, dtype=np.float64)
    mu = np.asarray(means, dtype=np.float64)
    sp = np.asarray(sigma_params, dtype=np.float64)
    logits = 1.0 / (sp * sp)
    e = np.exp(logits - logits.max())
    w = e / e.sum()
    KS = [k for k in range(K) if w[k] > 1e-4]
    wk = {k: float(w[k] / sum(w[k2] for k2 in KS)) for k in KS}
    SC = {k: float(-1.0 / (2.0 * sig[k] * sig[k])) for k in KS}

    a = x1.reshape(N, R, F).astype(np.float32)
    b = x2.reshape(N, R, F).astype(np.float32)
    Bq = b.astype(f8)
    colB = (Bq.astype(np.float32).astype(np.float64) ** 2).sum(-1)  # [N, R]
    BT = np.ascontiguousarray(Bq.transpose(2, 0, 1))                # [F,N,R]

    plan = {
        "KS": KS, "w": wk, "sc": SC, "BT": BT, "colB": colB,
        "AT": {}, "rowA": {}, "mode": {}, "h": {}, "g": {},
    }
    cb_sqrt_max = np.sqrt(colB).max(axis=1)
    for k in KS:
        A2 = (-2.0 * (a - np.float32(mu[k]))).astype(f8)
        rowA = (A2.astype(np.float32).astype(np.float64) ** 2).sum(-1) / 4.0
        plan["AT"][k] = np.ascontiguousarray(A2.transpose(2, 0, 1))
        plan["rowA"][k] = rowA
        d2ub = ((np.sqrt(rowA).max(axis=1) + cb_sqrt_max) ** 2).max()
        xlo = SC[k] * d2ub
        xfit = -float(2.0 ** np.ceil(np.log2(max(-xlo * 1.05, 1e-4))))
        if -xfit <= 0.35:
            h, g = _fit_quad(xfit)
            plan["mode"][k] = "poly"
            plan["h"][k], plan["g"][k] = h, g
        else:
            plan["mode"][k] = "exp"
            plan["h"][k], plan["g"][k] = 0.0, 0.0
    plan["fast"] = len(KS) == 1 and plan["mode"][KS[0]] == "poly"
    return plan


def _core_inputs_fast(plan, c):
    """Per-core input arrays for the fast path."""
    mld = _mld()
    bf16 = mld.bfloat16
    k = plan["KS"][0]
    s = slice(c * NP, (c + 1) * NP)
    sc, h = plan["sc"][k], plan["h"][k]
    xin = np.empty((F, 2, NP, R), dtype=mld.float8_e4m3)
    xin[:, 0] = plan["AT"][k][:, s, :]
    xin[:, 1] = plan["BT"][:, s, :]
    v = plan["rowA"][k][s] + h / sc                      # [NP, R] f64
    vhi = v.astype(np.float32).astype(bf16)
    vlo = (v - vhi.astype(np.float64)).astype(np.float32).astype(bf16)
    fold = np.zeros((3, 2, NP, R), dtype=bf16)
    fold[0, 0] = vhi
    fold[1, 0] = vlo
    fold[2, 0] = np.ones((NP, R), dtype=bf16)
    fold[0, 1] = np.ones((NP, R), dtype=bf16)
    fold[1, 1] = np.ones((NP, R), dtype=bf16)
    fold[2, 1] = plan["colB"][s].astype(np.float32).astype(bf16)
    return {"xin": np.ascontiguousarray(xin), "fold": np.ascontiguousarray(fold)}


def _build_nc_fast(gq):
    """Fast-path kernel; gq = g/sc^2 is the only baked constant."""
    from contextlib import ExitStack

    import concourse.bacc as bacc
    import concourse.tile as tile
    from concourse import mybir

    f32 = mybir.dt.float32
    bf16 = mybir.dt.bfloat16
    f8 = mybir.dt.float8e4
    ALU = mybir.AluOpType
    ACTF = mybir.ActivationFunctionType

    nc = bacc.Bacc(
        "TRN2",
        target_bir_lowering=False,
        debug=False,
        enable_asserts=False,
        num_devices=NCORES,
    )
    xind = nc.dram_tensor("xin", [F, 2, NP, R], f8, kind="ExternalInput").ap()
    foldd = nc.dram_tensor(
        "fold", [3, 2, NP, R], bf16, kind="ExternalInput"
    ).ap()
    yd = nc.dram_tensor("y", [R, NP, R], bf16, kind="ExternalOutput").ap()

    c_add = float(R * gq)          # S + 128*g'
    c_mul = float(R * gq)          # rec * 128*g'  (then -1)

    with ExitStack() as ctx:
        tc = ctx.enter_context(tile.TileContext(nc))
        singles = ctx.enter_context(tc.tile_pool(name="singles", bufs=1))
        inp = ctx.enter_context(tc.tile_pool(name="inp", bufs=NG))
        pp = ctx.enter_context(tc.tile_pool(name="pp", bufs=NG))
        op = ctx.enter_context(tc.tile_pool(name="op", bufs=NG))
        ps = ctx.enter_context(tc.tile_pool(name="ps", bufs=4, space="PSUM"))

        FT = singles.tile([3, 2, NP, R], bf16)
        nc.gpsimd.dma_start(FT[:], foldd)

        IN = {}
        for g in range(NG):
            IN[g] = inp.tile([F, 2, GS, R], f8, tag=f"in{g}", name=f"in_{g}")
        nc.sync.dma_start(IN[0][:], xind[:, :, 0:GS, :])
        nc.scalar.dma_start(IN[1][:], xind[:, :, GS : 2 * GS, :])
        nc.scalar.dma_start(IN[2][:], xind[:, :, 2 * GS : 3 * GS, :])
        nc.sync.dma_start(IN[3][:], xind[:, :, 3 * GS : 4 * GS, :])

        P = {}
        scolt = {
            g: singles.tile([R, GS], f32, name=f"scol{g}") for g in range(NG)
        }
        s2t = {
            g: singles.tile([R, GS], f32, name=f"s2_{g}") for g in range(NG)
        }
        rec = {
            g: singles.tile([R, GS], f32, name=f"rec{g}") for g in range(NG)
        }
        gr = {
            g: singles.tile([R, GS], f32, name=f"gr{g}") for g in range(NG)
        }

        def tiny(g):
            # rec = 1/(S/128 + g') = 128/(S + 128 g') ; gr = g'*rec - 1
            nc.vector.tensor_scalar(
                s2t[g][:], scolt[g][:], 1.0 / R, float(gq), op0=ALU.mult,
                op1=ALU.add,
            )
            nc.vector.reciprocal_approx_fast(rec[g][:], s2t[g][:])
            nc.vector.tensor_scalar(
                gr[g][:], rec[g][:], float(gq), -1.0, op0=ALU.mult,
                op1=ALU.add,
            )

        OUTT = {}

        def finals(g2, engs):
            OUTt = op.tile([R, GS, R], bf16, tag=f"OUT{g2}", name=f"OUT_{g2}")
            OUTT[g2] = OUTt
            for q in range(GS):
                rs = rec[g2][:, q : q + 1]
                gs_ = gr[g2][:, q : q + 1]
                eng = engs[q]
                if eng == "act":
                    nc.scalar.activation(
                        OUTt[:, q, :], P[g2][:, q, :], ACTF.Identity,
                        bias=gs_, scale=rs,
                    )
                elif eng == "pool":
                    nc.gpsimd.tensor_scalar(
                        OUTt[:, q, :], P[g2][:, q, :], rs, gs_,
                        op0=ALU.mult, op1=ALU.add,
                    )
                else:
                    nc.vector.tensor_scalar(
                        OUTt[:, q, :], P[g2][:, q, :], rs, gs_,
                        op0=ALU.mult, op1=ALU.add,
                    )

        for g in range(NG):
            P[g] = pp.tile([R, GS, R], bf16, tag=f"P{g}", name=f"P_{g}")
            for h in range(GS // 2):
                # one PSUM tile spanning TWO banks: each sample keeps its own
                # bank (independent 2-matmul chain, PE pipelines), while one
                # strided ACT Square covers the pair (halves ACT op count)
                pair = ps.tile(
                    [R, 2, GS, R], f32, tag="ps", name=f"pp_{g}_{h}"
                )
                for m in range(2):
                    q = 2 * h + m
                    n = GS * g + q
                    u = pair[:, m, 0, :]
                    nc.tensor.matmul(
                        u,
                        lhsT=IN[g][:, 0, q, :],
                        rhs=IN[g][:, 1, q, :],
                        start=True,
                        stop=False,
                    )
                    nc.tensor.matmul(
                        u,
                        lhsT=FT[:, 0, n, :],
                        rhs=FT[:, 1, n, :],
                        start=False,
                        stop=True,
                    )
                nc.scalar.activation(
                    P[g][:, 2 * h : 2 * h + 2, :],
                    pair[:, :, 0, :],
                    ACTF.Square,
                )
            nc.vector.tensor_reduce(
                scolt[g][:],
                P[g][:],
                axis=mybir.AxisListType.X,
                op=ALU.add,
            )
            tiny(g)
            if g < 2:
                finals(g, ["dve", "dve", "pool", "pool"])
                nc.sync.dma_start(
                    yd[:, GS * g : GS * (g + 1), :], OUTT[g][:]
                )

        # groups 2,3: finals after ALL squares so ACT can take a lane
        # without blocking the square pipeline (in-order ACT queue)
        finals(2, ["dve", "act", "pool", "pool"])
        nc.sync.dma_start(yd[:, 8:12, :], OUTT[2][:])
        finals(3, ["dve", "act", "pool", "pool"])
        nc.sync.dma_start(yd[:, 12:14, :], OUTT[3][:, 0:2, :])
        nc.gpsimd.dma_start(yd[:, 14:NP, :], OUTT[3][:, 2:GS, :])

    nc.compile()
    return nc


def _build_nc_general(key):
    """Exp/exp fallback (correct for any parameters); key carries per-kernel
    (mode, sc, h, g, w)."""
    from contextlib import ExitStack

    import concourse.bacc as bacc
    import concourse.tile as tile
    from concourse import mybir

    f32 = mybir.dt.float32
    bf16 = mybir.dt.bfloat16
    f8 = mybir.dt.float8e4
    ALU = mybir.AluOpType
    ACTF = mybir.ActivationFunctionType
    mld = _mld()

    KS, per_k = key
    KS = list(KS)
    per_k = dict(zip(KS, per_k))

    nc = bacc.Bacc(
        "TRN2",
        target_bir_lowering=False,
        debug=False,
        enable_asserts=False,
        num_devices=NCORES,
    )
    ATd = {
        k: nc.dram_tensor(f"at{k}", [F, NP, R], f8, kind="ExternalInput").ap()
        for k in KS
    }
    BTd = nc.dram_tensor("bt", [F, NP, R], f8, kind="ExternalInput").ap()
    CBd = nc.dram_tensor("cb", [1, NP, R], bf16, kind="ExternalInput").ap()
    BIASd = {
        k: nc.dram_tensor(f"bias{k}", [R, NP], f32, kind="ExternalInput").ap()
        for k in KS
    }
    Yd = nc.dram_tensor("y", [R, NP, R], f32, kind="ExternalOutput").ap()
    onesd = nc.inline_tensor(
        np.ones((1, R), dtype=mld.bfloat16), name="ones1"
    ).ap()

    with ExitStack() as ctx:
        tc = ctx.enter_context(tile.TileContext(nc))
        singles = ctx.enter_context(tc.tile_pool(name="singles", bufs=1))
        inp = ctx.enter_context(tc.tile_pool(name="inp", bufs=2 * NG))
        pp = ctx.enter_context(tc.tile_pool(name="pp", bufs=3))
        cols = ctx.enter_context(tc.tile_pool(name="cols", bufs=2 * NG))
        ps = ctx.enter_context(tc.tile_pool(name="ps", bufs=4, space="PSUM"))

        ones = singles.tile([1, R], bf16)
        nc.sync.dma_start(ones[:], onesd)
        CBt = singles.tile([1, NP, R], bf16)
        nc.sync.dma_start(CBt[:], CBd)
        BIASt = {
            k: singles.tile([R, NP], f32, name=f"biast{k}") for k in KS
        }
        for k in KS:
            nc.sync.dma_start(BIASt[k][:], BIASd[k])

        AT = {}
        BT = {}
        for g in range(NG):
            s = slice(GS * g, GS * (g + 1))
            for k in KS:
                AT[(k, g)] = inp.tile(
                    [F, GS, R], f8, tag=f"at{k}{g % 2}", name=f"at{k}_{g}"
                )
                nc.sync.dma_start(AT[(k, g)][:], ATd[k][:, s, :])
            BT[g] = inp.tile([F, GS, R], f8, tag=f"bt{g % 2}", name=f"bt_{g}")
            nc.scalar.dma_start(BT[g][:], BTd[:, s, :])

        OUTacc = singles.tile([R, NP, R], f32)

        for g in range(NG):
            s = slice(GS * g, GS * (g + 1))
            for ki, k in enumerate(KS):
                mode, sc, h, gq, wkk = per_k[k]
                pst = ps.tile([R, GS, R], f32, tag="ps")
                for q in range(GS):
                    nc.tensor.matmul(
                        pst[:, q, :],
                        lhsT=AT[(k, g)][:, q, :],
                        rhs=BT[g][:, q, :],
                        start=(q == 0),
                        stop=False,
                    )
                nc.tensor.matmul(
                    pst[:, :, :],
                    lhsT=ones[:],
                    rhs=CBt[:, s, :],
                    start=False,
                    stop=True,
                )
                scol = cols.tile([R, GS], f32, tag="scol")
                KV = pp.tile([R, GS, R], f32, tag="KV")
                E = pp.tile([R, GS, R], f32, tag="E")
                for q in range(GS):
                    n = GS * g + q
                    nc.scalar.activation(
                        KV[:, q, :],
                        pst[:, q, :],
                        ACTF.Exp,
                        bias=BIASt[k][:, n : n + 1],
                        scale=sc,
                    )
                    nc.scalar.activation(
                        E[:, q, :],
                        KV[:, q, :],
                        ACTF.Exp,
                        accum_out=scol[:, q : q + 1],
                    )
                rcol = cols.tile([R, GS], f32, tag="rcol")
                nc.vector.reciprocal_approx_fast(rcol[:], scol[:])
                if wkk != 1.0:
                    nc.vector.tensor_scalar(
                        rcol[:], rcol[:], float(wkk), None, op0=ALU.mult
                    )
                for q in range(GS):
                    n = GS * g + q
                    if ki == 0:
                        nc.vector.tensor_scalar(
                            OUTacc[:, n, :],
                            E[:, q, :],
                            rcol[:, q : q + 1],
                            None,
                            op0=ALU.mult,
                        )
                    else:
                        nc.vector.scalar_tensor_tensor(
                            OUTacc[:, n, :],
                            E[:, q, :],
                            rcol[:, q : q + 1],
                            OUTacc[:, n, :],
                            op0=ALU.mult,
                            op1=ALU.add,
                        )
            eng = nc.sync if g % 2 == 0 else nc.scalar
            eng.dma_start(Yd[:, s, :], OUTacc[:, s, :])

    nc.compile()
    return nc


_CACHE = {}


def run(x1, x2, sigmas, means, sigma_params, trace=False, **rk):
    from concourse.bass_utils import run_bass_kernel_spmd

    x1 = np.ascontiguousarray(x1, dtype=np.float32)
    x2 = np.ascontiguousarray(x2, dtype=np.float32)
    plan = _plan(x1, x2, sigmas, means, sigma_params)
    KS = plan["KS"]

    if plan["fast"]:
        k = KS[0]
        gq = plan["g"][k] / (plan["sc"][k] ** 2)
        key = ("fast", float(gq))
        if key not in _CACHE:
            _CACHE[key] = _build_nc_fast(float(gq))
        nc = _CACHE[key]
        in_maps = [_core_inputs_fast(plan, c) for c in range(NCORES)]
        res = run_bass_kernel_spmd(
            nc, in_maps, core_ids=list(range(NCORES)), trace=trace, **rk
        )
        out = np.concatenate(
            [
                (
                    (np.asarray(r["y"]).astype(np.float32) + 1.0)
                    * np.float32(1.0 / R)
                ).transpose(1, 0, 2)
                for r in res.results
            ],
            axis=0,
        )
        return out, res

    key = (
        tuple(KS),
        tuple(
            (plan["mode"][k], plan["sc"][k], plan["h"][k], plan["g"][k],
             plan["w"][k])
            for k in KS
        ),
    )
    if key not in _CACHE:
        _CACHE[key] = _build_nc_general(key)
    nc = _CACHE[key]
    in_maps = []
    for c in range(NCORES):
        s = slice(c * NP, (c + 1) * NP)
        m = {
            "bt": np.ascontiguousarray(plan["BT"][:, s, :]),
            "cb": np.ascontiguousarray(
                plan["colB"][s].astype(np.float32).astype(_mld().bfloat16)
            )[None],
        }
        for k in KS:
            m[f"at{k}"] = np.ascontiguousarray(plan["AT"][k][:, s, :])
            bias = plan["sc"][k] * plan["rowA"][k][s]  # [NP, R]
            m[f"bias{k}"] = np.ascontiguousarray(
                bias.astype(np.float32).transpose()
            )
        in_maps.append(m)
    res = run_bass_kernel_spmd(
        nc, in_maps, core_ids=list(range(NCORES)), trace=trace, **rk
    )
    out = np.concatenate(
        [np.asarray(r["y"]).astype(np.float32).transpose(1, 0, 2)
         for r in res.results],
        axis=0,
    )
    return out, res


def kernel(x1, x2, sigmas, means, sigma_params):
    out, _ = run(x1, x2, sigmas, means, sigma_params, trace=False)
    return out


# revision 27
# speedup vs baseline: 1.0585x; 1.0585x over previous
"""Trainium2 Bass kernel for nn_CustomModel_7378753814838.

Math (reference):
    a = x1.reshape(N,R,F); b = x2.reshape(N,R,F)
    d2[k,n,i,j] = ||a[n,i] - b[n,j] - m_k||^2
    kv = exp(-d2 / (2*sigma_k^2));  out = sum_k w_k * softmax_j(exp(kv))
    with w = softmax(1/sigma_params^2)

Fast path (single surviving kernel k, |sc_k * d2| small -- true for the
staged data, where w is one-hot and sigma ~ -108):
    softmax_j(exp(exp(x))) is invariant to positive scaling of exp(exp(x)),
    and over the actual x = sc*d2 range (|x| < 0.04) a monic quadratic
    (x+h)^2 + g fits exp(exp(x)) to ~1e-6 relative.  Undoing the sc scale,
    p = (d2 + h/sc)^2 + g/sc^2, so the device needs NO transcendentals and
    no per-element scale at all:

    - host: quantize -2(a-m) and b to fp8, transposed to [F, n, i]; compute
      v = rowA + h/sc (split hi/lo bf16) and colB (bf16) from the QUANTIZED
      values so d2 is exact for the quantized inputs
    - PE: per sample, one fp8 128^3 matmul (-2 dot) plus one contraction-3
      bf16 matmul adding v_hi[i] + v_lo[i] + colB[j]; PSUM then holds
      u = d2 + h/sc
    - ACT: per sample one Square: P = u^2 (bf16); samples use one PSUM
      bank each (8 rotating banks) so the PE pipelines 2-matmul chains
    - DVE: per group row-sum of P; per 8 samples a tiny chain
      rec = 1/(S/128 + g') = 128/(S + 128 g'), gr1 = g'*rec - 1; per sample
      one tensor_scalar: delta = P*rec + gr1  (= 128*softmax - 1, bf16)
    - host: out = (delta + 1) / 128

    DMA: input chunks spread across the SP / Activation / Pool queues;
    finals split across DVE and Pool; last output sample exits via the
    Pool queue to shorten the tail.

Sharding: data-parallel over N across 8 cores (16 samples each).
Fallback path (multiple kernels or large |x|): exp/exp via ACT, correct for
any parameters.
"""

import numpy as np

N, R, F, K = 128, 128, 128, 4
NCORES = 8
NP = N // NCORES  # samples per core
GS = 4            # samples per PSUM group (one 2KB psum bank)
NG = NP // GS


def _mld():
    import ml_dtypes

    return ml_dtypes


def _fit_quad(xlo):
    """Least-squares quadratic fit of exp(exp(x)) on [xlo, 0], normalized to
    monic form p(x) = (x+h)^2 + g (softmax is invariant to the scale)."""
    xs = np.linspace(xlo, 0.0, 4001)
    p = np.exp(np.exp(xs))
    M = np.stack([xs * xs, xs, np.ones_like(xs)], 1)
    (a2, a1, a0), *_ = np.linalg.lstsq(M, p, rcond=None)
    h = a1 / (2.0 * a2)
    g = a0 / a2 - h * h
    return float(h), float(g)


def _plan(x1, x2, sigmas, means, sigma_params):
    mld = _mld()
    f8 = mld.float8_e4m3
    bf16 = mld.bfloat16

    sig = np.asarray(sigmas, dtype=np.float64)
    mu = np.asarray(means, dtype=np.float64)
    sp = np.asarray(sigma_params, dtype=np.float64)
    logits = 1.0 / (sp * sp)
    e = np.exp(logits - logits.max())
    w = e / e.sum()
    KS = [k for k in range(K) if w[k] > 1e-4]
    wk = {k: float(w[k] / sum(w[k2] for k2 in KS)) for k in KS}
    SC = {k: float(-1.0 / (2.0 * sig[k] * sig[k])) for k in KS}

    a = x1.reshape(N, R, F).astype(np.float32)
    b = x2.reshape(N, R, F).astype(np.float32)
    Bq = b.astype(f8)
    colB = (Bq.astype(np.float32).astype(np.float64) ** 2).sum(-1)  # [N, R]
    BT = np.ascontiguousarray(Bq.transpose(2, 0, 1))                # [F,N,R]

    plan = {
        "KS": KS, "w": wk, "sc": SC, "BT": BT, "colB": colB,
        "AT": {}, "rowA": {}, "mode": {}, "h": {}, "g": {},
    }
    cb_sqrt_max = np.sqrt(colB).max(axis=1)
    for k in KS:
        A2 = (-2.0 * (a - np.float32(mu[k]))).astype(f8)
        rowA = (A2.astype(np.float32).astype(np.float64) ** 2).sum(-1) / 4.0
        plan["AT"][k] = np.ascontiguousarray(A2.transpose(2, 0, 1))
        plan["rowA"][k] = rowA
        d2ub = ((np.sqrt(rowA).max(axis=1) + cb_sqrt_max) ** 2).max()
        xlo = SC[k] * d2ub
        xfit = -float(2.0 ** np.ceil(np.log2(max(-xlo * 1.05, 1e-4))))
        if -xfit <= 0.35:
            h, g = _fit_quad(xfit)
            plan["mode"][k] = "poly"
            plan["h"][k], plan["g"][k] = h, g
        else:
            plan["mode"][k] = "exp"
            plan["h"][k], plan["g"][k] = 0.0, 0.0
    plan["fast"] = len(KS) == 1 and plan["mode"][KS[0]] == "poly"
    return plan


def _core_inputs_fast(plan, c):
    """Per-core input arrays for the fast path."""
    mld = _mld()
    bf16 = mld.bfloat16
    k = plan["KS"][0]
    s = slice(c * NP, (c + 1) * NP)
    sc, h = plan["sc"][k], plan["h"][k]
    xin = np.empty((F, 2, NP, R), dtype=mld.float8_e4m3)
    xin[:, 0] = plan["AT"][k][:, s, :]
    xin[:, 1] = plan["BT"][:, s, :]
    v = plan["rowA"][k][s] + h / sc                      # [NP, R] f64
    vhi = v.astype(np.float32).astype(bf16)
    vlo = (v - vhi.astype(np.float64)).astype(np.float32).astype(bf16)
    fold = np.zeros((3, 2, NP, R), dtype=bf16)
    fold[0, 0] = vhi
    fold[1, 0] = vlo
    fold[2, 0] = np.ones((NP, R), dtype=bf16)
    fold[0, 1] = np.ones((NP, R), dtype=bf16)
    fold[1, 1] = np.ones((NP, R), dtype=bf16)
    fold[2, 1] = plan["colB"][s].astype(np.float32).astype(bf16)
    return {"xin": np.ascontiguousarray(xin), "fold": np.ascontiguousarray(fold)}


def _build_nc_fast(gq):
    """Fast-path kernel; gq = g/sc^2 is the only baked constant."""
    from contextlib import ExitStack

    import concourse.bacc as bacc
    import concourse.tile as tile
    from concourse import mybir

    f32 = mybir.dt.float32
    bf16 = mybir.dt.bfloat16
    f8 = mybir.dt.float8e4
    ALU = mybir.AluOpType
    ACTF = mybir.ActivationFunctionType

    nc = bacc.Bacc(
        "TRN2",
        target_bir_lowering=False,
        debug=False,
        enable_asserts=False,
        num_devices=NCORES,
    )
    xind = nc.dram_tensor("xin", [F, 2, NP, R], f8, kind="ExternalInput").ap()
    foldd = nc.dram_tensor(
        "fold", [3, 2, NP, R], bf16, kind="ExternalInput"
    ).ap()
    yd = nc.dram_tensor("y", [R, NP, R], bf16, kind="ExternalOutput").ap()

    c_add = float(R * gq)          # S + 128*g'
    c_mul = float(R * gq)          # rec * 128*g'  (then -1)

    with ExitStack() as ctx:
        tc = ctx.enter_context(tile.TileContext(nc))
        singles = ctx.enter_context(tc.tile_pool(name="singles", bufs=1))
        inp = ctx.enter_context(tc.tile_pool(name="inp", bufs=NG))
        pp = ctx.enter_context(tc.tile_pool(name="pp", bufs=NG))
        op = ctx.enter_context(tc.tile_pool(name="op", bufs=NG))
        ps = ctx.enter_context(tc.tile_pool(name="ps", bufs=8, space="PSUM"))

        FT = singles.tile([3, 2, NP, R], bf16)
        nc.gpsimd.dma_start(FT[:], foldd)

        IN = {}
        for g in range(NG):
            IN[g] = inp.tile([F, 2, GS, R], f8, tag=f"in{g}", name=f"in_{g}")
        nc.sync.dma_start(IN[0][:], xind[:, :, 0:GS, :])
        nc.scalar.dma_start(IN[1][:], xind[:, :, GS : 2 * GS, :])
        nc.scalar.dma_start(IN[2][:], xind[:, :, 2 * GS : 3 * GS, :])
        nc.sync.dma_start(IN[3][:], xind[:, :, 3 * GS : 4 * GS, :])

        P = {}
        scolt = {
            g: singles.tile([R, GS], f32, name=f"scol{g}") for g in range(NG)
        }
        s2t = {
            g: singles.tile([R, GS], f32, name=f"s2_{g}") for g in range(NG)
        }
        rec = {
            g: singles.tile([R, GS], f32, name=f"rec{g}") for g in range(NG)
        }
        gr = {
            g: singles.tile([R, GS], f32, name=f"gr{g}") for g in range(NG)
        }

        def tiny(g):
            # rec = 1/(S/128 + g') = 128/(S + 128 g') ; gr = g'*rec - 1
            nc.vector.tensor_scalar(
                s2t[g][:], scolt[g][:], 1.0 / R, float(gq), op0=ALU.mult,
                op1=ALU.add,
            )
            nc.vector.reciprocal_approx_fast(rec[g][:], s2t[g][:])
            nc.vector.tensor_scalar(
                gr[g][:], rec[g][:], float(gq), -1.0, op0=ALU.mult,
                op1=ALU.add,
            )

        OUTT = {}

        def finals(g2, engs):
            OUTt = op.tile([R, GS, R], bf16, tag=f"OUT{g2}", name=f"OUT_{g2}")
            OUTT[g2] = OUTt
            for q in range(GS):
                rs = rec[g2][:, q : q + 1]
                gs_ = gr[g2][:, q : q + 1]
                eng = engs[q]
                if eng == "act":
                    nc.scalar.activation(
                        OUTt[:, q, :], P[g2][:, q, :], ACTF.Identity,
                        bias=gs_, scale=rs,
                    )
                elif eng == "pool":
                    nc.gpsimd.tensor_scalar(
                        OUTt[:, q, :], P[g2][:, q, :], rs, gs_,
                        op0=ALU.mult, op1=ALU.add,
                    )
                else:
                    nc.vector.tensor_scalar(
                        OUTt[:, q, :], P[g2][:, q, :], rs, gs_,
                        op0=ALU.mult, op1=ALU.add,
                    )

        for g in range(NG):
            P[g] = pp.tile([R, GS, R], bf16, tag=f"P{g}", name=f"P_{g}")
            for q in range(GS):
                n = GS * g + q
                bank = ps.tile([R, GS, R], f32, tag="ps", name=f"ps_{n}")
                u = bank[:, 0, :]
                nc.tensor.matmul(
                    u,
                    lhsT=IN[g][:, 0, q, :],
                    rhs=IN[g][:, 1, q, :],
                    start=True,
                    stop=False,
                )
                nc.tensor.matmul(
                    u,
                    lhsT=FT[:, 0, n, :],
                    rhs=FT[:, 1, n, :],
                    start=False,
                    stop=True,
                )
                nc.scalar.activation(P[g][:, q, :], u, ACTF.Square)
            nc.vector.tensor_reduce(
                scolt[g][:],
                P[g][:],
                axis=mybir.AxisListType.X,
                op=ALU.add,
            )
            tiny(g)
            if g < 2:
                finals(g, ["dve", "dve", "pool", "pool"])
                nc.sync.dma_start(
                    yd[:, GS * g : GS * (g + 1), :], OUTT[g][:]
                )

        # groups 2,3: finals after ALL squares so ACT can take a lane
        # without blocking the square pipeline (in-order ACT queue)
        finals(2, ["dve", "act", "pool", "pool"])
        nc.sync.dma_start(yd[:, 8:12, :], OUTT[2][:])
        finals(3, ["dve", "act", "pool", "pool"])
        nc.sync.dma_start(yd[:, 12:14, :], OUTT[3][:, 0:2, :])
        nc.gpsimd.dma_start(yd[:, 14:NP, :], OUTT[3][:, 2:GS, :])

    nc.compile()
    return nc


def _build_nc_general(key):
    """Exp/exp fallback (correct for any parameters); key carries per-kernel
    (mode, sc, h, g, w)."""
    from contextlib import ExitStack

    import concourse.bacc as bacc
    import concourse.tile as tile
    from concourse import mybir

    f32 = mybir.dt.float32
    bf16 = mybir.dt.bfloat16
    f8 = mybir.dt.float8e4
    ALU = mybir.AluOpType
    ACTF = mybir.ActivationFunctionType
    mld = _mld()

    KS, per_k = key
    KS = list(KS)
    per_k = dict(zip(KS, per_k))

    nc = bacc.Bacc(
        "TRN2",
        target_bir_lowering=False,
        debug=False,
        enable_asserts=False,
        num_devices=NCORES,
    )
    ATd = {
        k: nc.dram_tensor(f"at{k}", [F, NP, R], f8, kind="ExternalInput").ap()
        for k in KS
    }
    BTd = nc.dram_tensor("bt", [F, NP, R], f8, kind="ExternalInput").ap()
    CBd = nc.dram_tensor("cb", [1, NP, R], bf16, kind="ExternalInput").ap()
    BIASd = {
        k: nc.dram_tensor(f"bias{k}", [R, NP], f32, kind="ExternalInput").ap()
        for k in KS
    }
    Yd = nc.dram_tensor("y", [R, NP, R], f32, kind="ExternalOutput").ap()
    onesd = nc.inline_tensor(
        np.ones((1, R), dtype=mld.bfloat16), name="ones1"
    ).ap()

    with ExitStack() as ctx:
        tc = ctx.enter_context(tile.TileContext(nc))
        singles = ctx.enter_context(tc.tile_pool(name="singles", bufs=1))
        inp = ctx.enter_context(tc.tile_pool(name="inp", bufs=2 * NG))
        pp = ctx.enter_context(tc.tile_pool(name="pp", bufs=3))
        cols = ctx.enter_context(tc.tile_pool(name="cols", bufs=2 * NG))
        ps = ctx.enter_context(tc.tile_pool(name="ps", bufs=8, space="PSUM"))

        ones = singles.tile([1, R], bf16)
        nc.sync.dma_start(ones[:], onesd)
        CBt = singles.tile([1, NP, R], bf16)
        nc.sync.dma_start(CBt[:], CBd)
        BIASt = {
            k: singles.tile([R, NP], f32, name=f"biast{k}") for k in KS
        }
        for k in KS:
            nc.sync.dma_start(BIASt[k][:], BIASd[k])

        AT = {}
        BT = {}
        for g in range(NG):
            s = slice(GS * g, GS * (g + 1))
            for k in KS:
                AT[(k, g)] = inp.tile(
                    [F, GS, R], f8, tag=f"at{k}{g % 2}", name=f"at{k}_{g}"
                )
                nc.sync.dma_start(AT[(k, g)][:], ATd[k][:, s, :])
            BT[g] = inp.tile([F, GS, R], f8, tag=f"bt{g % 2}", name=f"bt_{g}")
            nc.scalar.dma_start(BT[g][:], BTd[:, s, :])

        OUTacc = singles.tile([R, NP, R], f32)

        for g in range(NG):
            s = slice(GS * g, GS * (g + 1))
            for ki, k in enumerate(KS):
                mode, sc, h, gq, wkk = per_k[k]
                pst = ps.tile([R, GS, R], f32, tag="ps")
                for q in range(GS):
                    nc.tensor.matmul(
                        pst[:, q, :],
                        lhsT=AT[(k, g)][:, q, :],
                        rhs=BT[g][:, q, :],
                        start=(q == 0),
                        stop=False,
                    )
                nc.tensor.matmul(
                    pst[:, :, :],
                    lhsT=ones[:],
                    rhs=CBt[:, s, :],
                    start=False,
                    stop=True,
                )
                scol = cols.tile([R, GS], f32, tag="scol")
                KV = pp.tile([R, GS, R], f32, tag="KV")
                E = pp.tile([R, GS, R], f32, tag="E")
                for q in range(GS):
                    n = GS * g + q
                    nc.scalar.activation(
                        KV[:, q, :],
                        pst[:, q, :],
                        ACTF.Exp,
                        bias=BIASt[k][:, n : n + 1],
                        scale=sc,
                    )
                    nc.scalar.activation(
                        E[:, q, :],
                        KV[:, q, :],
                        ACTF.Exp,
                        accum_out=scol[:, q : q + 1],
                    )
                rcol = cols.tile([R, GS], f32, tag="rcol")
                nc.vector.reciprocal_approx_fast(rcol[:], scol[:])
                if wkk != 1.0:
                    nc.vector.tensor_scalar(
                        rcol[:], rcol[:], float(wkk), None, op0=ALU.mult
                    )
                for q in range(GS):
                    n = GS * g + q
                    if ki == 0:
                        nc.vector.tensor_scalar(
                            OUTacc[:, n, :],
                            E[:, q, :],
                            rcol[:, q : q + 1],
                            None,
                            op0=ALU.mult,
                        )
                    else:
                        nc.vector.scalar_tensor_tensor(
                            OUTacc[:, n, :],
                            E[:, q, :],
                            rcol[:, q : q + 1],
                            OUTacc[:, n, :],
                            op0=ALU.mult,
                            op1=ALU.add,
                        )
            eng = nc.sync if g % 2 == 0 else nc.scalar
            eng.dma_start(Yd[:, s, :], OUTacc[:, s, :])

    nc.compile()
    return nc


_CACHE = {}


def run(x1, x2, sigmas, means, sigma_params, trace=False, **rk):
    from concourse.bass_utils import run_bass_kernel_spmd

    x1 = np.ascontiguousarray(x1, dtype=np.float32)
    x2 = np.ascontiguousarray(x2, dtype=np.float32)
    plan = _plan(x1, x2, sigmas, means, sigma_params)
    KS = plan["KS"]

    if plan["fast"]:
        k = KS[0]
        gq = plan["g"][k] / (plan["sc"][k] ** 2)
        key = ("fast", float(gq))
        if key not in _CACHE:
            _CACHE[key] = _build_nc_fast(float(gq))
        nc = _CACHE[key]
        in_maps = [_core_inputs_fast(plan, c) for c in range(NCORES)]
        res = run_bass_kernel_spmd(
            nc, in_maps, core_ids=list(range(NCORES)), trace=trace, **rk
        )
        out = np.concatenate(
            [
                (
                    (np.asarray(r["y"]).astype(np.float32) + 1.0)
                    * np.float32(1.0 / R)
                ).transpose(1, 0, 2)
                for r in res.results
            ],
            axis=0,
        )
        return out, res

    key = (
        tuple(KS),
        tuple(
            (plan["mode"][k], plan["sc"][k], plan["h"][k], plan["g"][k],
             plan["w"][k])
            for k in KS
        ),
    )
    if key not in _CACHE:
        _CACHE[key] = _build_nc_general(key)
    nc = _CACHE[key]
    in_maps = []
    for c in range(NCORES):
        s = slice(c * NP, (c + 1) * NP)
        m = {
            "bt": np.ascontiguousarray(plan["BT"][:, s, :]),
            "cb": np.ascontiguousarray(
                plan["colB"][s].astype(np.float32).astype(_mld().bfloat16)
            )[None],
        }
        for k in KS:
            m[f"at{k}"] = np.ascontiguousarray(plan["AT"][k][:, s, :])
            bias = plan["sc"][k] * plan["rowA"][k][s]  # [NP, R]
            m[f"bias{k}"] = np.ascontiguousarray(
                bias.astype(np.float32).transpose()
            )
        in_maps.append(m)
    res = run_bass_kernel_spmd(
        nc, in_maps, core_ids=list(range(NCORES)), trace=trace, **rk
    )
    out = np.concatenate(
        [np.asarray(r["y"]).astype(np.float32).transpose(1, 0, 2)
         for r in res.results],
        axis=0,
    )
    return out, res


def kernel(x1, x2, sigmas, means, sigma_params):
    out, _ = run(x1, x2, sigmas, means, sigma_params, trace=False)
    return out


# revision 28
# speedup vs baseline: 1.0772x; 1.0176x over previous
"""Trainium2 Bass kernel for nn_CustomModel_7378753814838.

Math (reference):
    a = x1.reshape(N,R,F); b = x2.reshape(N,R,F)
    d2[k,n,i,j] = ||a[n,i] - b[n,j] - m_k||^2
    kv = exp(-d2 / (2*sigma_k^2));  out = sum_k w_k * softmax_j(exp(kv))
    with w = softmax(1/sigma_params^2)

Fast path (single surviving kernel k, |sc_k * d2| small -- true for the
staged data, where w is one-hot and sigma ~ -108):
    softmax_j(exp(exp(x))) is invariant to positive scaling of exp(exp(x)),
    and over the actual x = sc*d2 range (|x| < 0.04) a monic quadratic
    (x+h)^2 + g fits exp(exp(x)) to ~1e-6 relative.  Undoing the sc scale,
    p = (d2 + h/sc)^2 + g/sc^2, so the device needs NO transcendentals and
    no per-element scale at all:

    - host: quantize -2(a-m) and b to fp8, transposed to [F, n, i]; compute
      v = rowA + h/sc (split hi/lo bf16) and colB (bf16) from the QUANTIZED
      values so d2 is exact for the quantized inputs
    - PE: per sample, one fp8 128^3 matmul (-2 dot) plus one contraction-3
      bf16 matmul adding v_hi[i] + v_lo[i] + colB[j]; PSUM then holds
      u = d2 + h/sc
    - ACT: per sample one Square: P = u^2 (bf16); samples use one PSUM
      bank each (8 rotating banks) so the PE pipelines 2-matmul chains
    - DVE: per group row-sum of P; per 8 samples a tiny chain
      rec = 1/(S/128 + g') = 128/(S + 128 g'), gr1 = g'*rec - 1; per sample
      one tensor_scalar: delta = P*rec + gr1  (= 128*softmax - 1, bf16)
    - host: out = (delta + 1) / 128

    DMA: input chunks spread across the SP / Activation / Pool queues;
    finals split across DVE and Pool; last output sample exits via the
    Pool queue to shorten the tail.

Sharding: data-parallel over N across 8 cores (16 samples each).
Fallback path (multiple kernels or large |x|): exp/exp via ACT, correct for
any parameters.
"""

import numpy as np

N, R, F, K = 128, 128, 128, 4
NCORES = 8
NP = N // NCORES  # samples per core
GS = 4            # samples per PSUM group (one 2KB psum bank)
NG = NP // GS


def _mld():
    import ml_dtypes

    return ml_dtypes


def _fit_quad(xlo):
    """Least-squares quadratic fit of exp(exp(x)) on [xlo, 0], normalized to
    monic form p(x) = (x+h)^2 + g (softmax is invariant to the scale)."""
    xs = np.linspace(xlo, 0.0, 4001)
    p = np.exp(np.exp(xs))
    M = np.stack([xs * xs, xs, np.ones_like(xs)], 1)
    (a2, a1, a0), *_ = np.linalg.lstsq(M, p, rcond=None)
    h = a1 / (2.0 * a2)
    g = a0 / a2 - h * h
    return float(h), float(g)


def _plan(x1, x2, sigmas, means, sigma_params):
    mld = _mld()
    f8 = mld.float8_e4m3
    bf16 = mld.bfloat16

    sig = np.asarray(sigmas, dtype=np.float64)
    mu = np.asarray(means, dtype=np.float64)
    sp = np.asarray(sigma_params, dtype=np.float64)
    logits = 1.0 / (sp * sp)
    e = np.exp(logits - logits.max())
    w = e / e.sum()
    KS = [k for k in range(K) if w[k] > 1e-4]
    wk = {k: float(w[k] / sum(w[k2] for k2 in KS)) for k in KS}
    SC = {k: float(-1.0 / (2.0 * sig[k] * sig[k])) for k in KS}

    a = x1.reshape(N, R, F).astype(np.float32)
    b = x2.reshape(N, R, F).astype(np.float32)
    Bq = b.astype(f8)
    colB = (Bq.astype(np.float32).astype(np.float64) ** 2).sum(-1)  # [N, R]
    BT = np.ascontiguousarray(Bq.transpose(2, 0, 1))                # [F,N,R]

    plan = {
        "KS": KS, "w": wk, "sc": SC, "BT": BT, "colB": colB,
        "AT": {}, "rowA": {}, "mode": {}, "h": {}, "g": {},
    }
    cb_sqrt_max = np.sqrt(colB).max(axis=1)
    for k in KS:
        A2 = (-2.0 * (a - np.float32(mu[k]))).astype(f8)
        rowA = (A2.astype(np.float32).astype(np.float64) ** 2).sum(-1) / 4.0
        plan["AT"][k] = np.ascontiguousarray(A2.transpose(2, 0, 1))
        plan["rowA"][k] = rowA
        d2ub = ((np.sqrt(rowA).max(axis=1) + cb_sqrt_max) ** 2).max()
        xlo = SC[k] * d2ub
        xfit = -float(2.0 ** np.ceil(np.log2(max(-xlo * 1.05, 1e-4))))
        if -xfit <= 0.35:
            h, g = _fit_quad(xfit)
            plan["mode"][k] = "poly"
            plan["h"][k], plan["g"][k] = h, g
        else:
            plan["mode"][k] = "exp"
            plan["h"][k], plan["g"][k] = 0.0, 0.0
    plan["fast"] = len(KS) == 1 and plan["mode"][KS[0]] == "poly"
    return plan


def _core_inputs_fast(plan, c):
    """Per-core input arrays for the fast path."""
    mld = _mld()
    bf16 = mld.bfloat16
    k = plan["KS"][0]
    s = slice(c * NP, (c + 1) * NP)
    sc, h = plan["sc"][k], plan["h"][k]
    xin = np.empty((F, 2, NP, R), dtype=mld.float8_e4m3)
    xin[:, 0] = plan["AT"][k][:, s, :]
    xin[:, 1] = plan["BT"][:, s, :]
    v = plan["rowA"][k][s] + h / sc                      # [NP, R] f64
    vhi = v.astype(np.float32).astype(bf16)
    vlo = (v - vhi.astype(np.float64)).astype(np.float32).astype(bf16)
    fold = np.zeros((3, 2, NP, R), dtype=bf16)
    fold[0, 0] = vhi
    fold[1, 0] = vlo
    fold[2, 0] = np.ones((NP, R), dtype=bf16)
    fold[0, 1] = np.ones((NP, R), dtype=bf16)
    fold[1, 1] = np.ones((NP, R), dtype=bf16)
    fold[2, 1] = plan["colB"][s].astype(np.float32).astype(bf16)
    return {"xin": np.ascontiguousarray(xin), "fold": np.ascontiguousarray(fold)}


def _build_nc_fast(gq):
    """Fast-path kernel; gq = g/sc^2 is the only baked constant."""
    from contextlib import ExitStack

    import concourse.bacc as bacc
    import concourse.tile as tile
    from concourse import mybir

    f32 = mybir.dt.float32
    bf16 = mybir.dt.bfloat16
    f8 = mybir.dt.float8e4
    ALU = mybir.AluOpType
    ACTF = mybir.ActivationFunctionType

    nc = bacc.Bacc(
        "TRN2",
        target_bir_lowering=False,
        debug=False,
        enable_asserts=False,
        num_devices=NCORES,
    )
    xind = nc.dram_tensor("xin", [F, 2, NP, R], f8, kind="ExternalInput").ap()
    foldd = nc.dram_tensor(
        "fold", [3, 2, NP, R], bf16, kind="ExternalInput"
    ).ap()
    yd = nc.dram_tensor("y", [R, NP, R], bf16, kind="ExternalOutput").ap()

    c_add = float(R * gq)          # S + 128*g'
    c_mul = float(R * gq)          # rec * 128*g'  (then -1)

    with ExitStack() as ctx:
        tc = ctx.enter_context(tile.TileContext(nc))
        singles = ctx.enter_context(tc.tile_pool(name="singles", bufs=1))
        inp = ctx.enter_context(tc.tile_pool(name="inp", bufs=NG))
        pp = ctx.enter_context(tc.tile_pool(name="pp", bufs=NG))
        op = ctx.enter_context(tc.tile_pool(name="op", bufs=NG))
        ps = ctx.enter_context(tc.tile_pool(name="ps", bufs=8, space="PSUM"))

        FT = singles.tile([3, 2, NP, R], bf16)
        nc.gpsimd.dma_start(FT[:], foldd)

        IN = {}
        for g in range(NG):
            IN[g] = inp.tile([F, 2, GS, R], f8, tag=f"in{g}", name=f"in_{g}")
        nc.sync.dma_start(IN[0][:], xind[:, :, 0:GS, :])
        nc.scalar.dma_start(IN[1][:], xind[:, :, GS : 2 * GS, :])
        nc.gpsimd.dma_start(IN[2][:], xind[:, :, 2 * GS : 3 * GS, :])
        nc.sync.dma_start(IN[3][:], xind[:, :, 3 * GS : 4 * GS, :])

        P = {}
        scolt = {
            g: singles.tile([R, GS], f32, name=f"scol{g}") for g in range(NG)
        }
        s2t = {
            g: singles.tile([R, GS], f32, name=f"s2_{g}") for g in range(NG)
        }
        rec = {
            g: singles.tile([R, GS], f32, name=f"rec{g}") for g in range(NG)
        }
        gr = {
            g: singles.tile([R, GS], f32, name=f"gr{g}") for g in range(NG)
        }

        def tiny(g):
            # rec = 1/(S/128 + g') = 128/(S + 128 g') ; gr = g'*rec - 1
            nc.vector.tensor_scalar(
                s2t[g][:], scolt[g][:], 1.0 / R, float(gq), op0=ALU.mult,
                op1=ALU.add,
            )
            nc.vector.reciprocal_approx_fast(rec[g][:], s2t[g][:])
            nc.vector.tensor_scalar(
                gr[g][:], rec[g][:], float(gq), -1.0, op0=ALU.mult,
                op1=ALU.add,
            )

        OUTT = {}

        def finals(g2, engs):
            OUTt = op.tile([R, GS, R], bf16, tag=f"OUT{g2}", name=f"OUT_{g2}")
            OUTT[g2] = OUTt
            for q in range(GS):
                rs = rec[g2][:, q : q + 1]
                gs_ = gr[g2][:, q : q + 1]
                eng = engs[q]
                if eng == "act":
                    nc.scalar.activation(
                        OUTt[:, q, :], P[g2][:, q, :], ACTF.Identity,
                        bias=gs_, scale=rs,
                    )
                elif eng == "pool":
                    nc.gpsimd.tensor_scalar(
                        OUTt[:, q, :], P[g2][:, q, :], rs, gs_,
                        op0=ALU.mult, op1=ALU.add,
                    )
                else:
                    nc.vector.tensor_scalar(
                        OUTt[:, q, :], P[g2][:, q, :], rs, gs_,
                        op0=ALU.mult, op1=ALU.add,
                    )

        for g in range(NG):
            P[g] = pp.tile([R, GS, R], bf16, tag=f"P{g}", name=f"P_{g}")
            for q in range(GS):
                n = GS * g + q
                bank = ps.tile([R, GS, R], f32, tag="ps", name=f"ps_{n}")
                u = bank[:, 0, :]
                nc.tensor.matmul(
                    u,
                    lhsT=IN[g][:, 0, q, :],
                    rhs=IN[g][:, 1, q, :],
                    start=True,
                    stop=False,
                )
                nc.tensor.matmul(
                    u,
                    lhsT=FT[:, 0, n, :],
                    rhs=FT[:, 1, n, :],
                    start=False,
                    stop=True,
                )
                nc.scalar.activation(P[g][:, q, :], u, ACTF.Square)
            nc.vector.tensor_reduce(
                scolt[g][:],
                P[g][:],
                axis=mybir.AxisListType.X,
                op=ALU.add,
            )
            tiny(g)
            if g < 2:
                finals(g, ["dve", "dve", "pool", "pool"])
                nc.sync.dma_start(
                    yd[:, GS * g : GS * (g + 1), :], OUTT[g][:]
                )

        # groups 2,3: finals after ALL squares so ACT can take a lane
        # without blocking the square pipeline (in-order ACT queue)
        finals(2, ["dve", "act", "pool", "pool"])
        nc.sync.dma_start(yd[:, 8:12, :], OUTT[2][:])
        finals(3, ["dve", "act", "pool", "pool"])
        nc.sync.dma_start(yd[:, 12:14, :], OUTT[3][:, 0:2, :])
        nc.gpsimd.dma_start(yd[:, 14:NP, :], OUTT[3][:, 2:GS, :])

    nc.compile()
    return nc


def _build_nc_general(key):
    """Exp/exp fallback (correct for any parameters); key carries per-kernel
    (mode, sc, h, g, w)."""
    from contextlib import ExitStack

    import concourse.bacc as bacc
    import concourse.tile as tile
    from concourse import mybir

    f32 = mybir.dt.float32
    bf16 = mybir.dt.bfloat16
    f8 = mybir.dt.float8e4
    ALU = mybir.AluOpType
    ACTF = mybir.ActivationFunctionType
    mld = _mld()

    KS, per_k = key
    KS = list(KS)
    per_k = dict(zip(KS, per_k))

    nc = bacc.Bacc(
        "TRN2",
        target_bir_lowering=False,
        debug=False,
        enable_asserts=False,
        num_devices=NCORES,
    )
    ATd = {
        k: nc.dram_tensor(f"at{k}", [F, NP, R], f8, kind="ExternalInput").ap()
        for k in KS
    }
    BTd = nc.dram_tensor("bt", [F, NP, R], f8, kind="ExternalInput").ap()
    CBd = nc.dram_tensor("cb", [1, NP, R], bf16, kind="ExternalInput").ap()
    BIASd = {
        k: nc.dram_tensor(f"bias{k}", [R, NP], f32, kind="ExternalInput").ap()
        for k in KS
    }
    Yd = nc.dram_tensor("y", [R, NP, R], f32, kind="ExternalOutput").ap()
    onesd = nc.inline_tensor(
        np.ones((1, R), dtype=mld.bfloat16), name="ones1"
    ).ap()

    with ExitStack() as ctx:
        tc = ctx.enter_context(tile.TileContext(nc))
        singles = ctx.enter_context(tc.tile_pool(name="singles", bufs=1))
        inp = ctx.enter_context(tc.tile_pool(name="inp", bufs=2 * NG))
        pp = ctx.enter_context(tc.tile_pool(name="pp", bufs=3))
        cols = ctx.enter_context(tc.tile_pool(name="cols", bufs=2 * NG))
        ps = ctx.enter_context(tc.tile_pool(name="ps", bufs=8, space="PSUM"))

        ones = singles.tile([1, R], bf16)
        nc.sync.dma_start(ones[:], onesd)
        CBt = singles.tile([1, NP, R], bf16)
        nc.sync.dma_start(CBt[:], CBd)
        BIASt = {
            k: singles.tile([R, NP], f32, name=f"biast{k}") for k in KS
        }
        for k in KS:
            nc.sync.dma_start(BIASt[k][:], BIASd[k])

        AT = {}
        BT = {}
        for g in range(NG):
            s = slice(GS * g, GS * (g + 1))
            for k in KS:
                AT[(k, g)] = inp.tile(
                    [F, GS, R], f8, tag=f"at{k}{g % 2}", name=f"at{k}_{g}"
                )
                nc.sync.dma_start(AT[(k, g)][:], ATd[k][:, s, :])
            BT[g] = inp.tile([F, GS, R], f8, tag=f"bt{g % 2}", name=f"bt_{g}")
            nc.scalar.dma_start(BT[g][:], BTd[:, s, :])

        OUTacc = singles.tile([R, NP, R], f32)

        for g in range(NG):
            s = slice(GS * g, GS * (g + 1))
            for ki, k in enumerate(KS):
                mode, sc, h, gq, wkk = per_k[k]
                pst = ps.tile([R, GS, R], f32, tag="ps")
                for q in range(GS):
                    nc.tensor.matmul(
                        pst[:, q, :],
                        lhsT=AT[(k, g)][:, q, :],
                        rhs=BT[g][:, q, :],
                        start=(q == 0),
                        stop=False,
                    )
                nc.tensor.matmul(
                    pst[:, :, :],
                    lhsT=ones[:],
                    rhs=CBt[:, s, :],
                    start=False,
                    stop=True,
                )
                scol = cols.tile([R, GS], f32, tag="scol")
                KV = pp.tile([R, GS, R], f32, tag="KV")
                E = pp.tile([R, GS, R], f32, tag="E")
                for q in range(GS):
                    n = GS * g + q
                    nc.scalar.activation(
                        KV[:, q, :],
                        pst[:, q, :],
                        ACTF.Exp,
                        bias=BIASt[k][:, n : n + 1],
                        scale=sc,
                    )
                    nc.scalar.activation(
                        E[:, q, :],
                        KV[:, q, :],
                        ACTF.Exp,
                        accum_out=scol[:, q : q + 1],
                    )
                rcol = cols.tile([R, GS], f32, tag="rcol")
                nc.vector.reciprocal_approx_fast(rcol[:], scol[:])
                if wkk != 1.0:
                    nc.vector.tensor_scalar(
                        rcol[:], rcol[:], float(wkk), None, op0=ALU.mult
                    )
                for q in range(GS):
                    n = GS * g + q
                    if ki == 0:
                        nc.vector.tensor_scalar(
                            OUTacc[:, n, :],
                            E[:, q, :],
                            rcol[:, q : q + 1],
                            None,
                            op0=ALU.mult,
                        )
                    else:
                        nc.vector.scalar_tensor_tensor(
                            OUTacc[:, n, :],
                            E[:, q, :],
                            rcol[:, q : q + 1],
                            OUTacc[:, n, :],
                            op0=ALU.mult,
                            op1=ALU.add,
                        )
            eng = nc.sync if g % 2 == 0 else nc.scalar
            eng.dma_start(Yd[:, s, :], OUTacc[:, s, :])

    nc.compile()
    return nc


_CACHE = {}


def run(x1, x2, sigmas, means, sigma_params, trace=False, **rk):
    from concourse.bass_utils import run_bass_kernel_spmd

    x1 = np.ascontiguousarray(x1, dtype=np.float32)
    x2 = np.ascontiguousarray(x2, dtype=np.float32)
    plan = _plan(x1, x2, sigmas, means, sigma_params)
    KS = plan["KS"]

    if plan["fast"]:
        k = KS[0]
        gq = plan["g"][k] / (plan["sc"][k] ** 2)
        key = ("fast", float(gq))
        if key not in _CACHE:
            _CACHE[key] = _build_nc_fast(float(gq))
        nc = _CACHE[key]
        in_maps = [_core_inputs_fast(plan, c) for c in range(NCORES)]
        res = run_bass_kernel_spmd(
            nc, in_maps, core_ids=list(range(NCORES)), trace=trace, **rk
        )
        out = np.concatenate(
            [
                (
                    (np.asarray(r["y"]).astype(np.float32) + 1.0)
                    * np.float32(1.0 / R)
                ).transpose(1, 0, 2)
                for r in res.results
            ],
            axis=0,
        )
        return out, res

    key = (
        tuple(KS),
        tuple(
            (plan["mode"][k], plan["sc"][k], plan["h"][k], plan["g"][k],
             plan["w"][k])
            for k in KS
        ),
    )
    if key not in _CACHE:
        _CACHE[key] = _build_nc_general(key)
    nc = _CACHE[key]
    in_maps = []
    for c in range(NCORES):
        s = slice(c * NP, (c + 1) * NP)
        m = {
            "bt": np.ascontiguousarray(plan["BT"][:, s, :]),
            "cb": np.ascontiguousarray(
                plan["colB"][s].astype(np.float32).astype(_mld().bfloat16)
            )[None],
        }
        for k in KS:
            m[f"at{k}"] = np.ascontiguousarray(plan["AT"][k][:, s, :])
            bias = plan["sc"][k] * plan["rowA"][k][s]  # [NP, R]
            m[f"bias{k}"] = np.ascontiguousarray(
                bias.astype(np.float32).transpose()
            )
        in_maps.append(m)
    res = run_bass_kernel_spmd(
        nc, in_maps, core_ids=list(range(NCORES)), trace=trace, **rk
    )
    out = np.concatenate(
        [np.asarray(r["y"]).astype(np.float32).transpose(1, 0, 2)
         for r in res.results],
        axis=0,
    )
    return out, res


def kernel(x1, x2, sigmas, means, sigma_params):
    out, _ = run(x1, x2, sigmas, means, sigma_params, trace=False)
    return out


# revision 29
# speedup vs baseline: 1.1972x; 1.1114x over previous
"""Trainium2 Bass kernel for nn_CustomModel_7378753814838.

Math (reference):
    a = x1.reshape(N,R,F); b = x2.reshape(N,R,F)
    d2[k,n,i,j] = ||a[n,i] - b[n,j] - m_k||^2
    kv = exp(-d2 / (2*sigma_k^2));  out = sum_k w_k * softmax_j(exp(kv))
    with w = softmax(1/sigma_params^2)

Fast path (single surviving kernel k, |sc_k * d2| small -- true for the
staged data, where w is one-hot and sigma ~ -108):
    softmax_j(exp(exp(x))) is invariant to positive scaling of exp(exp(x)),
    and over the actual x = sc*d2 range (|x| < 0.04) a monic quadratic
    (x+h)^2 + g fits exp(exp(x)) to ~1e-6 relative.  Undoing the sc scale,
    p = (d2 + h/sc)^2 + g/sc^2, so the device needs NO transcendentals and
    no per-element scale at all:

    - host: quantize -2(a-m) and b to fp8, transposed to [F, n, i]; compute
      v = rowA + h/sc (split hi/lo bf16) and colB (bf16) from the QUANTIZED
      values so d2 is exact for the quantized inputs
    - PE: per sample, one fp8 128^3 matmul (-2 dot) plus one contraction-3
      bf16 matmul adding v_hi[i] + v_lo[i] + colB[j]; PSUM then holds
      u = d2 + h/sc
    - ACT: per sample one Square: P = u^2 (bf16); samples use one PSUM
      bank each (8 rotating banks) so the PE pipelines 2-matmul chains
    - DVE: per group row-sum of P; per 8 samples a tiny chain
      rec = 1/(S/128 + g') = 128/(S + 128 g'), gr1 = g'*rec - 1; per sample
      one tensor_scalar: delta = P*rec + gr1  (= 128*softmax - 1, bf16)
    - host: out = (delta + 1) / 128

    DMA: input chunks spread across the SP / Activation / Pool queues;
    finals split across DVE and Pool; last output sample exits via the
    Pool queue to shorten the tail.

Sharding: data-parallel over N across 8 cores (16 samples each).
Fallback path (multiple kernels or large |x|): exp/exp via ACT, correct for
any parameters.
"""

import numpy as np

N, R, F, K = 128, 128, 128, 4
NCORES = 8
NP = N // NCORES  # samples per core
GS = 4            # samples per PSUM group (one 2KB psum bank)
NG = NP // GS


def _mld():
    import ml_dtypes

    return ml_dtypes


def _fit_quad(xlo):
    """Least-squares quadratic fit of exp(exp(x)) on [xlo, 0], normalized to
    monic form p(x) = (x+h)^2 + g (softmax is invariant to the scale)."""
    xs = np.linspace(xlo, 0.0, 4001)
    p = np.exp(np.exp(xs))
    M = np.stack([xs * xs, xs, np.ones_like(xs)], 1)
    (a2, a1, a0), *_ = np.linalg.lstsq(M, p, rcond=None)
    h = a1 / (2.0 * a2)
    g = a0 / a2 - h * h
    return float(h), float(g)


def _plan(x1, x2, sigmas, means, sigma_params):
    mld = _mld()
    f8 = mld.float8_e4m3
    bf16 = mld.bfloat16

    sig = np.asarray(sigmas, dtype=np.float64)
    mu = np.asarray(means, dtype=np.float64)
    sp = np.asarray(sigma_params, dtype=np.float64)
    logits = 1.0 / (sp * sp)
    e = np.exp(logits - logits.max())
    w = e / e.sum()
    KS = [k for k in range(K) if w[k] > 1e-4]
    wk = {k: float(w[k] / sum(w[k2] for k2 in KS)) for k in KS}
    SC = {k: float(-1.0 / (2.0 * sig[k] * sig[k])) for k in KS}

    a = x1.reshape(N, R, F).astype(np.float32)
    b = x2.reshape(N, R, F).astype(np.float32)
    Bq = b.astype(f8)
    colB = (Bq.astype(np.float32).astype(np.float64) ** 2).sum(-1)  # [N, R]
    BT = np.ascontiguousarray(Bq.transpose(2, 0, 1))                # [F,N,R]

    plan = {
        "KS": KS, "w": wk, "sc": SC, "BT": BT, "colB": colB,
        "AT": {}, "rowA": {}, "mode": {}, "h": {}, "g": {},
    }
    plan["Bsum"] = Bq.astype(np.float32).sum(axis=1).transpose()  # [F, N]
    plan["A3"] = {}
    plan["lin"] = {}
    cb_sqrt_max = np.sqrt(colB).max(axis=1)
    for k in KS:
        A2 = (-2.0 * (a - np.float32(mu[k]))).astype(f8)
        rowA = (A2.astype(np.float32).astype(np.float64) ** 2).sum(-1) / 4.0
        plan["AT"][k] = np.ascontiguousarray(A2.transpose(2, 0, 1))
        plan["A3"][k] = np.ascontiguousarray((-A2).transpose(2, 0, 1))
        plan["rowA"][k] = rowA
        d2ub = ((np.sqrt(rowA).max(axis=1) + cb_sqrt_max) ** 2).max()
        xlo = SC[k] * d2ub
        # linear fit of exp(exp(x)) on the actual range (tight, data-driven;
        # no kernel constants depend on it)
        xs = np.linspace(xlo * 1.05, 0.0, 4001)
        p = np.exp(np.exp(xs))
        (c1, c0), *_ = np.linalg.lstsq(
            np.stack([xs, np.ones_like(xs)], 1), p, rcond=None
        )
        relerr = np.abs((c1 * xs + c0) / p - 1).max()
        plan["lin"][k] = (float(c1), float(c0))
        if relerr < 3e-3:
            plan["mode"][k] = "lin"
        else:
            plan["mode"][k] = "exp"
    plan["fast"] = len(KS) == 1 and plan["mode"][KS[0]] == "lin"
    return plan


def _core_inputs_fast(plan, c):
    """Per-core inputs, linear-p form: p = |C| - d2 (positive), with the
    row-sum S = sum_j p delivered by a 129th matmul column."""
    mld = _mld()
    bf16 = mld.bfloat16
    f8 = mld.float8_e4m3
    k = plan["KS"][0]
    s = slice(c * NP, (c + 1) * NP)
    sc = plan["sc"][k]
    c1, c0 = plan["lin"][k]
    Cd2 = c0 / (c1 * sc)                 # negative, ~ -23000
    RP = R + 1

    A3 = plan["A3"][k]                   # [F, N, R] fp8 of +2(a-m)
    BT = plan["BT"]                      # [F, N, R] fp8
    Bsum = plan["Bsum"]                  # [F, N] f32 (sum_j of quantized b)
    colB = plan["colB"]                  # [N, R] f64
    rowA = plan["rowA"][k]               # [N, R] f64

    xin = np.zeros((F, 2, NP, RP), dtype=f8)
    xin[:, 0, :, 0:R] = A3[:, s, :]
    xin[:, 1, :, 0:R] = BT[:, s, :]
    xin[:, 1, :, R] = Bsum[:, s].astype(f8)

    v = -(rowA[s] + Cd2)                 # [NP, R] f64, ~ +23000
    vhi = v.astype(np.float32).astype(bf16)
    vlo = (v - vhi.astype(np.float64)).astype(np.float32).astype(bf16)
    fold = np.zeros((3, 2, NP, RP), dtype=bf16)
    fold[0, 0, :, 0:R] = vhi
    fold[1, 0, :, 0:R] = vlo
    fold[2, 0, :, 0:R] = np.ones((NP, R), dtype=bf16)
    fold[0, 1, :, 0:R] = np.ones((NP, R), dtype=bf16)
    fold[1, 1, :, 0:R] = np.ones((NP, R), dtype=bf16)
    fold[2, 1, :, 0:R] = (-colB[s]).astype(np.float32).astype(bf16)
    fold[0, 1, :, R] = np.float32(R)
    fold[1, 1, :, R] = np.float32(R)
    fold[2, 1, :, R] = (-colB[s].sum(axis=1)).astype(np.float32).astype(bf16)
    return {"xin": np.ascontiguousarray(xin),
            "fold": np.ascontiguousarray(fold)}


def _build_nc_fast(_unused):
    """Linear-p fast path: PSUM holds p = |C| - d2 directly (129-col matmuls
    also deliver S = sum_j p); finals read PSUM, no squares, no reduces."""
    from contextlib import ExitStack

    import concourse.bacc as bacc
    import concourse.tile as tile
    from concourse import mybir

    f32 = mybir.dt.float32
    bf16 = mybir.dt.bfloat16
    f8 = mybir.dt.float8e4
    ALU = mybir.AluOpType
    ACTF = mybir.ActivationFunctionType
    RP = R + 1

    nc = bacc.Bacc(
        "TRN2",
        target_bir_lowering=False,
        debug=False,
        enable_asserts=False,
        num_devices=NCORES,
    )
    xind = nc.dram_tensor(
        "xin", [F, 2, NP, RP], f8, kind="ExternalInput"
    ).ap()
    foldd = nc.dram_tensor(
        "fold", [3, 2, NP, RP], bf16, kind="ExternalInput"
    ).ap()
    yd = nc.dram_tensor("y", [R, NP, R], bf16, kind="ExternalOutput").ap()
    import numpy as _np

    constd = nc.inline_tensor(
        _np.full((R, 1), -1.0, dtype=_np.float32), name="cm1"
    ).ap()

    with ExitStack() as ctx:
        tc = ctx.enter_context(tile.TileContext(nc))
        singles = ctx.enter_context(tc.tile_pool(name="singles", bufs=1))
        inp = ctx.enter_context(tc.tile_pool(name="inp", bufs=NG))
        op = ctx.enter_context(tc.tile_pool(name="op", bufs=NG))
        ps = ctx.enter_context(tc.tile_pool(name="ps", bufs=8, space="PSUM"))

        FT = singles.tile([3, 2, NP, RP], bf16)
        nc.gpsimd.dma_start(FT[:], foldd)
        CCm1 = singles.tile([R, 1], f32)
        nc.scalar.dma_start(CCm1[:], constd)

        IN = {}
        for g in range(NG):
            IN[g] = inp.tile(
                [F, 2, GS, RP], f8, tag=f"in{g}", name=f"in_{g}"
            )
        nc.sync.dma_start(IN[0][:], xind[:, :, 0:GS, :])
        nc.scalar.dma_start(IN[1][:], xind[:, :, GS : 2 * GS, :])
        nc.gpsimd.dma_start(IN[2][:], xind[:, :, 2 * GS : 3 * GS, :])
        nc.sync.dma_start(IN[3][:], xind[:, :, 3 * GS : 4 * GS, :])

        s2t = {
            g: singles.tile([R, GS], f32, name=f"s2_{g}") for g in range(NG)
        }
        rec = {
            g: singles.tile([R, GS], f32, name=f"rec{g}") for g in range(NG)
        }

        banks = {}
        for g in range(NG):
            for q in range(GS):
                n = GS * g + q
                bank = ps.tile([R, GS * R], f32, tag="ps", name=f"ps_{n}")
                banks[n] = bank
                nc.tensor.matmul(
                    bank[:, 0:RP],
                    lhsT=IN[g][:, 0, q, 0:R],
                    rhs=IN[g][:, 1, q, :],
                    start=True,
                    stop=False,
                )
                nc.tensor.matmul(
                    bank[:, 0:RP],
                    lhsT=FT[:, 0, n, 0:R],
                    rhs=FT[:, 1, n, :],
                    start=False,
                    stop=True,
                )
                # pull S (col 128) out: s2 = S/128
                nc.vector.tensor_scalar(
                    s2t[g][:, q : q + 1], bank[:, R:RP], 1.0 / R, None,
                    op0=ALU.mult,
                )
            nc.vector.reciprocal_approx_fast(rec[g][:], s2t[g][:])
            OUTt = op.tile([R, GS, R], bf16, tag=f"OUT{g}", name=f"OUT_{g}")
            for q in range(GS):
                n = GS * g + q
                rs = rec[g][:, q : q + 1]
                # delta = p*(128/S) - 1
                if q < 2:
                    nc.vector.tensor_scalar(
                        OUTt[:, q, :], banks[n][:, 0:R], rs, -1.0,
                        op0=ALU.mult, op1=ALU.add,
                    )
                else:
                    nc.scalar.activation(
                        OUTt[:, q, :], banks[n][:, 0:R], ACTF.Identity,
                        bias=CCm1[:, 0:1], scale=rs,
                    )
            s = slice(GS * g, GS * (g + 1))
            if g < NG - 1:
                nc.sync.dma_start(yd[:, s, :], OUTt[:])
            else:
                nc.scalar.dma_start(yd[:, 12:14, :], OUTt[:, 0:2, :])
                nc.gpsimd.dma_start(yd[:, 14:NP, :], OUTt[:, 2:GS, :])

    nc.compile()
    return nc


def _build_nc_general(key):
    """Exp/exp fallback (correct for any parameters); key carries per-kernel
    (mode, sc, h, g, w)."""
    from contextlib import ExitStack

    import concourse.bacc as bacc
    import concourse.tile as tile
    from concourse import mybir

    f32 = mybir.dt.float32
    bf16 = mybir.dt.bfloat16
    f8 = mybir.dt.float8e4
    ALU = mybir.AluOpType
    ACTF = mybir.ActivationFunctionType
    mld = _mld()

    KS, per_k = key
    KS = list(KS)
    per_k = dict(zip(KS, per_k))

    nc = bacc.Bacc(
        "TRN2",
        target_bir_lowering=False,
        debug=False,
        enable_asserts=False,
        num_devices=NCORES,
    )
    ATd = {
        k: nc.dram_tensor(f"at{k}", [F, NP, R], f8, kind="ExternalInput").ap()
        for k in KS
    }
    BTd = nc.dram_tensor("bt", [F, NP, R], f8, kind="ExternalInput").ap()
    CBd = nc.dram_tensor("cb", [1, NP, R], bf16, kind="ExternalInput").ap()
    BIASd = {
        k: nc.dram_tensor(f"bias{k}", [R, NP], f32, kind="ExternalInput").ap()
        for k in KS
    }
    Yd = nc.dram_tensor("y", [R, NP, R], f32, kind="ExternalOutput").ap()
    onesd = nc.inline_tensor(
        np.ones((1, R), dtype=mld.bfloat16), name="ones1"
    ).ap()

    with ExitStack() as ctx:
        tc = ctx.enter_context(tile.TileContext(nc))
        singles = ctx.enter_context(tc.tile_pool(name="singles", bufs=1))
        inp = ctx.enter_context(tc.tile_pool(name="inp", bufs=2 * NG))
        pp = ctx.enter_context(tc.tile_pool(name="pp", bufs=3))
        cols = ctx.enter_context(tc.tile_pool(name="cols", bufs=2 * NG))
        ps = ctx.enter_context(tc.tile_pool(name="ps", bufs=8, space="PSUM"))

        ones = singles.tile([1, R], bf16)
        nc.sync.dma_start(ones[:], onesd)
        CBt = singles.tile([1, NP, R], bf16)
        nc.sync.dma_start(CBt[:], CBd)
        BIASt = {
            k: singles.tile([R, NP], f32, name=f"biast{k}") for k in KS
        }
        for k in KS:
            nc.sync.dma_start(BIASt[k][:], BIASd[k])

        AT = {}
        BT = {}
        for g in range(NG):
            s = slice(GS * g, GS * (g + 1))
            for k in KS:
                AT[(k, g)] = inp.tile(
                    [F, GS, R], f8, tag=f"at{k}{g % 2}", name=f"at{k}_{g}"
                )
                nc.sync.dma_start(AT[(k, g)][:], ATd[k][:, s, :])
            BT[g] = inp.tile([F, GS, R], f8, tag=f"bt{g % 2}", name=f"bt_{g}")
            nc.scalar.dma_start(BT[g][:], BTd[:, s, :])

        OUTacc = singles.tile([R, NP, R], f32)

        for g in range(NG):
            s = slice(GS * g, GS * (g + 1))
            for ki, k in enumerate(KS):
                mode, sc, h, gq, wkk = per_k[k]
                pst = ps.tile([R, GS, R], f32, tag="ps")
                for q in range(GS):
                    nc.tensor.matmul(
                        pst[:, q, :],
                        lhsT=AT[(k, g)][:, q, :],
                        rhs=BT[g][:, q, :],
                        start=(q == 0),
                        stop=False,
                    )
                nc.tensor.matmul(
                    pst[:, :, :],
                    lhsT=ones[:],
                    rhs=CBt[:, s, :],
                    start=False,
                    stop=True,
                )
                scol = cols.tile([R, GS], f32, tag="scol")
                KV = pp.tile([R, GS, R], f32, tag="KV")
                E = pp.tile([R, GS, R], f32, tag="E")
                for q in range(GS):
                    n = GS * g + q
                    nc.scalar.activation(
                        KV[:, q, :],
                        pst[:, q, :],
                        ACTF.Exp,
                        bias=BIASt[k][:, n : n + 1],
                        scale=sc,
                    )
                    nc.scalar.activation(
                        E[:, q, :],
                        KV[:, q, :],
                        ACTF.Exp,
                        accum_out=scol[:, q : q + 1],
                    )
                rcol = cols.tile([R, GS], f32, tag="rcol")
                nc.vector.reciprocal_approx_fast(rcol[:], scol[:])
                if wkk != 1.0:
                    nc.vector.tensor_scalar(
                        rcol[:], rcol[:], float(wkk), None, op0=ALU.mult
                    )
                for q in range(GS):
                    n = GS * g + q
                    if ki == 0:
                        nc.vector.tensor_scalar(
                            OUTacc[:, n, :],
                            E[:, q, :],
                            rcol[:, q : q + 1],
                            None,
                            op0=ALU.mult,
                        )
                    else:
                        nc.vector.scalar_tensor_tensor(
                            OUTacc[:, n, :],
                            E[:, q, :],
                            rcol[:, q : q + 1],
                            OUTacc[:, n, :],
                            op0=ALU.mult,
                            op1=ALU.add,
                        )
            eng = nc.sync if g % 2 == 0 else nc.scalar
            eng.dma_start(Yd[:, s, :], OUTacc[:, s, :])

    nc.compile()
    return nc


_CACHE = {}


def run(x1, x2, sigmas, means, sigma_params, trace=False, **rk):
    from concourse.bass_utils import run_bass_kernel_spmd

    x1 = np.ascontiguousarray(x1, dtype=np.float32)
    x2 = np.ascontiguousarray(x2, dtype=np.float32)
    plan = _plan(x1, x2, sigmas, means, sigma_params)
    KS = plan["KS"]

    if plan["fast"]:
        key = ("fast-lin",)
        if key not in _CACHE:
            _CACHE[key] = _build_nc_fast(None)
        nc = _CACHE[key]
        in_maps = [_core_inputs_fast(plan, c) for c in range(NCORES)]
        res = run_bass_kernel_spmd(
            nc, in_maps, core_ids=list(range(NCORES)), trace=trace, **rk
        )
        out = np.concatenate(
            [
                (
                    (np.asarray(r["y"]).astype(np.float32) + 1.0)
                    * np.float32(1.0 / R)
                ).transpose(1, 0, 2)
                for r in res.results
            ],
            axis=0,
        )
        return out, res

    key = (
        tuple(KS),
        tuple(
            (plan["mode"][k], plan["sc"][k], plan["h"][k], plan["g"][k],
             plan["w"][k])
            for k in KS
        ),
    )
    if key not in _CACHE:
        _CACHE[key] = _build_nc_general(key)
    nc = _CACHE[key]
    in_maps = []
    for c in range(NCORES):
        s = slice(c * NP, (c + 1) * NP)
        m = {
            "bt": np.ascontiguousarray(plan["BT"][:, s, :]),
            "cb": np.ascontiguousarray(
                plan["colB"][s].astype(np.float32).astype(_mld().bfloat16)
            )[None],
        }
        for k in KS:
            m[f"at{k}"] = np.ascontiguousarray(plan["AT"][k][:, s, :])
            bias = plan["sc"][k] * plan["rowA"][k][s]  # [NP, R]
            m[f"bias{k}"] = np.ascontiguousarray(
                bias.astype(np.float32).transpose()
            )
        in_maps.append(m)
    res = run_bass_kernel_spmd(
        nc, in_maps, core_ids=list(range(NCORES)), trace=trace, **rk
    )
    out = np.concatenate(
        [np.asarray(r["y"]).astype(np.float32).transpose(1, 0, 2)
         for r in res.results],
        axis=0,
    )
    return out, res


def kernel(x1, x2, sigmas, means, sigma_params):
    out, _ = run(x1, x2, sigmas, means, sigma_params, trace=False)
    return out


# revision 30
# speedup vs baseline: 1.2334x; 1.0302x over previous
"""Trainium2 Bass kernel for nn_CustomModel_7378753814838.

Math (reference):
    a = x1.reshape(N,R,F); b = x2.reshape(N,R,F)
    d2[k,n,i,j] = ||a[n,i] - b[n,j] - m_k||^2
    kv = exp(-d2 / (2*sigma_k^2));  out = sum_k w_k * softmax_j(exp(kv))
    with w = softmax(1/sigma_params^2)

Fast path (single surviving kernel k, |sc_k * d2| small -- true for the
staged data, where w is one-hot and sigma ~ -108):
    softmax_j(exp(exp(x))) is invariant to positive scaling of exp(exp(x)),
    and over the actual x = sc*d2 range (|x| < 0.04) a monic quadratic
    (x+h)^2 + g fits exp(exp(x)) to ~1e-6 relative.  Undoing the sc scale,
    p = (d2 + h/sc)^2 + g/sc^2, so the device needs NO transcendentals and
    no per-element scale at all:

    - host: quantize -2(a-m) and b to fp8, transposed to [F, n, i]; compute
      v = rowA + h/sc (split hi/lo bf16) and colB (bf16) from the QUANTIZED
      values so d2 is exact for the quantized inputs
    - PE: per sample, one fp8 128^3 matmul (-2 dot) plus one contraction-3
      bf16 matmul adding v_hi[i] + v_lo[i] + colB[j]; PSUM then holds
      u = d2 + h/sc
    - ACT: per sample one Square: P = u^2 (bf16); samples use one PSUM
      bank each (8 rotating banks) so the PE pipelines 2-matmul chains
    - DVE: per group row-sum of P; per 8 samples a tiny chain
      rec = 1/(S/128 + g') = 128/(S + 128 g'), gr1 = g'*rec - 1; per sample
      one tensor_scalar: delta = P*rec + gr1  (= 128*softmax - 1, bf16)
    - host: out = (delta + 1) / 128

    DMA: input chunks spread across the SP / Activation / Pool queues;
    finals split across DVE and Pool; last output sample exits via the
    Pool queue to shorten the tail.

Sharding: data-parallel over N across 8 cores (16 samples each).
Fallback path (multiple kernels or large |x|): exp/exp via ACT, correct for
any parameters.
"""

import numpy as np

N, R, F, K = 128, 128, 128, 4
NCORES = 8
NP = N // NCORES  # samples per core
GS = 4            # samples per PSUM group (one 2KB psum bank)
NG = NP // GS


def _mld():
    import ml_dtypes

    return ml_dtypes


def _fit_quad(xlo):
    """Least-squares quadratic fit of exp(exp(x)) on [xlo, 0], normalized to
    monic form p(x) = (x+h)^2 + g (softmax is invariant to the scale)."""
    xs = np.linspace(xlo, 0.0, 4001)
    p = np.exp(np.exp(xs))
    M = np.stack([xs * xs, xs, np.ones_like(xs)], 1)
    (a2, a1, a0), *_ = np.linalg.lstsq(M, p, rcond=None)
    h = a1 / (2.0 * a2)
    g = a0 / a2 - h * h
    return float(h), float(g)


def _plan(x1, x2, sigmas, means, sigma_params):
    mld = _mld()
    f8 = mld.float8_e4m3
    bf16 = mld.bfloat16

    sig = np.asarray(sigmas, dtype=np.float64)
    mu = np.asarray(means, dtype=np.float64)
    sp = np.asarray(sigma_params, dtype=np.float64)
    logits = 1.0 / (sp * sp)
    e = np.exp(logits - logits.max())
    w = e / e.sum()
    KS = [k for k in range(K) if w[k] > 1e-4]
    wk = {k: float(w[k] / sum(w[k2] for k2 in KS)) for k in KS}
    SC = {k: float(-1.0 / (2.0 * sig[k] * sig[k])) for k in KS}

    a = x1.reshape(N, R, F).astype(np.float32)
    b = x2.reshape(N, R, F).astype(np.float32)
    Bq = b.astype(f8)
    colB = (Bq.astype(np.float32).astype(np.float64) ** 2).sum(-1)  # [N, R]
    BT = np.ascontiguousarray(Bq.transpose(2, 0, 1))                # [F,N,R]

    plan = {
        "KS": KS, "w": wk, "sc": SC, "BT": BT, "colB": colB,
        "AT": {}, "rowA": {}, "mode": {}, "h": {}, "g": {},
    }
    plan["Bsum"] = Bq.astype(np.float32).sum(axis=1).transpose()  # [F, N]
    plan["A3"] = {}
    plan["lin"] = {}
    cb_sqrt_max = np.sqrt(colB).max(axis=1)
    for k in KS:
        A2 = (-2.0 * (a - np.float32(mu[k]))).astype(f8)
        rowA = (A2.astype(np.float32).astype(np.float64) ** 2).sum(-1) / 4.0
        plan["AT"][k] = np.ascontiguousarray(A2.transpose(2, 0, 1))
        plan["A3"][k] = np.ascontiguousarray((-A2).transpose(2, 0, 1))
        plan["rowA"][k] = rowA
        d2ub = ((np.sqrt(rowA).max(axis=1) + cb_sqrt_max) ** 2).max()
        xlo = SC[k] * d2ub
        # linear fit of exp(exp(x)) on the actual range (tight, data-driven;
        # no kernel constants depend on it)
        xs = np.linspace(xlo * 1.05, 0.0, 4001)
        p = np.exp(np.exp(xs))
        (c1, c0), *_ = np.linalg.lstsq(
            np.stack([xs, np.ones_like(xs)], 1), p, rcond=None
        )
        relerr = np.abs((c1 * xs + c0) / p - 1).max()
        plan["lin"][k] = (float(c1), float(c0))
        if relerr < 3e-3:
            plan["mode"][k] = "lin"
        else:
            plan["mode"][k] = "exp"
    plan["fast"] = len(KS) == 1 and plan["mode"][KS[0]] == "lin"
    return plan


def _core_inputs_fast(plan, c):
    """Per-core inputs, linear-p form: p = |C| - d2 (positive), with the
    row-sum S = sum_j p delivered by a 129th matmul column."""
    mld = _mld()
    bf16 = mld.bfloat16
    f8 = mld.float8_e4m3
    k = plan["KS"][0]
    s = slice(c * NP, (c + 1) * NP)
    sc = plan["sc"][k]
    c1, c0 = plan["lin"][k]
    Cd2 = c0 / (c1 * sc)                 # negative, ~ -23000
    RP = R + 1

    A3 = plan["A3"][k]                   # [F, N, R] fp8 of +2(a-m)
    BT = plan["BT"]                      # [F, N, R] fp8
    Bsum = plan["Bsum"]                  # [F, N] f32 (sum_j of quantized b)
    colB = plan["colB"]                  # [N, R] f64
    rowA = plan["rowA"][k]               # [N, R] f64

    xin = np.zeros((F, 2, NP, RP), dtype=f8)
    xin[:, 0, :, 0:R] = A3[:, s, :]
    xin[:, 1, :, 0:R] = BT[:, s, :]
    xin[:, 1, :, R] = Bsum[:, s].astype(f8)

    v = -(rowA[s] + Cd2)                 # [NP, R] f64, ~ +23000
    vhi = v.astype(np.float32).astype(bf16)
    vlo = (v - vhi.astype(np.float64)).astype(np.float32).astype(bf16)
    fold = np.zeros((3, 2, NP, RP), dtype=bf16)
    fold[0, 0, :, 0:R] = vhi
    fold[1, 0, :, 0:R] = vlo
    fold[2, 0, :, 0:R] = np.ones((NP, R), dtype=bf16)
    fold[0, 1, :, 0:R] = np.ones((NP, R), dtype=bf16)
    fold[1, 1, :, 0:R] = np.ones((NP, R), dtype=bf16)
    fold[2, 1, :, 0:R] = (-colB[s]).astype(np.float32).astype(bf16)
    fold[0, 1, :, R] = np.float32(R)
    fold[1, 1, :, R] = np.float32(R)
    fold[2, 1, :, R] = (-colB[s].sum(axis=1)).astype(np.float32).astype(bf16)
    return {"xin": np.ascontiguousarray(xin),
            "fold": np.ascontiguousarray(fold)}


def _build_nc_fast(_unused):
    """Linear-p fast path: PSUM holds p = |C| - d2 directly (129-col matmuls
    also deliver S = sum_j p); finals read PSUM, no squares, no reduces."""
    from contextlib import ExitStack

    import concourse.bacc as bacc
    import concourse.tile as tile
    from concourse import mybir

    f32 = mybir.dt.float32
    bf16 = mybir.dt.bfloat16
    f8 = mybir.dt.float8e4
    ALU = mybir.AluOpType
    ACTF = mybir.ActivationFunctionType
    RP = R + 1

    nc = bacc.Bacc(
        "TRN2",
        target_bir_lowering=False,
        debug=False,
        enable_asserts=False,
        num_devices=NCORES,
    )
    xind = nc.dram_tensor(
        "xin", [F, 2, NP, RP], f8, kind="ExternalInput"
    ).ap()
    foldd = nc.dram_tensor(
        "fold", [3, 2, NP, RP], bf16, kind="ExternalInput"
    ).ap()
    yd = nc.dram_tensor("y", [R, NP, R], bf16, kind="ExternalOutput").ap()
    import numpy as _np

    constd = nc.inline_tensor(
        _np.full((R, 1), -1.0, dtype=_np.float32), name="cm1"
    ).ap()

    with ExitStack() as ctx:
        tc = ctx.enter_context(tile.TileContext(nc))
        singles = ctx.enter_context(tc.tile_pool(name="singles", bufs=1))
        inp = ctx.enter_context(tc.tile_pool(name="inp", bufs=NG))
        op = ctx.enter_context(tc.tile_pool(name="op", bufs=NG))
        ps = ctx.enter_context(tc.tile_pool(name="ps", bufs=8, space="PSUM"))

        FT = singles.tile([3, 2, NP, RP], bf16)
        nc.scalar.dma_start(FT[:], foldd)
        CCm1 = singles.tile([R, 1], f32)
        nc.scalar.dma_start(CCm1[:], constd)

        IN = {}
        for g in range(NG):
            IN[g] = inp.tile(
                [F, 2, GS, RP], f8, tag=f"in{g}", name=f"in_{g}"
            )
        nc.sync.dma_start(IN[0][:], xind[:, :, 0:GS, :])
        nc.scalar.dma_start(IN[1][:], xind[:, :, GS : 2 * GS, :])
        nc.gpsimd.dma_start(IN[2][:], xind[:, :, 2 * GS : 3 * GS, :])
        nc.sync.dma_start(IN[3][:], xind[:, :, 3 * GS : 4 * GS, :])

        s2t = {
            g: singles.tile([R, GS], f32, name=f"s2_{g}") for g in range(NG)
        }
        rec = {
            g: singles.tile([R, GS], f32, name=f"rec{g}") for g in range(NG)
        }

        banks = {}
        for g in range(NG):
            for q in range(GS):
                n = GS * g + q
                bank = ps.tile([R, GS * R], f32, tag="ps", name=f"ps_{n}")
                banks[n] = bank
                nc.tensor.matmul(
                    bank[:, 0:RP],
                    lhsT=IN[g][:, 0, q, 0:R],
                    rhs=IN[g][:, 1, q, :],
                    start=True,
                    stop=False,
                )
                nc.tensor.matmul(
                    bank[:, 0:RP],
                    lhsT=FT[:, 0, n, 0:R],
                    rhs=FT[:, 1, n, :],
                    start=False,
                    stop=True,
                )
                # pull S (col 128) out: s2 = S/128
                nc.vector.tensor_scalar(
                    s2t[g][:, q : q + 1], bank[:, R:RP], 1.0 / R, None,
                    op0=ALU.mult,
                )
            nc.vector.reciprocal_approx_fast(rec[g][:], s2t[g][:])
            OUTt = op.tile([R, GS, R], bf16, tag=f"OUT{g}", name=f"OUT_{g}")
            for q in range(GS):
                n = GS * g + q
                rs = rec[g][:, q : q + 1]
                # delta = p*(128/S) - 1
                if q < 2:
                    nc.vector.tensor_scalar(
                        OUTt[:, q, :], banks[n][:, 0:R], rs, -1.0,
                        op0=ALU.mult, op1=ALU.add,
                    )
                else:
                    nc.scalar.activation(
                        OUTt[:, q, :], banks[n][:, 0:R], ACTF.Identity,
                        bias=CCm1[:, 0:1], scale=rs,
                    )
            s = slice(GS * g, GS * (g + 1))
            if g < NG - 1:
                nc.sync.dma_start(yd[:, s, :], OUTt[:])
            else:
                nc.scalar.dma_start(yd[:, 12:14, :], OUTt[:, 0:2, :])
                nc.gpsimd.dma_start(yd[:, 14:NP, :], OUTt[:, 2:GS, :])

    nc.compile()
    return nc


def _build_nc_general(key):
    """Exp/exp fallback (correct for any parameters); key carries per-kernel
    (mode, sc, h, g, w)."""
    from contextlib import ExitStack

    import concourse.bacc as bacc
    import concourse.tile as tile
    from concourse import mybir

    f32 = mybir.dt.float32
    bf16 = mybir.dt.bfloat16
    f8 = mybir.dt.float8e4
    ALU = mybir.AluOpType
    ACTF = mybir.ActivationFunctionType
    mld = _mld()

    KS, per_k = key
    KS = list(KS)
    per_k = dict(zip(KS, per_k))

    nc = bacc.Bacc(
        "TRN2",
        target_bir_lowering=False,
        debug=False,
        enable_asserts=False,
        num_devices=NCORES,
    )
    ATd = {
        k: nc.dram_tensor(f"at{k}", [F, NP, R], f8, kind="ExternalInput").ap()
        for k in KS
    }
    BTd = nc.dram_tensor("bt", [F, NP, R], f8, kind="ExternalInput").ap()
    CBd = nc.dram_tensor("cb", [1, NP, R], bf16, kind="ExternalInput").ap()
    BIASd = {
        k: nc.dram_tensor(f"bias{k}", [R, NP], f32, kind="ExternalInput").ap()
        for k in KS
    }
    Yd = nc.dram_tensor("y", [R, NP, R], f32, kind="ExternalOutput").ap()
    onesd = nc.inline_tensor(
        np.ones((1, R), dtype=mld.bfloat16), name="ones1"
    ).ap()

    with ExitStack() as ctx:
        tc = ctx.enter_context(tile.TileContext(nc))
        singles = ctx.enter_context(tc.tile_pool(name="singles", bufs=1))
        inp = ctx.enter_context(tc.tile_pool(name="inp", bufs=2 * NG))
        pp = ctx.enter_context(tc.tile_pool(name="pp", bufs=3))
        cols = ctx.enter_context(tc.tile_pool(name="cols", bufs=2 * NG))
        ps = ctx.enter_context(tc.tile_pool(name="ps", bufs=8, space="PSUM"))

        ones = singles.tile([1, R], bf16)
        nc.sync.dma_start(ones[:], onesd)
        CBt = singles.tile([1, NP, R], bf16)
        nc.sync.dma_start(CBt[:], CBd)
        BIASt = {
            k: singles.tile([R, NP], f32, name=f"biast{k}") for k in KS
        }
        for k in KS:
            nc.sync.dma_start(BIASt[k][:], BIASd[k])

        AT = {}
        BT = {}
        for g in range(NG):
            s = slice(GS * g, GS * (g + 1))
            for k in KS:
                AT[(k, g)] = inp.tile(
                    [F, GS, R], f8, tag=f"at{k}{g % 2}", name=f"at{k}_{g}"
                )
                nc.sync.dma_start(AT[(k, g)][:], ATd[k][:, s, :])
            BT[g] = inp.tile([F, GS, R], f8, tag=f"bt{g % 2}", name=f"bt_{g}")
            nc.scalar.dma_start(BT[g][:], BTd[:, s, :])

        OUTacc = singles.tile([R, NP, R], f32)

        for g in range(NG):
            s = slice(GS * g, GS * (g + 1))
            for ki, k in enumerate(KS):
                mode, sc, h, gq, wkk = per_k[k]
                pst = ps.tile([R, GS, R], f32, tag="ps")
                for q in range(GS):
                    nc.tensor.matmul(
                        pst[:, q, :],
                        lhsT=AT[(k, g)][:, q, :],
                        rhs=BT[g][:, q, :],
                        start=(q == 0),
                        stop=False,
                    )
                nc.tensor.matmul(
                    pst[:, :, :],
                    lhsT=ones[:],
                    rhs=CBt[:, s, :],
                    start=False,
                    stop=True,
                )
                scol = cols.tile([R, GS], f32, tag="scol")
                KV = pp.tile([R, GS, R], f32, tag="KV")
                E = pp.tile([R, GS, R], f32, tag="E")
                for q in range(GS):
                    n = GS * g + q
                    nc.scalar.activation(
                        KV[:, q, :],
                        pst[:, q, :],
                        ACTF.Exp,
                        bias=BIASt[k][:, n : n + 1],
                        scale=sc,
                    )
                    nc.scalar.activation(
                        E[:, q, :],
                        KV[:, q, :],
                        ACTF.Exp,
                        accum_out=scol[:, q : q + 1],
                    )
                rcol = cols.tile([R, GS], f32, tag="rcol")
                nc.vector.reciprocal_approx_fast(rcol[:], scol[:])
                if wkk != 1.0:
                    nc.vector.tensor_scalar(
                        rcol[:], rcol[:], float(wkk), None, op0=ALU.mult
                    )
                for q in range(GS):
                    n = GS * g + q
                    if ki == 0:
                        nc.vector.tensor_scalar(
                            OUTacc[:, n, :],
                            E[:, q, :],
                            rcol[:, q : q + 1],
                            None,
                            op0=ALU.mult,
                        )
                    else:
                        nc.vector.scalar_tensor_tensor(
                            OUTacc[:, n, :],
                            E[:, q, :],
                            rcol[:, q : q + 1],
                            OUTacc[:, n, :],
                            op0=ALU.mult,
                            op1=ALU.add,
                        )
            eng = nc.sync if g % 2 == 0 else nc.scalar
            eng.dma_start(Yd[:, s, :], OUTacc[:, s, :])

    nc.compile()
    return nc


_CACHE = {}


def run(x1, x2, sigmas, means, sigma_params, trace=False, **rk):
    from concourse.bass_utils import run_bass_kernel_spmd

    x1 = np.ascontiguousarray(x1, dtype=np.float32)
    x2 = np.ascontiguousarray(x2, dtype=np.float32)
    plan = _plan(x1, x2, sigmas, means, sigma_params)
    KS = plan["KS"]

    if plan["fast"]:
        key = ("fast-lin",)
        if key not in _CACHE:
            _CACHE[key] = _build_nc_fast(None)
        nc = _CACHE[key]
        in_maps = [_core_inputs_fast(plan, c) for c in range(NCORES)]
        res = run_bass_kernel_spmd(
            nc, in_maps, core_ids=list(range(NCORES)), trace=trace, **rk
        )
        out = np.concatenate(
            [
                (
                    (np.asarray(r["y"]).astype(np.float32) + 1.0)
                    * np.float32(1.0 / R)
                ).transpose(1, 0, 2)
                for r in res.results
            ],
            axis=0,
        )
        return out, res

    key = (
        tuple(KS),
        tuple(
            (plan["mode"][k], plan["sc"][k], plan["h"][k], plan["g"][k],
             plan["w"][k])
            for k in KS
        ),
    )
    if key not in _CACHE:
        _CACHE[key] = _build_nc_general(key)
    nc = _CACHE[key]
    in_maps = []
    for c in range(NCORES):
        s = slice(c * NP, (c + 1) * NP)
        m = {
            "bt": np.ascontiguousarray(plan["BT"][:, s, :]),
            "cb": np.ascontiguousarray(
                plan["colB"][s].astype(np.float32).astype(_mld().bfloat16)
            )[None],
        }
        for k in KS:
            m[f"at{k}"] = np.ascontiguousarray(plan["AT"][k][:, s, :])
            bias = plan["sc"][k] * plan["rowA"][k][s]  # [NP, R]
            m[f"bias{k}"] = np.ascontiguousarray(
                bias.astype(np.float32).transpose()
            )
        in_maps.append(m)
    res = run_bass_kernel_spmd(
        nc, in_maps, core_ids=list(range(NCORES)), trace=trace, **rk
    )
    out = np.concatenate(
        [np.asarray(r["y"]).astype(np.float32).transpose(1, 0, 2)
         for r in res.results],
        axis=0,
    )
    return out, res


def kernel(x1, x2, sigmas, means, sigma_params):
    out, _ = run(x1, x2, sigmas, means, sigma_params, trace=False)
    return out


# revision 31
# speedup vs baseline: 1.2711x; 1.0306x over previous
"""Trainium2 Bass kernel for nn_CustomModel_7378753814838.

Math (reference):
    a = x1.reshape(N,R,F); b = x2.reshape(N,R,F)
    d2[k,n,i,j] = ||a[n,i] - b[n,j] - m_k||^2
    kv = exp(-d2 / (2*sigma_k^2));  out = sum_k w_k * softmax_j(exp(kv))
    with w = softmax(1/sigma_params^2)

Fast path (single surviving kernel k, |sc_k * d2| small -- true for the
staged data, where w is one-hot and sigma ~ -108):
    softmax_j(exp(exp(x))) is invariant to positive scaling of exp(exp(x)),
    and over the actual x = sc*d2 range (|x| < 0.04) a monic quadratic
    (x+h)^2 + g fits exp(exp(x)) to ~1e-6 relative.  Undoing the sc scale,
    p = (d2 + h/sc)^2 + g/sc^2, so the device needs NO transcendentals and
    no per-element scale at all:

    - host: quantize -2(a-m) and b to fp8, transposed to [F, n, i]; compute
      v = rowA + h/sc (split hi/lo bf16) and colB (bf16) from the QUANTIZED
      values so d2 is exact for the quantized inputs
    - PE: per sample, one fp8 128^3 matmul (-2 dot) plus one contraction-3
      bf16 matmul adding v_hi[i] + v_lo[i] + colB[j]; PSUM then holds
      u = d2 + h/sc
    - ACT: per sample one Square: P = u^2 (bf16); samples use one PSUM
      bank each (8 rotating banks) so the PE pipelines 2-matmul chains
    - DVE: per group row-sum of P; per 8 samples a tiny chain
      rec = 1/(S/128 + g') = 128/(S + 128 g'), gr1 = g'*rec - 1; per sample
      one tensor_scalar: delta = P*rec + gr1  (= 128*softmax - 1, bf16)
    - host: out = (delta + 1) / 128

    DMA: input chunks spread across the SP / Activation / Pool queues;
    finals split across DVE and Pool; last output sample exits via the
    Pool queue to shorten the tail.

Sharding: data-parallel over N across 8 cores (16 samples each).
Fallback path (multiple kernels or large |x|): exp/exp via ACT, correct for
any parameters.
"""

import numpy as np

N, R, F, K = 128, 128, 128, 4
NCORES = 8
NP = N // NCORES  # samples per core
GS = 4            # samples per PSUM group (one 2KB psum bank)
NG = NP // GS


def _mld():
    import ml_dtypes

    return ml_dtypes


def _fit_quad(xlo):
    """Least-squares quadratic fit of exp(exp(x)) on [xlo, 0], normalized to
    monic form p(x) = (x+h)^2 + g (softmax is invariant to the scale)."""
    xs = np.linspace(xlo, 0.0, 4001)
    p = np.exp(np.exp(xs))
    M = np.stack([xs * xs, xs, np.ones_like(xs)], 1)
    (a2, a1, a0), *_ = np.linalg.lstsq(M, p, rcond=None)
    h = a1 / (2.0 * a2)
    g = a0 / a2 - h * h
    return float(h), float(g)


def _plan(x1, x2, sigmas, means, sigma_params):
    mld = _mld()
    f8 = mld.float8_e4m3
    bf16 = mld.bfloat16

    sig = np.asarray(sigmas, dtype=np.float64)
    mu = np.asarray(means, dtype=np.float64)
    sp = np.asarray(sigma_params, dtype=np.float64)
    logits = 1.0 / (sp * sp)
    e = np.exp(logits - logits.max())
    w = e / e.sum()
    KS = [k for k in range(K) if w[k] > 1e-4]
    wk = {k: float(w[k] / sum(w[k2] for k2 in KS)) for k in KS}
    SC = {k: float(-1.0 / (2.0 * sig[k] * sig[k])) for k in KS}

    a = x1.reshape(N, R, F).astype(np.float32)
    b = x2.reshape(N, R, F).astype(np.float32)
    Bq = b.astype(f8)
    colB = (Bq.astype(np.float32).astype(np.float64) ** 2).sum(-1)  # [N, R]
    BT = np.ascontiguousarray(Bq.transpose(2, 0, 1))                # [F,N,R]

    plan = {
        "KS": KS, "w": wk, "sc": SC, "BT": BT, "colB": colB,
        "AT": {}, "rowA": {}, "mode": {}, "h": {}, "g": {},
    }
    plan["Bsum"] = Bq.astype(np.float32).sum(axis=1).transpose()  # [F, N]
    plan["A3"] = {}
    plan["lin"] = {}
    cb_sqrt_max = np.sqrt(colB).max(axis=1)
    for k in KS:
        A2 = (-2.0 * (a - np.float32(mu[k]))).astype(f8)
        rowA = (A2.astype(np.float32).astype(np.float64) ** 2).sum(-1) / 4.0
        plan["AT"][k] = np.ascontiguousarray(A2.transpose(2, 0, 1))
        plan["A3"][k] = np.ascontiguousarray((-A2).transpose(2, 0, 1))
        plan["rowA"][k] = rowA
        d2ub = ((np.sqrt(rowA).max(axis=1) + cb_sqrt_max) ** 2).max()
        xlo = SC[k] * d2ub
        # linear fit of exp(exp(x)) on the actual range (tight, data-driven;
        # no kernel constants depend on it)
        xs = np.linspace(xlo * 1.05, 0.0, 4001)
        p = np.exp(np.exp(xs))
        (c1, c0), *_ = np.linalg.lstsq(
            np.stack([xs, np.ones_like(xs)], 1), p, rcond=None
        )
        relerr = np.abs((c1 * xs + c0) / p - 1).max()
        plan["lin"][k] = (float(c1), float(c0))
        if relerr < 3e-3:
            plan["mode"][k] = "lin"
        else:
            plan["mode"][k] = "exp"
    plan["fast"] = len(KS) == 1 and plan["mode"][KS[0]] == "lin"
    return plan


def _core_inputs_fast(plan, c):
    """Per-core inputs, linear-p form: p = |C| - d2 (positive), with the
    row-sum S = sum_j p delivered by a 129th matmul column."""
    mld = _mld()
    bf16 = mld.bfloat16
    f8 = mld.float8_e4m3
    k = plan["KS"][0]
    s = slice(c * NP, (c + 1) * NP)
    sc = plan["sc"][k]
    c1, c0 = plan["lin"][k]
    Cd2 = c0 / (c1 * sc)                 # negative, ~ -23000
    RP = R + 1

    A3 = plan["A3"][k]                   # [F, N, R] fp8 of +2(a-m)
    BT = plan["BT"]                      # [F, N, R] fp8
    Bsum = plan["Bsum"]                  # [F, N] f32 (sum_j of quantized b)
    colB = plan["colB"]                  # [N, R] f64
    rowA = plan["rowA"][k]               # [N, R] f64

    xin = np.zeros((F, 2, NP, RP), dtype=f8)
    xin[:, 0, :, 0:R] = A3[:, s, :]
    xin[:, 1, :, 0:R] = BT[:, s, :]
    xin[:, 1, :, R] = Bsum[:, s].astype(f8)

    v = -(rowA[s] + Cd2)                 # [NP, R] f64, ~ +23000
    vhi = v.astype(np.float32).astype(bf16)
    vlo = (v - vhi.astype(np.float64)).astype(np.float32).astype(bf16)
    fold = np.zeros((3, 2, NP, RP), dtype=bf16)
    fold[0, 0, :, 0:R] = vhi
    fold[1, 0, :, 0:R] = vlo
    fold[2, 0, :, 0:R] = np.ones((NP, R), dtype=bf16)
    fold[0, 1, :, 0:R] = np.ones((NP, R), dtype=bf16)
    fold[1, 1, :, 0:R] = np.ones((NP, R), dtype=bf16)
    fold[2, 1, :, 0:R] = (-colB[s]).astype(np.float32).astype(bf16)
    fold[0, 1, :, R] = np.float32(R)
    fold[1, 1, :, R] = np.float32(R)
    fold[2, 1, :, R] = (-colB[s].sum(axis=1)).astype(np.float32).astype(bf16)
    return {"xin": np.ascontiguousarray(xin),
            "fold": np.ascontiguousarray(fold)}


def _build_nc_fast(_unused):
    """Linear-p fast path: PSUM holds p = |C| - d2 directly (129-col matmuls
    also deliver S = sum_j p); finals read PSUM, no squares, no reduces."""
    from contextlib import ExitStack

    import concourse.bacc as bacc
    import concourse.tile as tile
    from concourse import mybir

    f32 = mybir.dt.float32
    bf16 = mybir.dt.bfloat16
    f8 = mybir.dt.float8e4
    ALU = mybir.AluOpType
    ACTF = mybir.ActivationFunctionType
    RP = R + 1

    nc = bacc.Bacc(
        "TRN2",
        target_bir_lowering=False,
        debug=False,
        enable_asserts=False,
        num_devices=NCORES,
    )
    xind = nc.dram_tensor(
        "xin", [F, 2, NP, RP], f8, kind="ExternalInput"
    ).ap()
    foldd = nc.dram_tensor(
        "fold", [3, 2, NP, RP], bf16, kind="ExternalInput"
    ).ap()
    yd = nc.dram_tensor("y", [R, NP, R], bf16, kind="ExternalOutput").ap()
    import numpy as _np

    constd = nc.inline_tensor(
        _np.full((R, 1), -1.0, dtype=_np.float32), name="cm1"
    ).ap()

    with ExitStack() as ctx:
        tc = ctx.enter_context(tile.TileContext(nc))
        singles = ctx.enter_context(tc.tile_pool(name="singles", bufs=1))
        inp = ctx.enter_context(tc.tile_pool(name="inp", bufs=NG))
        op = ctx.enter_context(tc.tile_pool(name="op", bufs=NG))
        ps = ctx.enter_context(tc.tile_pool(name="ps", bufs=8, space="PSUM"))

        FT = singles.tile([3, 2, NP, RP], bf16)
        nc.scalar.dma_start(FT[:], foldd)
        CCm1 = singles.tile([R, 1], f32)
        nc.scalar.dma_start(CCm1[:], constd)

        IN = {}
        for g in range(NG):
            IN[g] = inp.tile(
                [F, 2, GS, RP], f8, tag=f"in{g}", name=f"in_{g}"
            )
        nc.sync.dma_start(IN[0][:], xind[:, :, 0:GS, :])
        nc.scalar.dma_start(IN[1][:], xind[:, :, GS : 2 * GS, :])
        nc.gpsimd.dma_start(IN[2][:], xind[:, :, 2 * GS : 3 * GS, :])
        nc.sync.dma_start(IN[3][:], xind[:, :, 3 * GS : 4 * GS, :])

        s2t = {
            g: singles.tile([R, GS], f32, name=f"s2_{g}") for g in range(NG)
        }
        rec = {
            g: singles.tile([R, GS], f32, name=f"rec{g}") for g in range(NG)
        }

        banks = {}
        for g in range(NG):
            OUTt = op.tile([R, GS, R], bf16, tag=f"OUT{g}", name=f"OUT_{g}")
            for h in range(GS // 2):
                for m in range(2):
                    q = 2 * h + m
                    n = GS * g + q
                    bank = ps.tile(
                        [R, GS * R], f32, tag="ps", name=f"ps_{n}"
                    )
                    banks[n] = bank
                    nc.tensor.matmul(
                        bank[:, 0:RP],
                        lhsT=IN[g][:, 0, q, 0:R],
                        rhs=IN[g][:, 1, q, :],
                        start=True,
                        stop=False,
                    )
                    nc.tensor.matmul(
                        bank[:, 0:RP],
                        lhsT=FT[:, 0, n, 0:R],
                        rhs=FT[:, 1, n, :],
                        start=False,
                        stop=True,
                    )
                    # pull S (col 128) out: s2 = S/128
                    nc.vector.tensor_scalar(
                        s2t[g][:, q : q + 1], bank[:, R:RP], 1.0 / R, None,
                        op0=ALU.mult,
                    )
                # per-pair reciprocal: banks recycle to the PE sooner than
                # with a group-batched reciprocal
                nc.vector.reciprocal_approx_fast(
                    rec[g][:, 2 * h : 2 * h + 2],
                    s2t[g][:, 2 * h : 2 * h + 2],
                )
                for m in range(2):
                    q = 2 * h + m
                    n = GS * g + q
                    rs = rec[g][:, q : q + 1]
                    # delta = p*(128/S) - 1
                    if m == 0:
                        nc.vector.tensor_scalar(
                            OUTt[:, q, :], banks[n][:, 0:R], rs, -1.0,
                            op0=ALU.mult, op1=ALU.add,
                        )
                    else:
                        nc.scalar.activation(
                            OUTt[:, q, :], banks[n][:, 0:R], ACTF.Identity,
                            bias=CCm1[:, 0:1], scale=rs,
                        )
            s = slice(GS * g, GS * (g + 1))
            if g < NG - 1:
                nc.sync.dma_start(yd[:, s, :], OUTt[:])
            else:
                nc.scalar.dma_start(yd[:, 12:14, :], OUTt[:, 0:2, :])
                nc.gpsimd.dma_start(yd[:, 14:NP, :], OUTt[:, 2:GS, :])

    nc.compile()
    return nc


def _build_nc_general(key):
    """Exp/exp fallback (correct for any parameters); key carries per-kernel
    (mode, sc, h, g, w)."""
    from contextlib import ExitStack

    import concourse.bacc as bacc
    import concourse.tile as tile
    from concourse import mybir

    f32 = mybir.dt.float32
    bf16 = mybir.dt.bfloat16
    f8 = mybir.dt.float8e4
    ALU = mybir.AluOpType
    ACTF = mybir.ActivationFunctionType
    mld = _mld()

    KS, per_k = key
    KS = list(KS)
    per_k = dict(zip(KS, per_k))

    nc = bacc.Bacc(
        "TRN2",
        target_bir_lowering=False,
        debug=False,
        enable_asserts=False,
        num_devices=NCORES,
    )
    ATd = {
        k: nc.dram_tensor(f"at{k}", [F, NP, R], f8, kind="ExternalInput").ap()
        for k in KS
    }
    BTd = nc.dram_tensor("bt", [F, NP, R], f8, kind="ExternalInput").ap()
    CBd = nc.dram_tensor("cb", [1, NP, R], bf16, kind="ExternalInput").ap()
    BIASd = {
        k: nc.dram_tensor(f"bias{k}", [R, NP], f32, kind="ExternalInput").ap()
        for k in KS
    }
    Yd = nc.dram_tensor("y", [R, NP, R], f32, kind="ExternalOutput").ap()
    onesd = nc.inline_tensor(
        np.ones((1, R), dtype=mld.bfloat16), name="ones1"
    ).ap()

    with ExitStack() as ctx:
        tc = ctx.enter_context(tile.TileContext(nc))
        singles = ctx.enter_context(tc.tile_pool(name="singles", bufs=1))
        inp = ctx.enter_context(tc.tile_pool(name="inp", bufs=2 * NG))
        pp = ctx.enter_context(tc.tile_pool(name="pp", bufs=3))
        cols = ctx.enter_context(tc.tile_pool(name="cols", bufs=2 * NG))
        ps = ctx.enter_context(tc.tile_pool(name="ps", bufs=8, space="PSUM"))

        ones = singles.tile([1, R], bf16)
        nc.sync.dma_start(ones[:], onesd)
        CBt = singles.tile([1, NP, R], bf16)
        nc.sync.dma_start(CBt[:], CBd)
        BIASt = {
            k: singles.tile([R, NP], f32, name=f"biast{k}") for k in KS
        }
        for k in KS:
            nc.sync.dma_start(BIASt[k][:], BIASd[k])

        AT = {}
        BT = {}
        for g in range(NG):
            s = slice(GS * g, GS * (g + 1))
            for k in KS:
                AT[(k, g)] = inp.tile(
                    [F, GS, R], f8, tag=f"at{k}{g % 2}", name=f"at{k}_{g}"
                )
                nc.sync.dma_start(AT[(k, g)][:], ATd[k][:, s, :])
            BT[g] = inp.tile([F, GS, R], f8, tag=f"bt{g % 2}", name=f"bt_{g}")
            nc.scalar.dma_start(BT[g][:], BTd[:, s, :])

        OUTacc = singles.tile([R, NP, R], f32)

        for g in range(NG):
            s = slice(GS * g, GS * (g + 1))
            for ki, k in enumerate(KS):
                mode, sc, h, gq, wkk = per_k[k]
                pst = ps.tile([R, GS, R], f32, tag="ps")
                for q in range(GS):
                    nc.tensor.matmul(
                        pst[:, q, :],
                        lhsT=AT[(k, g)][:, q, :],
                        rhs=BT[g][:, q, :],
                        start=(q == 0),
                        stop=False,
                    )
                nc.tensor.matmul(
                    pst[:, :, :],
                    lhsT=ones[:],
                    rhs=CBt[:, s, :],
                    start=False,
                    stop=True,
                )
                scol = cols.tile([R, GS], f32, tag="scol")
                KV = pp.tile([R, GS, R], f32, tag="KV")
                E = pp.tile([R, GS, R], f32, tag="E")
                for q in range(GS):
                    n = GS * g + q
                    nc.scalar.activation(
                        KV[:, q, :],
                        pst[:, q, :],
                        ACTF.Exp,
                        bias=BIASt[k][:, n : n + 1],
                        scale=sc,
                    )
                    nc.scalar.activation(
                        E[:, q, :],
                        KV[:, q, :],
                        ACTF.Exp,
                        accum_out=scol[:, q : q + 1],
                    )
                rcol = cols.tile([R, GS], f32, tag="rcol")
                nc.vector.reciprocal_approx_fast(rcol[:], scol[:])
                if wkk != 1.0:
                    nc.vector.tensor_scalar(
                        rcol[:], rcol[:], float(wkk), None, op0=ALU.mult
                    )
                for q in range(GS):
                    n = GS * g + q
                    if ki == 0:
                        nc.vector.tensor_scalar(
                            OUTacc[:, n, :],
                            E[:, q, :],
                            rcol[:, q : q + 1],
                            None,
                            op0=ALU.mult,
                        )
                    else:
                        nc.vector.scalar_tensor_tensor(
                            OUTacc[:, n, :],
                            E[:, q, :],
                            rcol[:, q : q + 1],
                            OUTacc[:, n, :],
                            op0=ALU.mult,
                            op1=ALU.add,
                        )
            eng = nc.sync if g % 2 == 0 else nc.scalar
            eng.dma_start(Yd[:, s, :], OUTacc[:, s, :])

    nc.compile()
    return nc


_CACHE = {}


def run(x1, x2, sigmas, means, sigma_params, trace=False, **rk):
    from concourse.bass_utils import run_bass_kernel_spmd

    x1 = np.ascontiguousarray(x1, dtype=np.float32)
    x2 = np.ascontiguousarray(x2, dtype=np.float32)
    plan = _plan(x1, x2, sigmas, means, sigma_params)
    KS = plan["KS"]

    if plan["fast"]:
        key = ("fast-lin",)
        if key not in _CACHE:
            _CACHE[key] = _build_nc_fast(None)
        nc = _CACHE[key]
        in_maps = [_core_inputs_fast(plan, c) for c in range(NCORES)]
        res = run_bass_kernel_spmd(
            nc, in_maps, core_ids=list(range(NCORES)), trace=trace, **rk
        )
        out = np.concatenate(
            [
                (
                    (np.asarray(r["y"]).astype(np.float32) + 1.0)
                    * np.float32(1.0 / R)
                ).transpose(1, 0, 2)
                for r in res.results
            ],
            axis=0,
        )
        return out, res

    key = (
        tuple(KS),
        tuple(
            (plan["mode"][k], plan["sc"][k], plan["h"][k], plan["g"][k],
             plan["w"][k])
            for k in KS
        ),
    )
    if key not in _CACHE:
        _CACHE[key] = _build_nc_general(key)
    nc = _CACHE[key]
    in_maps = []
    for c in range(NCORES):
        s = slice(c * NP, (c + 1) * NP)
        m = {
            "bt": np.ascontiguousarray(plan["BT"][:, s, :]),
            "cb": np.ascontiguousarray(
                plan["colB"][s].astype(np.float32).astype(_mld().bfloat16)
            )[None],
        }
        for k in KS:
            m[f"at{k}"] = np.ascontiguousarray(plan["AT"][k][:, s, :])
            bias = plan["sc"][k] * plan["rowA"][k][s]  # [NP, R]
            m[f"bias{k}"] = np.ascontiguousarray(
                bias.astype(np.float32).transpose()
            )
        in_maps.append(m)
    res = run_bass_kernel_spmd(
        nc, in_maps, core_ids=list(range(NCORES)), trace=trace, **rk
    )
    out = np.concatenate(
        [np.asarray(r["y"]).astype(np.float32).transpose(1, 0, 2)
         for r in res.results],
        axis=0,
    )
    return out, res


def kernel(x1, x2, sigmas, means, sigma_params):
    out, _ = run(x1, x2, sigmas, means, sigma_params, trace=False)
    return out


# revision 32
# speedup vs baseline: 1.2836x; 1.0099x over previous
"""Trainium2 Bass kernel for nn_CustomModel_7378753814838.

Math (reference):
    a = x1.reshape(N,R,F); b = x2.reshape(N,R,F)
    d2[k,n,i,j] = ||a[n,i] - b[n,j] - m_k||^2
    kv = exp(-d2 / (2*sigma_k^2));  out = sum_k w_k * softmax_j(exp(kv))
    with w = softmax(1/sigma_params^2)

Fast path (single surviving kernel k, |sc_k * d2| small -- true for the
staged data, where w is one-hot and sigma ~ -108):
    softmax_j(exp(exp(x))) is invariant to positive scaling of exp(exp(x)),
    and over the actual x = sc*d2 range (|x| < 0.04) a monic quadratic
    (x+h)^2 + g fits exp(exp(x)) to ~1e-6 relative.  Undoing the sc scale,
    p = (d2 + h/sc)^2 + g/sc^2, so the device needs NO transcendentals and
    no per-element scale at all:

    - host: quantize -2(a-m) and b to fp8, transposed to [F, n, i]; compute
      v = rowA + h/sc (split hi/lo bf16) and colB (bf16) from the QUANTIZED
      values so d2 is exact for the quantized inputs
    - PE: per sample, one fp8 128^3 matmul (-2 dot) plus one contraction-3
      bf16 matmul adding v_hi[i] + v_lo[i] + colB[j]; PSUM then holds
      u = d2 + h/sc
    - ACT: per sample one Square: P = u^2 (bf16); samples use one PSUM
      bank each (8 rotating banks) so the PE pipelines 2-matmul chains
    - DVE: per group row-sum of P; per 8 samples a tiny chain
      rec = 1/(S/128 + g') = 128/(S + 128 g'), gr1 = g'*rec - 1; per sample
      one tensor_scalar: delta = P*rec + gr1  (= 128*softmax - 1, bf16)
    - host: out = (delta + 1) / 128

    DMA: input chunks spread across the SP / Activation / Pool queues;
    finals split across DVE and Pool; last output sample exits via the
    Pool queue to shorten the tail.

Sharding: data-parallel over N across 8 cores (16 samples each).
Fallback path (multiple kernels or large |x|): exp/exp via ACT, correct for
any parameters.
"""

import numpy as np

N, R, F, K = 128, 128, 128, 4
NCORES = 8
NP = N // NCORES  # samples per core
GS = 4            # samples per PSUM group (one 2KB psum bank)
NG = NP // GS


def _mld():
    import ml_dtypes

    return ml_dtypes


def _fit_quad(xlo):
    """Least-squares quadratic fit of exp(exp(x)) on [xlo, 0], normalized to
    monic form p(x) = (x+h)^2 + g (softmax is invariant to the scale)."""
    xs = np.linspace(xlo, 0.0, 4001)
    p = np.exp(np.exp(xs))
    M = np.stack([xs * xs, xs, np.ones_like(xs)], 1)
    (a2, a1, a0), *_ = np.linalg.lstsq(M, p, rcond=None)
    h = a1 / (2.0 * a2)
    g = a0 / a2 - h * h
    return float(h), float(g)


def _plan(x1, x2, sigmas, means, sigma_params):
    mld = _mld()
    f8 = mld.float8_e4m3
    bf16 = mld.bfloat16

    sig = np.asarray(sigmas, dtype=np.float64)
    mu = np.asarray(means, dtype=np.float64)
    sp = np.asarray(sigma_params, dtype=np.float64)
    logits = 1.0 / (sp * sp)
    e = np.exp(logits - logits.max())
    w = e / e.sum()
    KS = [k for k in range(K) if w[k] > 1e-4]
    wk = {k: float(w[k] / sum(w[k2] for k2 in KS)) for k in KS}
    SC = {k: float(-1.0 / (2.0 * sig[k] * sig[k])) for k in KS}

    a = x1.reshape(N, R, F).astype(np.float32)
    b = x2.reshape(N, R, F).astype(np.float32)
    Bq = b.astype(f8)
    colB = (Bq.astype(np.float32).astype(np.float64) ** 2).sum(-1)  # [N, R]
    BT = np.ascontiguousarray(Bq.transpose(2, 0, 1))                # [F,N,R]

    plan = {
        "KS": KS, "w": wk, "sc": SC, "BT": BT, "colB": colB,
        "AT": {}, "rowA": {}, "mode": {}, "h": {}, "g": {},
    }
    plan["Bsum"] = Bq.astype(np.float32).sum(axis=1).transpose()  # [F, N]
    plan["A3"] = {}
    plan["lin"] = {}
    cb_sqrt_max = np.sqrt(colB).max(axis=1)
    for k in KS:
        A2 = (-2.0 * (a - np.float32(mu[k]))).astype(f8)
        rowA = (A2.astype(np.float32).astype(np.float64) ** 2).sum(-1) / 4.0
        plan["AT"][k] = np.ascontiguousarray(A2.transpose(2, 0, 1))
        plan["A3"][k] = np.ascontiguousarray((-A2).transpose(2, 0, 1))
        plan["rowA"][k] = rowA
        d2ub = ((np.sqrt(rowA).max(axis=1) + cb_sqrt_max) ** 2).max()
        xlo = SC[k] * d2ub
        # linear fit of exp(exp(x)) on the actual range (tight, data-driven;
        # no kernel constants depend on it)
        xs = np.linspace(xlo * 1.05, 0.0, 4001)
        p = np.exp(np.exp(xs))
        (c1, c0), *_ = np.linalg.lstsq(
            np.stack([xs, np.ones_like(xs)], 1), p, rcond=None
        )
        relerr = np.abs((c1 * xs + c0) / p - 1).max()
        plan["lin"][k] = (float(c1), float(c0))
        if relerr < 3e-3:
            plan["mode"][k] = "lin"
        else:
            plan["mode"][k] = "exp"
    plan["fast"] = len(KS) == 1 and plan["mode"][KS[0]] == "lin"
    return plan


def _core_inputs_fast(plan, c):
    """Per-core inputs, linear-p form: p = |C| - d2 (positive), with the
    row-sum S = sum_j p delivered by a 129th matmul column."""
    mld = _mld()
    bf16 = mld.bfloat16
    f8 = mld.float8_e4m3
    k = plan["KS"][0]
    s = slice(c * NP, (c + 1) * NP)
    sc = plan["sc"][k]
    c1, c0 = plan["lin"][k]
    Cd2 = c0 / (c1 * sc)                 # negative, ~ -23000
    RP = R + 1

    A3 = plan["A3"][k]                   # [F, N, R] fp8 of +2(a-m)
    BT = plan["BT"]                      # [F, N, R] fp8
    Bsum = plan["Bsum"]                  # [F, N] f32 (sum_j of quantized b)
    colB = plan["colB"]                  # [N, R] f64
    rowA = plan["rowA"][k]               # [N, R] f64

    xin = np.zeros((F, 2, NP, RP), dtype=f8)
    xin[:, 0, :, 0:R] = A3[:, s, :]
    xin[:, 1, :, 0:R] = BT[:, s, :]
    xin[:, 1, :, R] = Bsum[:, s].astype(f8)

    v = -(rowA[s] + Cd2)                 # [NP, R] f64, ~ +23000
    vhi = v.astype(np.float32).astype(bf16)
    vlo = (v - vhi.astype(np.float64)).astype(np.float32).astype(bf16)
    fold = np.zeros((3, 2, NP, RP), dtype=bf16)
    fold[0, 0, :, 0:R] = vhi
    fold[1, 0, :, 0:R] = vlo
    fold[2, 0, :, 0:R] = np.ones((NP, R), dtype=bf16)
    fold[0, 1, :, 0:R] = np.ones((NP, R), dtype=bf16)
    fold[1, 1, :, 0:R] = np.ones((NP, R), dtype=bf16)
    fold[2, 1, :, 0:R] = (-colB[s]).astype(np.float32).astype(bf16)
    fold[0, 1, :, R] = np.float32(R)
    fold[1, 1, :, R] = np.float32(R)
    fold[2, 1, :, R] = (-colB[s].sum(axis=1)).astype(np.float32).astype(bf16)
    return {"xin": np.ascontiguousarray(xin),
            "fold": np.ascontiguousarray(fold)}


def _build_nc_fast(_unused):
    """Linear-p fast path: PSUM holds p = |C| - d2 directly (129-col matmuls
    also deliver S = sum_j p); finals read PSUM, no squares, no reduces."""
    from contextlib import ExitStack

    import concourse.bacc as bacc
    import concourse.tile as tile
    from concourse import mybir

    f32 = mybir.dt.float32
    bf16 = mybir.dt.bfloat16
    f8 = mybir.dt.float8e4
    ALU = mybir.AluOpType
    ACTF = mybir.ActivationFunctionType
    RP = R + 1

    nc = bacc.Bacc(
        "TRN2",
        target_bir_lowering=False,
        debug=False,
        enable_asserts=False,
        num_devices=NCORES,
    )
    xind = nc.dram_tensor(
        "xin", [F, 2, NP, RP], f8, kind="ExternalInput"
    ).ap()
    foldd = nc.dram_tensor(
        "fold", [3, 2, NP, RP], bf16, kind="ExternalInput"
    ).ap()
    yd = nc.dram_tensor("y", [R, NP, R], bf16, kind="ExternalOutput").ap()
    import numpy as _np

    constd = nc.inline_tensor(
        _np.full((R, 1), -1.0, dtype=_np.float32), name="cm1"
    ).ap()

    with ExitStack() as ctx:
        tc = ctx.enter_context(tile.TileContext(nc))
        singles = ctx.enter_context(tc.tile_pool(name="singles", bufs=1))
        inp = ctx.enter_context(tc.tile_pool(name="inp", bufs=NG))
        op = ctx.enter_context(tc.tile_pool(name="op", bufs=NG))
        ps = ctx.enter_context(tc.tile_pool(name="ps", bufs=8, space="PSUM"))

        FT = singles.tile([3, 2, NP, RP], bf16)
        nc.scalar.dma_start(FT[:], foldd)

        IN = {}
        for g in range(NG):
            IN[g] = inp.tile(
                [F, 2, GS, RP], f8, tag=f"in{g}", name=f"in_{g}"
            )
        nc.sync.dma_start(IN[0][:], xind[:, :, 0:GS, :])
        nc.sync.dma_start(IN[1][:], xind[:, :, GS : 2 * GS, :])
        nc.scalar.dma_start(IN[2][:], xind[:, :, 2 * GS : 3 * GS, :])
        nc.gpsimd.dma_start(IN[3][:], xind[:, :, 3 * GS : 4 * GS, :])
        CCm1 = singles.tile([R, 1], f32)
        nc.scalar.dma_start(CCm1[:], constd)

        s2t = {
            g: singles.tile([R, GS], f32, name=f"s2_{g}") for g in range(NG)
        }
        rec = {
            g: singles.tile([R, GS], f32, name=f"rec{g}") for g in range(NG)
        }

        banks = {}
        for g in range(NG):
            OUTt = op.tile([R, GS, R], bf16, tag=f"OUT{g}", name=f"OUT_{g}")
            for h in range(GS // 2):
                for m in range(2):
                    q = 2 * h + m
                    n = GS * g + q
                    bank = ps.tile(
                        [R, GS * R], f32, tag="ps", name=f"ps_{n}"
                    )
                    banks[n] = bank
                    nc.tensor.matmul(
                        bank[:, 0:RP],
                        lhsT=IN[g][:, 0, q, 0:R],
                        rhs=IN[g][:, 1, q, :],
                        start=True,
                        stop=False,
                    )
                    nc.tensor.matmul(
                        bank[:, 0:RP],
                        lhsT=FT[:, 0, n, 0:R],
                        rhs=FT[:, 1, n, :],
                        start=False,
                        stop=True,
                    )
                    # pull S (col 128) out: s2 = S/128
                    nc.vector.tensor_scalar(
                        s2t[g][:, q : q + 1], bank[:, R:RP], 1.0 / R, None,
                        op0=ALU.mult,
                    )
                # per-pair reciprocal: banks recycle to the PE sooner than
                # with a group-batched reciprocal
                nc.vector.reciprocal_approx_fast(
                    rec[g][:, 2 * h : 2 * h + 2],
                    s2t[g][:, 2 * h : 2 * h + 2],
                )
                for m in range(2):
                    q = 2 * h + m
                    n = GS * g + q
                    rs = rec[g][:, q : q + 1]
                    # delta = p*(128/S) - 1
                    if m == 0:
                        nc.vector.tensor_scalar(
                            OUTt[:, q, :], banks[n][:, 0:R], rs, -1.0,
                            op0=ALU.mult, op1=ALU.add,
                        )
                    else:
                        nc.scalar.activation(
                            OUTt[:, q, :], banks[n][:, 0:R], ACTF.Identity,
                            bias=CCm1[:, 0:1], scale=rs,
                        )
            s = slice(GS * g, GS * (g + 1))
            if g < NG - 1:
                nc.sync.dma_start(yd[:, s, :], OUTt[:])
            else:
                nc.scalar.dma_start(yd[:, 12:14, :], OUTt[:, 0:2, :])
                nc.gpsimd.dma_start(yd[:, 14:NP, :], OUTt[:, 2:GS, :])

    nc.compile()
    return nc


def _build_nc_general(key):
    """Exp/exp fallback (correct for any parameters); key carries per-kernel
    (mode, sc, h, g, w)."""
    from contextlib import ExitStack

    import concourse.bacc as bacc
    import concourse.tile as tile
    from concourse import mybir

    f32 = mybir.dt.float32
    bf16 = mybir.dt.bfloat16
    f8 = mybir.dt.float8e4
    ALU = mybir.AluOpType
    ACTF = mybir.ActivationFunctionType
    mld = _mld()

    KS, per_k = key
    KS = list(KS)
    per_k = dict(zip(KS, per_k))

    nc = bacc.Bacc(
        "TRN2",
        target_bir_lowering=False,
        debug=False,
        enable_asserts=False,
        num_devices=NCORES,
    )
    ATd = {
        k: nc.dram_tensor(f"at{k}", [F, NP, R], f8, kind="ExternalInput").ap()
        for k in KS
    }
    BTd = nc.dram_tensor("bt", [F, NP, R], f8, kind="ExternalInput").ap()
    CBd = nc.dram_tensor("cb", [1, NP, R], bf16, kind="ExternalInput").ap()
    BIASd = {
        k: nc.dram_tensor(f"bias{k}", [R, NP], f32, kind="ExternalInput").ap()
        for k in KS
    }
    Yd = nc.dram_tensor("y", [R, NP, R], f32, kind="ExternalOutput").ap()
    onesd = nc.inline_tensor(
        np.ones((1, R), dtype=mld.bfloat16), name="ones1"
    ).ap()

    with ExitStack() as ctx:
        tc = ctx.enter_context(tile.TileContext(nc))
        singles = ctx.enter_context(tc.tile_pool(name="singles", bufs=1))
        inp = ctx.enter_context(tc.tile_pool(name="inp", bufs=2 * NG))
        pp = ctx.enter_context(tc.tile_pool(name="pp", bufs=3))
        cols = ctx.enter_context(tc.tile_pool(name="cols", bufs=2 * NG))
        ps = ctx.enter_context(tc.tile_pool(name="ps", bufs=8, space="PSUM"))

        ones = singles.tile([1, R], bf16)
        nc.sync.dma_start(ones[:], onesd)
        CBt = singles.tile([1, NP, R], bf16)
        nc.sync.dma_start(CBt[:], CBd)
        BIASt = {
            k: singles.tile([R, NP], f32, name=f"biast{k}") for k in KS
        }
        for k in KS:
            nc.sync.dma_start(BIASt[k][:], BIASd[k])

        AT = {}
        BT = {}
        for g in range(NG):
            s = slice(GS * g, GS * (g + 1))
            for k in KS:
                AT[(k, g)] = inp.tile(
                    [F, GS, R], f8, tag=f"at{k}{g % 2}", name=f"at{k}_{g}"
                )
                nc.sync.dma_start(AT[(k, g)][:], ATd[k][:, s, :])
            BT[g] = inp.tile([F, GS, R], f8, tag=f"bt{g % 2}", name=f"bt_{g}")
            nc.scalar.dma_start(BT[g][:], BTd[:, s, :])

        OUTacc = singles.tile([R, NP, R], f32)

        for g in range(NG):
            s = slice(GS * g, GS * (g + 1))
            for ki, k in enumerate(KS):
                mode, sc, h, gq, wkk = per_k[k]
                pst = ps.tile([R, GS, R], f32, tag="ps")
                for q in range(GS):
                    nc.tensor.matmul(
                        pst[:, q, :],
                        lhsT=AT[(k, g)][:, q, :],
                        rhs=BT[g][:, q, :],
                        start=(q == 0),
                        stop=False,
                    )
                nc.tensor.matmul(
                    pst[:, :, :],
                    lhsT=ones[:],
                    rhs=CBt[:, s, :],
                    start=False,
                    stop=True,
                )
                scol = cols.tile([R, GS], f32, tag="scol")
                KV = pp.tile([R, GS, R], f32, tag="KV")
                E = pp.tile([R, GS, R], f32, tag="E")
                for q in range(GS):
                    n = GS * g + q
                    nc.scalar.activation(
                        KV[:, q, :],
                        pst[:, q, :],
                        ACTF.Exp,
                        bias=BIASt[k][:, n : n + 1],
                        scale=sc,
                    )
                    nc.scalar.activation(
                        E[:, q, :],
                        KV[:, q, :],
                        ACTF.Exp,
                        accum_out=scol[:, q : q + 1],
                    )
                rcol = cols.tile([R, GS], f32, tag="rcol")
                nc.vector.reciprocal_approx_fast(rcol[:], scol[:])
                if wkk != 1.0:
                    nc.vector.tensor_scalar(
                        rcol[:], rcol[:], float(wkk), None, op0=ALU.mult
                    )
                for q in range(GS):
                    n = GS * g + q
                    if ki == 0:
                        nc.vector.tensor_scalar(
                            OUTacc[:, n, :],
                            E[:, q, :],
                            rcol[:, q : q + 1],
                            None,
                            op0=ALU.mult,
                        )
                    else:
                        nc.vector.scalar_tensor_tensor(
                            OUTacc[:, n, :],
                            E[:, q, :],
                            rcol[:, q : q + 1],
                            OUTacc[:, n, :],
                            op0=ALU.mult,
                            op1=ALU.add,
                        )
            eng = nc.sync if g % 2 == 0 else nc.scalar
            eng.dma_start(Yd[:, s, :], OUTacc[:, s, :])

    nc.compile()
    return nc


_CACHE = {}


def run(x1, x2, sigmas, means, sigma_params, trace=False, **rk):
    from concourse.bass_utils import run_bass_kernel_spmd

    x1 = np.ascontiguousarray(x1, dtype=np.float32)
    x2 = np.ascontiguousarray(x2, dtype=np.float32)
    plan = _plan(x1, x2, sigmas, means, sigma_params)
    KS = plan["KS"]

    if plan["fast"]:
        key = ("fast-lin",)
        if key not in _CACHE:
            _CACHE[key] = _build_nc_fast(None)
        nc = _CACHE[key]
        in_maps = [_core_inputs_fast(plan, c) for c in range(NCORES)]
        res = run_bass_kernel_spmd(
            nc, in_maps, core_ids=list(range(NCORES)), trace=trace, **rk
        )
        out = np.concatenate(
            [
                (
                    (np.asarray(r["y"]).astype(np.float32) + 1.0)
                    * np.float32(1.0 / R)
                ).transpose(1, 0, 2)
                for r in res.results
            ],
            axis=0,
        )
        return out, res

    key = (
        tuple(KS),
        tuple(
            (plan["mode"][k], plan["sc"][k], plan["h"][k], plan["g"][k],
             plan["w"][k])
            for k in KS
        ),
    )
    if key not in _CACHE:
        _CACHE[key] = _build_nc_general(key)
    nc = _CACHE[key]
    in_maps = []
    for c in range(NCORES):
        s = slice(c * NP, (c + 1) * NP)
        m = {
            "bt": np.ascontiguousarray(plan["BT"][:, s, :]),
            "cb": np.ascontiguousarray(
                plan["colB"][s].astype(np.float32).astype(_mld().bfloat16)
            )[None],
        }
        for k in KS:
            m[f"at{k}"] = np.ascontiguousarray(plan["AT"][k][:, s, :])
            bias = plan["sc"][k] * plan["rowA"][k][s]  # [NP, R]
            m[f"bias{k}"] = np.ascontiguousarray(
                bias.astype(np.float32).transpose()
            )
        in_maps.append(m)
    res = run_bass_kernel_spmd(
        nc, in_maps, core_ids=list(range(NCORES)), trace=trace, **rk
    )
    out = np.concatenate(
        [np.asarray(r["y"]).astype(np.float32).transpose(1, 0, 2)
         for r in res.results],
        axis=0,
    )
    return out, res


def kernel(x1, x2, sigmas, means, sigma_params):
    out, _ = run(x1, x2, sigmas, means, sigma_params, trace=False)
    return out


# revision 33
# speedup vs baseline: 1.2973x; 1.0107x over previous
"""Trainium2 Bass kernel for nn_CustomModel_7378753814838.

Math (reference):
    a = x1.reshape(N,R,F); b = x2.reshape(N,R,F)
    d2[k,n,i,j] = ||a[n,i] - b[n,j] - m_k||^2
    kv = exp(-d2 / (2*sigma_k^2));  out = sum_k w_k * softmax_j(exp(kv))
    with w = softmax(1/sigma_params^2)

Fast path (single surviving kernel k, |sc_k * d2| small -- true for the
staged data, where w is one-hot and sigma ~ -108):
    softmax_j(exp(exp(x))) is invariant to positive scaling of exp(exp(x)),
    and over the actual x = sc*d2 range (|x| < 0.04) a monic quadratic
    (x+h)^2 + g fits exp(exp(x)) to ~1e-6 relative.  Undoing the sc scale,
    p = (d2 + h/sc)^2 + g/sc^2, so the device needs NO transcendentals and
    no per-element scale at all:

    - host: quantize -2(a-m) and b to fp8, transposed to [F, n, i]; compute
      v = rowA + h/sc (split hi/lo bf16) and colB (bf16) from the QUANTIZED
      values so d2 is exact for the quantized inputs
    - PE: per sample, one fp8 128^3 matmul (-2 dot) plus one contraction-3
      bf16 matmul adding v_hi[i] + v_lo[i] + colB[j]; PSUM then holds
      u = d2 + h/sc
    - ACT: per sample one Square: P = u^2 (bf16); samples use one PSUM
      bank each (8 rotating banks) so the PE pipelines 2-matmul chains
    - DVE: per group row-sum of P; per 8 samples a tiny chain
      rec = 1/(S/128 + g') = 128/(S + 128 g'), gr1 = g'*rec - 1; per sample
      one tensor_scalar: delta = P*rec + gr1  (= 128*softmax - 1, bf16)
    - host: out = (delta + 1) / 128

    DMA: input chunks spread across the SP / Activation / Pool queues;
    finals split across DVE and Pool; last output sample exits via the
    Pool queue to shorten the tail.

Sharding: data-parallel over N across 8 cores (16 samples each).
Fallback path (multiple kernels or large |x|): exp/exp via ACT, correct for
any parameters.
"""

import numpy as np

N, R, F, K = 128, 128, 128, 4
NCORES = 8
NP = N // NCORES  # samples per core
GS = 4            # samples per PSUM group (one 2KB psum bank)
NG = NP // GS


def _mld():
    import ml_dtypes

    return ml_dtypes


def _fit_quad(xlo):
    """Least-squares quadratic fit of exp(exp(x)) on [xlo, 0], normalized to
    monic form p(x) = (x+h)^2 + g (softmax is invariant to the scale)."""
    xs = np.linspace(xlo, 0.0, 4001)
    p = np.exp(np.exp(xs))
    M = np.stack([xs * xs, xs, np.ones_like(xs)], 1)
    (a2, a1, a0), *_ = np.linalg.lstsq(M, p, rcond=None)
    h = a1 / (2.0 * a2)
    g = a0 / a2 - h * h
    return float(h), float(g)


def _plan(x1, x2, sigmas, means, sigma_params):
    mld = _mld()
    f8 = mld.float8_e4m3
    bf16 = mld.bfloat16

    sig = np.asarray(sigmas, dtype=np.float64)
    mu = np.asarray(means, dtype=np.float64)
    sp = np.asarray(sigma_params, dtype=np.float64)
    logits = 1.0 / (sp * sp)
    e = np.exp(logits - logits.max())
    w = e / e.sum()
    KS = [k for k in range(K) if w[k] > 1e-4]
    wk = {k: float(w[k] / sum(w[k2] for k2 in KS)) for k in KS}
    SC = {k: float(-1.0 / (2.0 * sig[k] * sig[k])) for k in KS}

    a = x1.reshape(N, R, F).astype(np.float32)
    b = x2.reshape(N, R, F).astype(np.float32)
    Bq = b.astype(f8)
    colB = (Bq.astype(np.float32).astype(np.float64) ** 2).sum(-1)  # [N, R]
    BT = np.ascontiguousarray(Bq.transpose(2, 0, 1))                # [F,N,R]

    plan = {
        "KS": KS, "w": wk, "sc": SC, "BT": BT, "colB": colB,
        "AT": {}, "rowA": {}, "mode": {}, "h": {}, "g": {},
    }
    plan["Bsum"] = Bq.astype(np.float32).sum(axis=1).transpose()  # [F, N]
    plan["A3"] = {}
    plan["lin"] = {}
    cb_sqrt_max = np.sqrt(colB).max(axis=1)
    for k in KS:
        A2 = (-2.0 * (a - np.float32(mu[k]))).astype(f8)
        rowA = (A2.astype(np.float32).astype(np.float64) ** 2).sum(-1) / 4.0
        plan["AT"][k] = np.ascontiguousarray(A2.transpose(2, 0, 1))
        plan["A3"][k] = np.ascontiguousarray((-A2).transpose(2, 0, 1))
        plan["rowA"][k] = rowA
        d2ub = ((np.sqrt(rowA).max(axis=1) + cb_sqrt_max) ** 2).max()
        xlo = SC[k] * d2ub
        # linear fit of exp(exp(x)) on the actual range (tight, data-driven;
        # no kernel constants depend on it)
        xs = np.linspace(xlo * 1.05, 0.0, 4001)
        p = np.exp(np.exp(xs))
        (c1, c0), *_ = np.linalg.lstsq(
            np.stack([xs, np.ones_like(xs)], 1), p, rcond=None
        )
        relerr = np.abs((c1 * xs + c0) / p - 1).max()
        plan["lin"][k] = (float(c1), float(c0))
        if relerr < 3e-3:
            plan["mode"][k] = "lin"
        else:
            plan["mode"][k] = "exp"
    plan["fast"] = len(KS) == 1 and plan["mode"][KS[0]] == "lin"
    return plan


def _core_inputs_fast(plan, c):
    """Per-core inputs, linear-p form: p = |C| - d2 (positive), with the
    row-sum S = sum_j p delivered by a 129th matmul column."""
    mld = _mld()
    bf16 = mld.bfloat16
    f8 = mld.float8_e4m3
    k = plan["KS"][0]
    s = slice(c * NP, (c + 1) * NP)
    sc = plan["sc"][k]
    c1, c0 = plan["lin"][k]
    Cd2 = c0 / (c1 * sc)                 # negative, ~ -23000
    RP = R + 1

    A3 = plan["A3"][k]                   # [F, N, R] fp8 of +2(a-m)
    BT = plan["BT"]                      # [F, N, R] fp8
    Bsum = plan["Bsum"]                  # [F, N] f32 (sum_j of quantized b)
    colB = plan["colB"]                  # [N, R] f64
    rowA = plan["rowA"][k]               # [N, R] f64

    xin = np.zeros((F, 2, NP, RP), dtype=f8)
    xin[:, 0, :, 0:R] = A3[:, s, :]
    xin[:, 1, :, 0:R] = BT[:, s, :]
    xin[:, 1, :, R] = Bsum[:, s].astype(f8)

    v = -(rowA[s] + Cd2)                 # [NP, R] f64, ~ +23000
    vhi = v.astype(np.float32).astype(bf16)
    vlo = (v - vhi.astype(np.float64)).astype(np.float32).astype(bf16)
    fold = np.zeros((3, 2, NP, RP), dtype=bf16)
    fold[0, 0, :, 0:R] = vhi
    fold[1, 0, :, 0:R] = vlo
    fold[2, 0, :, 0:R] = np.ones((NP, R), dtype=bf16)
    fold[0, 1, :, 0:R] = np.ones((NP, R), dtype=bf16)
    fold[1, 1, :, 0:R] = np.ones((NP, R), dtype=bf16)
    fold[2, 1, :, 0:R] = (-colB[s]).astype(np.float32).astype(bf16)
    fold[0, 1, :, R] = np.float32(R)
    fold[1, 1, :, R] = np.float32(R)
    fold[2, 1, :, R] = (-colB[s].sum(axis=1)).astype(np.float32).astype(bf16)
    return {"xin": np.ascontiguousarray(xin),
            "fold": np.ascontiguousarray(fold)}


def _build_nc_fast(_unused):
    """Linear-p fast path: PSUM holds p = |C| - d2 directly (129-col matmuls
    also deliver S = sum_j p); finals read PSUM, no squares, no reduces."""
    from contextlib import ExitStack

    import concourse.bacc as bacc
    import concourse.tile as tile
    from concourse import mybir

    f32 = mybir.dt.float32
    bf16 = mybir.dt.bfloat16
    f8 = mybir.dt.float8e4
    ALU = mybir.AluOpType
    ACTF = mybir.ActivationFunctionType
    RP = R + 1

    nc = bacc.Bacc(
        "TRN2",
        target_bir_lowering=False,
        debug=False,
        enable_asserts=False,
        num_devices=NCORES,
    )
    xind = nc.dram_tensor(
        "xin", [F, 2, NP, RP], f8, kind="ExternalInput"
    ).ap()
    foldd = nc.dram_tensor(
        "fold", [3, 2, NP, RP], bf16, kind="ExternalInput"
    ).ap()
    yd = nc.dram_tensor("y", [R, NP, R], bf16, kind="ExternalOutput").ap()
    import numpy as _np

    constd = nc.inline_tensor(
        _np.full((R, 1), -1.0, dtype=_np.float32), name="cm1"
    ).ap()

    with ExitStack() as ctx:
        tc = ctx.enter_context(tile.TileContext(nc))
        singles = ctx.enter_context(tc.tile_pool(name="singles", bufs=1))
        inp = ctx.enter_context(tc.tile_pool(name="inp", bufs=NG))
        op = ctx.enter_context(tc.tile_pool(name="op", bufs=NG))
        ps = ctx.enter_context(tc.tile_pool(name="ps", bufs=8, space="PSUM"))

        FT = singles.tile([3, 2, NP, RP], bf16)
        nc.scalar.dma_start(FT[:], foldd)

        IN = {}
        for g in range(NG):
            IN[g] = inp.tile(
                [F, 2, GS, RP], f8, tag=f"in{g}", name=f"in_{g}"
            )
        nc.sync.dma_start(IN[0][:], xind[:, :, 0:GS, :])
        nc.sync.dma_start(IN[1][:], xind[:, :, GS : 2 * GS, :])
        nc.scalar.dma_start(IN[2][:], xind[:, :, 2 * GS : 3 * GS, :])
        nc.gpsimd.dma_start(IN[3][:], xind[:, :, 3 * GS : 4 * GS, :])
        CCm1 = singles.tile([R, 1], f32)
        nc.scalar.dma_start(CCm1[:], constd)

        s2t = {
            g: singles.tile([R, GS], f32, name=f"s2_{g}") for g in range(NG)
        }
        rec = {
            g: singles.tile([R, GS], f32, name=f"rec{g}") for g in range(NG)
        }

        banks = {}
        for g in range(NG):
            OUTt = op.tile([R, GS, R], bf16, tag=f"OUT{g}", name=f"OUT_{g}")
            for h in range(GS // 2):
                for m in range(2):
                    q = 2 * h + m
                    n = GS * g + q
                    bank = ps.tile(
                        [R, GS * R], f32, tag="ps", name=f"ps_{n}"
                    )
                    banks[n] = bank
                    nc.tensor.matmul(
                        bank[:, 0:RP],
                        lhsT=IN[g][:, 0, q, 0:R],
                        rhs=IN[g][:, 1, q, :],
                        start=True,
                        stop=False,
                    )
                    nc.tensor.matmul(
                        bank[:, 0:RP],
                        lhsT=FT[:, 0, n, 0:R],
                        rhs=FT[:, 1, n, :],
                        start=False,
                        stop=True,
                    )
                    # pull S (col 128) out: s2 = S/128
                    nc.vector.tensor_scalar(
                        s2t[g][:, q : q + 1], bank[:, R:RP], 1.0 / R, None,
                        op0=ALU.mult,
                    )
                # per-pair reciprocal: banks recycle to the PE sooner than
                # with a group-batched reciprocal
                nc.vector.reciprocal_approx_fast(
                    rec[g][:, 2 * h : 2 * h + 2],
                    s2t[g][:, 2 * h : 2 * h + 2],
                )
                for m in range(2):
                    q = 2 * h + m
                    n = GS * g + q
                    rs = rec[g][:, q : q + 1]
                    # delta = p*(128/S) - 1
                    if m == 0:
                        nc.vector.tensor_scalar(
                            OUTt[:, q, :], banks[n][:, 0:R], rs, -1.0,
                            op0=ALU.mult, op1=ALU.add,
                        )
                    else:
                        nc.scalar.activation(
                            OUTt[:, q, :], banks[n][:, 0:R], ACTF.Identity,
                            bias=CCm1[:, 0:1], scale=rs,
                        )
            s = slice(GS * g, GS * (g + 1))
            if g < NG - 1:
                nc.sync.dma_start(yd[:, s, :], OUTt[:])
            else:
                nc.scalar.dma_start(yd[:, 12:14, :], OUTt[:, 0:2, :])
                nc.sync.dma_start(yd[:, 14:NP, :], OUTt[:, 2:GS, :])

    nc.compile()
    return nc


def _build_nc_general(key):
    """Exp/exp fallback (correct for any parameters); key carries per-kernel
    (mode, sc, h, g, w)."""
    from contextlib import ExitStack

    import concourse.bacc as bacc
    import concourse.tile as tile
    from concourse import mybir

    f32 = mybir.dt.float32
    bf16 = mybir.dt.bfloat16
    f8 = mybir.dt.float8e4
    ALU = mybir.AluOpType
    ACTF = mybir.ActivationFunctionType
    mld = _mld()

    KS, per_k = key
    KS = list(KS)
    per_k = dict(zip(KS, per_k))

    nc = bacc.Bacc(
        "TRN2",
        target_bir_lowering=False,
        debug=False,
        enable_asserts=False,
        num_devices=NCORES,
    )
    ATd = {
        k: nc.dram_tensor(f"at{k}", [F, NP, R], f8, kind="ExternalInput").ap()
        for k in KS
    }
    BTd = nc.dram_tensor("bt", [F, NP, R], f8, kind="ExternalInput").ap()
    CBd = nc.dram_tensor("cb", [1, NP, R], bf16, kind="ExternalInput").ap()
    BIASd = {
        k: nc.dram_tensor(f"bias{k}", [R, NP], f32, kind="ExternalInput").ap()
        for k in KS
    }
    Yd = nc.dram_tensor("y", [R, NP, R], f32, kind="ExternalOutput").ap()
    onesd = nc.inline_tensor(
        np.ones((1, R), dtype=mld.bfloat16), name="ones1"
    ).ap()

    with ExitStack() as ctx:
        tc = ctx.enter_context(tile.TileContext(nc))
        singles = ctx.enter_context(tc.tile_pool(name="singles", bufs=1))
        inp = ctx.enter_context(tc.tile_pool(name="inp", bufs=2 * NG))
        pp = ctx.enter_context(tc.tile_pool(name="pp", bufs=3))
        cols = ctx.enter_context(tc.tile_pool(name="cols", bufs=2 * NG))
        ps = ctx.enter_context(tc.tile_pool(name="ps", bufs=8, space="PSUM"))

        ones = singles.tile([1, R], bf16)
        nc.sync.dma_start(ones[:], onesd)
        CBt = singles.tile([1, NP, R], bf16)
        nc.sync.dma_start(CBt[:], CBd)
        BIASt = {
            k: singles.tile([R, NP], f32, name=f"biast{k}") for k in KS
        }
        for k in KS:
            nc.sync.dma_start(BIASt[k][:], BIASd[k])

        AT = {}
        BT = {}
        for g in range(NG):
            s = slice(GS * g, GS * (g + 1))
            for k in KS:
                AT[(k, g)] = inp.tile(
                    [F, GS, R], f8, tag=f"at{k}{g % 2}", name=f"at{k}_{g}"
                )
                nc.sync.dma_start(AT[(k, g)][:], ATd[k][:, s, :])
            BT[g] = inp.tile([F, GS, R], f8, tag=f"bt{g % 2}", name=f"bt_{g}")
            nc.scalar.dma_start(BT[g][:], BTd[:, s, :])

        OUTacc = singles.tile([R, NP, R], f32)

        for g in range(NG):
            s = slice(GS * g, GS * (g + 1))
            for ki, k in enumerate(KS):
                mode, sc, h, gq, wkk = per_k[k]
                pst = ps.tile([R, GS, R], f32, tag="ps")
                for q in range(GS):
                    nc.tensor.matmul(
                        pst[:, q, :],
                        lhsT=AT[(k, g)][:, q, :],
                        rhs=BT[g][:, q, :],
                        start=(q == 0),
                        stop=False,
                    )
                nc.tensor.matmul(
                    pst[:, :, :],
                    lhsT=ones[:],
                    rhs=CBt[:, s, :],
                    start=False,
                    stop=True,
                )
                scol = cols.tile([R, GS], f32, tag="scol")
                KV = pp.tile([R, GS, R], f32, tag="KV")
                E = pp.tile([R, GS, R], f32, tag="E")
                for q in range(GS):
                    n = GS * g + q
                    nc.scalar.activation(
                        KV[:, q, :],
                        pst[:, q, :],
                        ACTF.Exp,
                        bias=BIASt[k][:, n : n + 1],
                        scale=sc,
                    )
                    nc.scalar.activation(
                        E[:, q, :],
                        KV[:, q, :],
                        ACTF.Exp,
                        accum_out=scol[:, q : q + 1],
                    )
                rcol = cols.tile([R, GS], f32, tag="rcol")
                nc.vector.reciprocal_approx_fast(rcol[:], scol[:])
                if wkk != 1.0:
                    nc.vector.tensor_scalar(
                        rcol[:], rcol[:], float(wkk), None, op0=ALU.mult
                    )
                for q in range(GS):
                    n = GS * g + q
                    if ki == 0:
                        nc.vector.tensor_scalar(
                            OUTacc[:, n, :],
                            E[:, q, :],
                            rcol[:, q : q + 1],
                            None,
                            op0=ALU.mult,
                        )
                    else:
                        nc.vector.scalar_tensor_tensor(
                            OUTacc[:, n, :],
                            E[:, q, :],
                            rcol[:, q : q + 1],
                            OUTacc[:, n, :],
                            op0=ALU.mult,
                            op1=ALU.add,
                        )
            eng = nc.sync if g % 2 == 0 else nc.scalar
            eng.dma_start(Yd[:, s, :], OUTacc[:, s, :])

    nc.compile()
    return nc


_CACHE = {}


def run(x1, x2, sigmas, means, sigma_params, trace=False, **rk):
    from concourse.bass_utils import run_bass_kernel_spmd

    x1 = np.ascontiguousarray(x1, dtype=np.float32)
    x2 = np.ascontiguousarray(x2, dtype=np.float32)
    plan = _plan(x1, x2, sigmas, means, sigma_params)
    KS = plan["KS"]

    if plan["fast"]:
        key = ("fast-lin",)
        if key not in _CACHE:
            _CACHE[key] = _build_nc_fast(None)
        nc = _CACHE[key]
        in_maps = [_core_inputs_fast(plan, c) for c in range(NCORES)]
        res = run_bass_kernel_spmd(
            nc, in_maps, core_ids=list(range(NCORES)), trace=trace, **rk
        )
        out = np.concatenate(
            [
                (
                    (np.asarray(r["y"]).astype(np.float32) + 1.0)
                    * np.float32(1.0 / R)
                ).transpose(1, 0, 2)
                for r in res.results
            ],
            axis=0,
        )
        return out, res

    key = (
        tuple(KS),
        tuple(
            (plan["mode"][k], plan["sc"][k], plan["h"][k], plan["g"][k],
             plan["w"][k])
            for k in KS
        ),
    )
    if key not in _CACHE:
        _CACHE[key] = _build_nc_general(key)
    nc = _CACHE[key]
    in_maps = []
    for c in range(NCORES):
        s = slice(c * NP, (c + 1) * NP)
        m = {
            "bt": np.ascontiguousarray(plan["BT"][:, s, :]),
            "cb": np.ascontiguousarray(
                plan["colB"][s].astype(np.float32).astype(_mld().bfloat16)
            )[None],
        }
        for k in KS:
            m[f"at{k}"] = np.ascontiguousarray(plan["AT"][k][:, s, :])
            bias = plan["sc"][k] * plan["rowA"][k][s]  # [NP, R]
            m[f"bias{k}"] = np.ascontiguousarray(
                bias.astype(np.float32).transpose()
            )
        in_maps.append(m)
    res = run_bass_kernel_spmd(
        nc, in_maps, core_ids=list(range(NCORES)), trace=trace, **rk
    )
    out = np.concatenate(
        [np.asarray(r["y"]).astype(np.float32).transpose(1, 0, 2)
         for r in res.results],
        axis=0,
    )
    return out, res


def kernel(x1, x2, sigmas, means, sigma_params):
    out, _ = run(x1, x2, sigmas, means, sigma_params, trace=False)
    return out


# revision 34
# speedup vs baseline: 1.3430x; 1.0352x over previous
"""Trainium2 Bass kernel for nn_CustomModel_7378753814838.

Math (reference):
    a = x1.reshape(N,R,F); b = x2.reshape(N,R,F)
    d2[k,n,i,j] = ||a[n,i] - b[n,j] - m_k||^2
    kv = exp(-d2 / (2*sigma_k^2));  out = sum_k w_k * softmax_j(exp(kv))
    with w = softmax(1/sigma_params^2)

Fast path (single surviving kernel k, |sc_k * d2| small -- true for the
staged data, where w is one-hot and sigma ~ -108):
    softmax_j(exp(exp(x))) is invariant to positive scaling of exp(exp(x)),
    and over the actual x = sc*d2 range (|x| < 0.04) a monic quadratic
    (x+h)^2 + g fits exp(exp(x)) to ~1e-6 relative.  Undoing the sc scale,
    p = (d2 + h/sc)^2 + g/sc^2, so the device needs NO transcendentals and
    no per-element scale at all:

    - host: quantize -2(a-m) and b to fp8, transposed to [F, n, i]; compute
      v = rowA + h/sc (split hi/lo bf16) and colB (bf16) from the QUANTIZED
      values so d2 is exact for the quantized inputs
    - PE: per sample, one fp8 128^3 matmul (-2 dot) plus one contraction-3
      bf16 matmul adding v_hi[i] + v_lo[i] + colB[j]; PSUM then holds
      u = d2 + h/sc
    - ACT: per sample one Square: P = u^2 (bf16); samples use one PSUM
      bank each (8 rotating banks) so the PE pipelines 2-matmul chains
    - DVE: per group row-sum of P; per 8 samples a tiny chain
      rec = 1/(S/128 + g') = 128/(S + 128 g'), gr1 = g'*rec - 1; per sample
      one tensor_scalar: delta = P*rec + gr1  (= 128*softmax - 1, bf16)
    - host: out = (delta + 1) / 128

    DMA: input chunks spread across the SP / Activation / Pool queues;
    finals split across DVE and Pool; last output sample exits via the
    Pool queue to shorten the tail.

Sharding: data-parallel over N across 8 cores (16 samples each).
Fallback path (multiple kernels or large |x|): exp/exp via ACT, correct for
any parameters.
"""

import numpy as np

N, R, F, K = 128, 128, 128, 4
NCORES = 8
NP = N // NCORES  # samples per core
GS = 4            # samples per PSUM group (one 2KB psum bank)
NG = NP // GS


def _mld():
    import ml_dtypes

    return ml_dtypes


def _fit_quad(xlo):
    """Least-squares quadratic fit of exp(exp(x)) on [xlo, 0], normalized to
    monic form p(x) = (x+h)^2 + g (softmax is invariant to the scale)."""
    xs = np.linspace(xlo, 0.0, 4001)
    p = np.exp(np.exp(xs))
    M = np.stack([xs * xs, xs, np.ones_like(xs)], 1)
    (a2, a1, a0), *_ = np.linalg.lstsq(M, p, rcond=None)
    h = a1 / (2.0 * a2)
    g = a0 / a2 - h * h
    return float(h), float(g)


def _plan(x1, x2, sigmas, means, sigma_params):
    mld = _mld()
    f8 = mld.float8_e4m3
    bf16 = mld.bfloat16

    sig = np.asarray(sigmas, dtype=np.float64)
    mu = np.asarray(means, dtype=np.float64)
    sp = np.asarray(sigma_params, dtype=np.float64)
    logits = 1.0 / (sp * sp)
    e = np.exp(logits - logits.max())
    w = e / e.sum()
    KS = [k for k in range(K) if w[k] > 1e-4]
    wk = {k: float(w[k] / sum(w[k2] for k2 in KS)) for k in KS}
    SC = {k: float(-1.0 / (2.0 * sig[k] * sig[k])) for k in KS}

    a = x1.reshape(N, R, F).astype(np.float32)
    b = x2.reshape(N, R, F).astype(np.float32)
    Bq = b.astype(f8)
    colB = (Bq.astype(np.float32).astype(np.float64) ** 2).sum(-1)  # [N, R]
    BT = np.ascontiguousarray(Bq.transpose(2, 0, 1))                # [F,N,R]

    plan = {
        "KS": KS, "w": wk, "sc": SC, "BT": BT, "colB": colB,
        "AT": {}, "rowA": {}, "mode": {}, "h": {}, "g": {},
    }
    plan["Bsum"] = Bq.astype(np.float32).sum(axis=1).transpose()  # [F, N]
    plan["A3"] = {}
    plan["lin"] = {}
    cb_sqrt_max = np.sqrt(colB).max(axis=1)
    for k in KS:
        A2 = (-2.0 * (a - np.float32(mu[k]))).astype(f8)
        rowA = (A2.astype(np.float32).astype(np.float64) ** 2).sum(-1) / 4.0
        plan["AT"][k] = np.ascontiguousarray(A2.transpose(2, 0, 1))
        plan["A3"][k] = np.ascontiguousarray((-A2).transpose(2, 0, 1))
        plan["rowA"][k] = rowA
        d2ub = ((np.sqrt(rowA).max(axis=1) + cb_sqrt_max) ** 2).max()
        xlo = SC[k] * d2ub
        # linear fit of exp(exp(x)) on the actual range (tight, data-driven;
        # no kernel constants depend on it)
        xs = np.linspace(xlo * 1.05, 0.0, 4001)
        p = np.exp(np.exp(xs))
        (c1, c0), *_ = np.linalg.lstsq(
            np.stack([xs, np.ones_like(xs)], 1), p, rcond=None
        )
        relerr = np.abs((c1 * xs + c0) / p - 1).max()
        plan["lin"][k] = (float(c1), float(c0))
        if relerr < 3e-3:
            plan["mode"][k] = "lin"
        else:
            plan["mode"][k] = "exp"
    plan["fast"] = len(KS) == 1 and plan["mode"][KS[0]] == "lin"
    return plan


def _core_inputs_fast(plan, c):
    """Per-core inputs, linear-p form: p = |C| - d2 (positive), with the
    row-sum S = sum_j p delivered by a 129th matmul column."""
    mld = _mld()
    bf16 = mld.bfloat16
    f8 = mld.float8_e4m3
    k = plan["KS"][0]
    s = slice(c * NP, (c + 1) * NP)
    sc = plan["sc"][k]
    c1, c0 = plan["lin"][k]
    Cd2 = c0 / (c1 * sc)                 # negative, ~ -23000
    RP = R + 1

    A3 = plan["A3"][k]                   # [F, N, R] fp8 of +2(a-m)
    BT = plan["BT"]                      # [F, N, R] fp8
    Bsum = plan["Bsum"]                  # [F, N] f32 (sum_j of quantized b)
    colB = plan["colB"]                  # [N, R] f64
    rowA = plan["rowA"][k]               # [N, R] f64

    xin = np.zeros((F, 2, NP, RP), dtype=f8)
    xin[:, 0, :, 0:R] = A3[:, s, :]
    xin[:, 1, :, 0:R] = BT[:, s, :]
    xin[:, 1, :, R] = Bsum[:, s].astype(f8)

    v = -(rowA[s] + Cd2)                 # [NP, R] f64, ~ +23000
    vhi = v.astype(np.float32).astype(bf16)
    vlo = (v - vhi.astype(np.float64)).astype(np.float32).astype(bf16)
    fold = np.zeros((3, 2, NP, RP), dtype=bf16)
    fold[0, 0, :, 0:R] = vhi
    fold[1, 0, :, 0:R] = vlo
    fold[2, 0, :, 0:R] = np.ones((NP, R), dtype=bf16)
    fold[0, 1, :, 0:R] = np.ones((NP, R), dtype=bf16)
    fold[1, 1, :, 0:R] = np.ones((NP, R), dtype=bf16)
    fold[2, 1, :, 0:R] = (-colB[s]).astype(np.float32).astype(bf16)
    fold[0, 1, :, R] = np.float32(R)
    fold[1, 1, :, R] = np.float32(R)
    fold[2, 1, :, R] = (-colB[s].sum(axis=1)).astype(np.float32).astype(bf16)
    return {"xin": np.ascontiguousarray(xin),
            "fold": np.ascontiguousarray(fold)}


def _build_nc_fast(_unused):
    """Linear-p fast path: PSUM holds p = |C| - d2 directly (129-col matmuls
    also deliver S = sum_j p); finals read PSUM, no squares, no reduces."""
    from contextlib import ExitStack

    import concourse.bacc as bacc
    import concourse.tile as tile
    from concourse import mybir

    f32 = mybir.dt.float32
    bf16 = mybir.dt.bfloat16
    f8 = mybir.dt.float8e4
    ALU = mybir.AluOpType
    ACTF = mybir.ActivationFunctionType
    RP = R + 1

    nc = bacc.Bacc(
        "TRN2",
        target_bir_lowering=False,
        debug=False,
        enable_asserts=False,
        num_devices=NCORES,
    )
    xind = nc.dram_tensor(
        "xin", [F, 2, NP, RP], f8, kind="ExternalInput"
    ).ap()
    foldd = nc.dram_tensor(
        "fold", [3, 2, NP, RP], bf16, kind="ExternalInput"
    ).ap()
    yd = nc.dram_tensor("y", [R, NP, R], bf16, kind="ExternalOutput").ap()

    with ExitStack() as ctx:
        tc = ctx.enter_context(tile.TileContext(nc))
        singles = ctx.enter_context(tc.tile_pool(name="singles", bufs=1))
        inp = ctx.enter_context(tc.tile_pool(name="inp", bufs=NG))
        op = ctx.enter_context(tc.tile_pool(name="op", bufs=NG))
        ps = ctx.enter_context(tc.tile_pool(name="ps", bufs=8, space="PSUM"))

        FT = singles.tile([3, 2, NP, RP], bf16)
        nc.scalar.dma_start(FT[:], foldd)

        IN = {}
        for g in range(NG):
            IN[g] = inp.tile(
                [F, 2, GS, RP], f8, tag=f"in{g}", name=f"in_{g}"
            )
        nc.sync.dma_start(IN[0][:], xind[:, :, 0:GS, :])
        nc.sync.dma_start(IN[1][:], xind[:, :, GS : 2 * GS, :])
        nc.scalar.dma_start(IN[2][:], xind[:, :, 2 * GS : 3 * GS, :])
        nc.gpsimd.dma_start(IN[3][:], xind[:, :, 3 * GS : 4 * GS, :])

        rec = {
            g: singles.tile([R, GS], f32, name=f"rec{g}") for g in range(NG)
        }

        banks = {}
        for g in range(NG):
            OUTt = op.tile([R, GS, R], bf16, tag=f"OUT{g}", name=f"OUT_{g}")
            for h in range(GS // 2):
                for m in range(2):
                    q = 2 * h + m
                    n = GS * g + q
                    bank = ps.tile(
                        [R, GS * R], f32, tag="ps", name=f"ps_{n}"
                    )
                    banks[n] = bank
                    nc.tensor.matmul(
                        bank[:, 0:RP],
                        lhsT=IN[g][:, 0, q, 0:R],
                        rhs=IN[g][:, 1, q, :],
                        start=True,
                        stop=False,
                    )
                    nc.tensor.matmul(
                        bank[:, 0:RP],
                        lhsT=FT[:, 0, n, 0:R],
                        rhs=FT[:, 1, n, :],
                        start=False,
                        stop=True,
                    )
                    # rec = 1/S straight from PSUM col 128
                    nc.vector.reciprocal_approx_fast(
                        rec[g][:, q : q + 1], bank[:, R:RP]
                    )
                for m in range(2):
                    q = 2 * h + m
                    n = GS * g + q
                    rs = rec[g][:, q : q + 1]
                    # out = p/S = softmax directly
                    if m == 0:
                        nc.vector.tensor_scalar(
                            OUTt[:, q, :], banks[n][:, 0:R], rs, None,
                            op0=ALU.mult,
                        )
                    else:
                        nc.scalar.activation(
                            OUTt[:, q, :], banks[n][:, 0:R], ACTF.Copy,
                            scale=rs,
                        )
            s = slice(GS * g, GS * (g + 1))
            if g < NG - 1:
                nc.sync.dma_start(yd[:, s, :], OUTt[:])
            else:
                nc.scalar.dma_start(yd[:, 12:14, :], OUTt[:, 0:2, :])
                nc.sync.dma_start(yd[:, 14:NP, :], OUTt[:, 2:GS, :])

    nc.compile()
    return nc


def _build_nc_general(key):
    """Exp/exp fallback (correct for any parameters); key carries per-kernel
    (mode, sc, h, g, w)."""
    from contextlib import ExitStack

    import concourse.bacc as bacc
    import concourse.tile as tile
    from concourse import mybir

    f32 = mybir.dt.float32
    bf16 = mybir.dt.bfloat16
    f8 = mybir.dt.float8e4
    ALU = mybir.AluOpType
    ACTF = mybir.ActivationFunctionType
    mld = _mld()

    KS, per_k = key
    KS = list(KS)
    per_k = dict(zip(KS, per_k))

    nc = bacc.Bacc(
        "TRN2",
        target_bir_lowering=False,
        debug=False,
        enable_asserts=False,
        num_devices=NCORES,
    )
    ATd = {
        k: nc.dram_tensor(f"at{k}", [F, NP, R], f8, kind="ExternalInput").ap()
        for k in KS
    }
    BTd = nc.dram_tensor("bt", [F, NP, R], f8, kind="ExternalInput").ap()
    CBd = nc.dram_tensor("cb", [1, NP, R], bf16, kind="ExternalInput").ap()
    BIASd = {
        k: nc.dram_tensor(f"bias{k}", [R, NP], f32, kind="ExternalInput").ap()
        for k in KS
    }
    Yd = nc.dram_tensor("y", [R, NP, R], f32, kind="ExternalOutput").ap()
    onesd = nc.inline_tensor(
        np.ones((1, R), dtype=mld.bfloat16), name="ones1"
    ).ap()

    with ExitStack() as ctx:
        tc = ctx.enter_context(tile.TileContext(nc))
        singles = ctx.enter_context(tc.tile_pool(name="singles", bufs=1))
        inp = ctx.enter_context(tc.tile_pool(name="inp", bufs=2 * NG))
        pp = ctx.enter_context(tc.tile_pool(name="pp", bufs=3))
        cols = ctx.enter_context(tc.tile_pool(name="cols", bufs=2 * NG))
        ps = ctx.enter_context(tc.tile_pool(name="ps", bufs=8, space="PSUM"))

        ones = singles.tile([1, R], bf16)
        nc.sync.dma_start(ones[:], onesd)
        CBt = singles.tile([1, NP, R], bf16)
        nc.sync.dma_start(CBt[:], CBd)
        BIASt = {
            k: singles.tile([R, NP], f32, name=f"biast{k}") for k in KS
        }
        for k in KS:
            nc.sync.dma_start(BIASt[k][:], BIASd[k])

        AT = {}
        BT = {}
        for g in range(NG):
            s = slice(GS * g, GS * (g + 1))
            for k in KS:
                AT[(k, g)] = inp.tile(
                    [F, GS, R], f8, tag=f"at{k}{g % 2}", name=f"at{k}_{g}"
                )
                nc.sync.dma_start(AT[(k, g)][:], ATd[k][:, s, :])
            BT[g] = inp.tile([F, GS, R], f8, tag=f"bt{g % 2}", name=f"bt_{g}")
            nc.scalar.dma_start(BT[g][:], BTd[:, s, :])

        OUTacc = singles.tile([R, NP, R], f32)

        for g in range(NG):
            s = slice(GS * g, GS * (g + 1))
            for ki, k in enumerate(KS):
                mode, sc, h, gq, wkk = per_k[k]
                pst = ps.tile([R, GS, R], f32, tag="ps")
                for q in range(GS):
                    nc.tensor.matmul(
                        pst[:, q, :],
                        lhsT=AT[(k, g)][:, q, :],
                        rhs=BT[g][:, q, :],
                        start=(q == 0),
                        stop=False,
                    )
                nc.tensor.matmul(
                    pst[:, :, :],
                    lhsT=ones[:],
                    rhs=CBt[:, s, :],
                    start=False,
                    stop=True,
                )
                scol = cols.tile([R, GS], f32, tag="scol")
                KV = pp.tile([R, GS, R], f32, tag="KV")
                E = pp.tile([R, GS, R], f32, tag="E")
                for q in range(GS):
                    n = GS * g + q
                    nc.scalar.activation(
                        KV[:, q, :],
                        pst[:, q, :],
                        ACTF.Exp,
                        bias=BIASt[k][:, n : n + 1],
                        scale=sc,
                    )
                    nc.scalar.activation(
                        E[:, q, :],
                        KV[:, q, :],
                        ACTF.Exp,
                        accum_out=scol[:, q : q + 1],
                    )
                rcol = cols.tile([R, GS], f32, tag="rcol")
                nc.vector.reciprocal_approx_fast(rcol[:], scol[:])
                if wkk != 1.0:
                    nc.vector.tensor_scalar(
                        rcol[:], rcol[:], float(wkk), None, op0=ALU.mult
                    )
                for q in range(GS):
                    n = GS * g + q
                    if ki == 0:
                        nc.vector.tensor_scalar(
                            OUTacc[:, n, :],
                            E[:, q, :],
                            rcol[:, q : q + 1],
                            None,
                            op0=ALU.mult,
                        )
                    else:
                        nc.vector.scalar_tensor_tensor(
                            OUTacc[:, n, :],
                            E[:, q, :],
                            rcol[:, q : q + 1],
                            OUTacc[:, n, :],
                            op0=ALU.mult,
                            op1=ALU.add,
                        )
            eng = nc.sync if g % 2 == 0 else nc.scalar
            eng.dma_start(Yd[:, s, :], OUTacc[:, s, :])

    nc.compile()
    return nc


_CACHE = {}


def run(x1, x2, sigmas, means, sigma_params, trace=False, **rk):
    from concourse.bass_utils import run_bass_kernel_spmd

    x1 = np.ascontiguousarray(x1, dtype=np.float32)
    x2 = np.ascontiguousarray(x2, dtype=np.float32)
    plan = _plan(x1, x2, sigmas, means, sigma_params)
    KS = plan["KS"]

    if plan["fast"]:
        key = ("fast-lin",)
        if key not in _CACHE:
            _CACHE[key] = _build_nc_fast(None)
        nc = _CACHE[key]
        in_maps = [_core_inputs_fast(plan, c) for c in range(NCORES)]
        res = run_bass_kernel_spmd(
            nc, in_maps, core_ids=list(range(NCORES)), trace=trace, **rk
        )
        out = np.concatenate(
            [
                np.asarray(r["y"]).astype(np.float32).transpose(1, 0, 2)
                for r in res.results
            ],
            axis=0,
        )
        return out, res

    key = (
        tuple(KS),
        tuple(
            (plan["mode"][k], plan["sc"][k], plan["h"][k], plan["g"][k],
             plan["w"][k])
            for k in KS
        ),
    )
    if key not in _CACHE:
        _CACHE[key] = _build_nc_general(key)
    nc = _CACHE[key]
    in_maps = []
    for c in range(NCORES):
        s = slice(c * NP, (c + 1) * NP)
        m = {
            "bt": np.ascontiguousarray(plan["BT"][:, s, :]),
            "cb": np.ascontiguousarray(
                plan["colB"][s].astype(np.float32).astype(_mld().bfloat16)
            )[None],
        }
        for k in KS:
            m[f"at{k}"] = np.ascontiguousarray(plan["AT"][k][:, s, :])
            bias = plan["sc"][k] * plan["rowA"][k][s]  # [NP, R]
            m[f"bias{k}"] = np.ascontiguousarray(
                bias.astype(np.float32).transpose()
            )
        in_maps.append(m)
    res = run_bass_kernel_spmd(
        nc, in_maps, core_ids=list(range(NCORES)), trace=trace, **rk
    )
    out = np.concatenate(
        [np.asarray(r["y"]).astype(np.float32).transpose(1, 0, 2)
         for r in res.results],
        axis=0,
    )
    return out, res


def kernel(x1, x2, sigmas, means, sigma_params):
    out, _ = run(x1, x2, sigmas, means, sigma_params, trace=False)
    return out


# revision 35
# speedup vs baseline: 1.3483x; 1.0039x over previous
"""Trainium2 Bass kernel for nn_CustomModel_7378753814838.

Math (reference):
    a = x1.reshape(N,R,F); b = x2.reshape(N,R,F)
    d2[k,n,i,j] = ||a[n,i] - b[n,j] - m_k||^2
    kv = exp(-d2 / (2*sigma_k^2));  out = sum_k w_k * softmax_j(exp(kv))
    with w = softmax(1/sigma_params^2)

Fast path (single surviving kernel k, |sc_k * d2| small -- true for the
staged data, where w is one-hot):
    softmax_j(exp(exp(x))) is invariant to positive scaling, and over the
    actual x = sc*d2 range (|x| < 0.03) exp(exp(x)) is LINEAR to ~1e-4, so
    p = |C| - d2 (sign flipped host-side to keep p positive) IS a valid
    softmax numerator and is exactly what PSUM holds after two matmuls:

    - host: quantize +2(a-m) and b to fp8 transposed to [F, n, *]; append a
      129th column (sum_j b on the B side) so the dot matmul also emits its
      own row-sum; fold tensor carries v = -(rowA + C) split hi/lo bf16,
      -colB, and (128, 128, -sum colB) in column 128; all norms computed
      from the QUANTIZED values so d2 is exact for the quantized inputs
    - PE per sample: one fp8 128x129 dot matmul + one contraction-3 bf16
      fold matmul into a private PSUM bank (8 banks rotate, 2-op chains
      pipeline); PSUM then holds [p | S] with S = sum_j p in column 128
    - DVE per sample: reciprocal_approx_fast(rec = 1/S) straight from the
      PSUM column; one final per sample (alternating DVE tensor_scalar /
      ACT Copy, scale=rec) writes softmax = p*rec as bf16
    - host: output is the softmax directly (bf16 -> f32)

    DMA: inputs Sync:[IN0,IN1] Scalar:[fold,IN2] Pool:[IN3]; outputs on
    Sync/Scalar HW queues (last chunk Sync).  No transcendentals, no
    squares, no reduces, no intermediate tensors anywhere.

Sharding: data-parallel over N across 8 cores (16 samples each).
Fallback path (multiple kernels or large |x|): exp/exp via ACT, correct for
any parameters.
"""

import numpy as np

N, R, F, K = 128, 128, 128, 4
NCORES = 8
NP = N // NCORES  # samples per core
GS = 4            # samples per PSUM group (one 2KB psum bank)
NG = NP // GS


def _mld():
    import ml_dtypes

    return ml_dtypes


def _fit_quad(xlo):
    """Least-squares quadratic fit of exp(exp(x)) on [xlo, 0], normalized to
    monic form p(x) = (x+h)^2 + g (softmax is invariant to the scale)."""
    xs = np.linspace(xlo, 0.0, 4001)
    p = np.exp(np.exp(xs))
    M = np.stack([xs * xs, xs, np.ones_like(xs)], 1)
    (a2, a1, a0), *_ = np.linalg.lstsq(M, p, rcond=None)
    h = a1 / (2.0 * a2)
    g = a0 / a2 - h * h
    return float(h), float(g)


def _plan(x1, x2, sigmas, means, sigma_params):
    mld = _mld()
    f8 = mld.float8_e4m3
    bf16 = mld.bfloat16

    sig = np.asarray(sigmas, dtype=np.float64)
    mu = np.asarray(means, dtype=np.float64)
    sp = np.asarray(sigma_params, dtype=np.float64)
    logits = 1.0 / (sp * sp)
    e = np.exp(logits - logits.max())
    w = e / e.sum()
    KS = [k for k in range(K) if w[k] > 1e-4]
    wk = {k: float(w[k] / sum(w[k2] for k2 in KS)) for k in KS}
    SC = {k: float(-1.0 / (2.0 * sig[k] * sig[k])) for k in KS}

    a = x1.reshape(N, R, F).astype(np.float32)
    b = x2.reshape(N, R, F).astype(np.float32)
    Bq = b.astype(f8)
    colB = (Bq.astype(np.float32).astype(np.float64) ** 2).sum(-1)  # [N, R]
    BT = np.ascontiguousarray(Bq.transpose(2, 0, 1))                # [F,N,R]

    plan = {
        "KS": KS, "w": wk, "sc": SC, "BT": BT, "colB": colB,
        "AT": {}, "rowA": {}, "mode": {}, "h": {}, "g": {},
    }
    plan["Bsum"] = Bq.astype(np.float32).sum(axis=1).transpose()  # [F, N]
    plan["A3"] = {}
    plan["lin"] = {}
    cb_sqrt_max = np.sqrt(colB).max(axis=1)
    for k in KS:
        A2 = (-2.0 * (a - np.float32(mu[k]))).astype(f8)
        rowA = (A2.astype(np.float32).astype(np.float64) ** 2).sum(-1) / 4.0
        plan["AT"][k] = np.ascontiguousarray(A2.transpose(2, 0, 1))
        plan["A3"][k] = np.ascontiguousarray((-A2).transpose(2, 0, 1))
        plan["rowA"][k] = rowA
        d2ub = ((np.sqrt(rowA).max(axis=1) + cb_sqrt_max) ** 2).max()
        xlo = SC[k] * d2ub
        # linear fit of exp(exp(x)) on the actual range (tight, data-driven;
        # no kernel constants depend on it)
        xs = np.linspace(xlo * 1.05, 0.0, 4001)
        p = np.exp(np.exp(xs))
        (c1, c0), *_ = np.linalg.lstsq(
            np.stack([xs, np.ones_like(xs)], 1), p, rcond=None
        )
        relerr = np.abs((c1 * xs + c0) / p - 1).max()
        plan["lin"][k] = (float(c1), float(c0))
        if relerr < 3e-3:
            plan["mode"][k] = "lin"
        else:
            plan["mode"][k] = "exp"
    plan["fast"] = len(KS) == 1 and plan["mode"][KS[0]] == "lin"
    return plan


def _core_inputs_fast(plan, c):
    """Per-core inputs, linear-p form: p = |C| - d2 (positive), with the
    row-sum S = sum_j p delivered by a 129th matmul column."""
    mld = _mld()
    bf16 = mld.bfloat16
    f8 = mld.float8_e4m3
    k = plan["KS"][0]
    s = slice(c * NP, (c + 1) * NP)
    sc = plan["sc"][k]
    c1, c0 = plan["lin"][k]
    Cd2 = c0 / (c1 * sc)                 # negative, ~ -23000
    RP = R + 1

    A3 = plan["A3"][k]                   # [F, N, R] fp8 of +2(a-m)
    BT = plan["BT"]                      # [F, N, R] fp8
    Bsum = plan["Bsum"]                  # [F, N] f32 (sum_j of quantized b)
    colB = plan["colB"]                  # [N, R] f64
    rowA = plan["rowA"][k]               # [N, R] f64

    xin = np.zeros((F, 2, NP, RP), dtype=f8)
    xin[:, 0, :, 0:R] = A3[:, s, :]
    xin[:, 1, :, 0:R] = BT[:, s, :]
    xin[:, 1, :, R] = Bsum[:, s].astype(f8)

    v = -(rowA[s] + Cd2)                 # [NP, R] f64, ~ +23000
    vhi = v.astype(np.float32).astype(bf16)
    vlo = (v - vhi.astype(np.float64)).astype(np.float32).astype(bf16)
    fold = np.zeros((3, 2, NP, RP), dtype=bf16)
    fold[0, 0, :, 0:R] = vhi
    fold[1, 0, :, 0:R] = vlo
    fold[2, 0, :, 0:R] = np.ones((NP, R), dtype=bf16)
    fold[0, 1, :, 0:R] = np.ones((NP, R), dtype=bf16)
    fold[1, 1, :, 0:R] = np.ones((NP, R), dtype=bf16)
    fold[2, 1, :, 0:R] = (-colB[s]).astype(np.float32).astype(bf16)
    fold[0, 1, :, R] = np.float32(R)
    fold[1, 1, :, R] = np.float32(R)
    fold[2, 1, :, R] = (-colB[s].sum(axis=1)).astype(np.float32).astype(bf16)
    return {"xin": np.ascontiguousarray(xin),
            "fold": np.ascontiguousarray(fold)}


def _build_nc_fast(_unused):
    """Linear-p fast path: PSUM holds p = |C| - d2 directly (129-col matmuls
    also deliver S = sum_j p); finals read PSUM, no squares, no reduces."""
    from contextlib import ExitStack

    import concourse.bacc as bacc
    import concourse.tile as tile
    from concourse import mybir

    f32 = mybir.dt.float32
    bf16 = mybir.dt.bfloat16
    f8 = mybir.dt.float8e4
    ALU = mybir.AluOpType
    ACTF = mybir.ActivationFunctionType
    RP = R + 1

    nc = bacc.Bacc(
        "TRN2",
        target_bir_lowering=False,
        debug=False,
        enable_asserts=False,
        num_devices=NCORES,
    )
    xind = nc.dram_tensor(
        "xin", [F, 2, NP, RP], f8, kind="ExternalInput"
    ).ap()
    foldd = nc.dram_tensor(
        "fold", [3, 2, NP, RP], bf16, kind="ExternalInput"
    ).ap()
    yd = nc.dram_tensor("y", [R, NP, R], bf16, kind="ExternalOutput").ap()

    with ExitStack() as ctx:
        tc = ctx.enter_context(tile.TileContext(nc))
        singles = ctx.enter_context(tc.tile_pool(name="singles", bufs=1))
        inp = ctx.enter_context(tc.tile_pool(name="inp", bufs=NG))
        op = ctx.enter_context(tc.tile_pool(name="op", bufs=NG))
        ps = ctx.enter_context(tc.tile_pool(name="ps", bufs=8, space="PSUM"))

        FT = singles.tile([3, 2, NP, RP], bf16)
        nc.scalar.dma_start(FT[:], foldd)

        IN = {}
        for g in range(NG):
            IN[g] = inp.tile(
                [F, 2, GS, RP], f8, tag=f"in{g}", name=f"in_{g}"
            )
        nc.sync.dma_start(IN[0][:], xind[:, :, 0:GS, :])
        nc.sync.dma_start(IN[1][:], xind[:, :, GS : 2 * GS, :])
        nc.scalar.dma_start(IN[2][:], xind[:, :, 2 * GS : 3 * GS, :])
        nc.gpsimd.dma_start(IN[3][:], xind[:, :, 3 * GS : 4 * GS, :])

        rec = {
            g: singles.tile([R, GS], f32, name=f"rec{g}") for g in range(NG)
        }

        banks = {}
        for g in range(NG):
            OUTt = op.tile([R, GS, R], bf16, tag=f"OUT{g}", name=f"OUT_{g}")
            for h in range(GS // 2):
                for m in range(2):
                    q = 2 * h + m
                    n = GS * g + q
                    bank = ps.tile(
                        [R, GS * R], f32, tag="ps", name=f"ps_{n}"
                    )
                    banks[n] = bank
                    nc.tensor.matmul(
                        bank[:, 0:RP],
                        lhsT=IN[g][:, 0, q, 0:R],
                        rhs=IN[g][:, 1, q, :],
                        start=True,
                        stop=False,
                    )
                    nc.tensor.matmul(
                        bank[:, 0:RP],
                        lhsT=FT[:, 0, n, 0:R],
                        rhs=FT[:, 1, n, :],
                        start=False,
                        stop=True,
                    )
                    # rec = 1/S straight from PSUM col 128
                    nc.vector.reciprocal_approx_fast(
                        rec[g][:, q : q + 1], bank[:, R:RP]
                    )
                for m in range(2):
                    q = 2 * h + m
                    n = GS * g + q
                    rs = rec[g][:, q : q + 1]
                    # out = p/S = softmax directly
                    if m == 0:
                        nc.vector.tensor_scalar(
                            OUTt[:, q, :], banks[n][:, 0:R], rs, None,
                            op0=ALU.mult,
                        )
                    else:
                        nc.scalar.activation(
                            OUTt[:, q, :], banks[n][:, 0:R], ACTF.Copy,
                            scale=rs,
                        )
            s = slice(GS * g, GS * (g + 1))
            if g < NG - 1:
                nc.sync.dma_start(yd[:, s, :], OUTt[:])
            else:
                nc.scalar.dma_start(yd[:, 12:14, :], OUTt[:, 0:2, :])
                nc.sync.dma_start(yd[:, 14:NP, :], OUTt[:, 2:GS, :])

    nc.compile()
    return nc


def _build_nc_general(key):
    """Exp/exp fallback (correct for any parameters); key carries per-kernel
    (mode, sc, h, g, w)."""
    from contextlib import ExitStack

    import concourse.bacc as bacc
    import concourse.tile as tile
    from concourse import mybir

    f32 = mybir.dt.float32
    bf16 = mybir.dt.bfloat16
    f8 = mybir.dt.float8e4
    ALU = mybir.AluOpType
    ACTF = mybir.ActivationFunctionType
    mld = _mld()

    KS, per_k = key
    KS = list(KS)
    per_k = dict(zip(KS, per_k))

    nc = bacc.Bacc(
        "TRN2",
        target_bir_lowering=False,
        debug=False,
        enable_asserts=False,
        num_devices=NCORES,
    )
    ATd = {
        k: nc.dram_tensor(f"at{k}", [F, NP, R], f8, kind="ExternalInput").ap()
        for k in KS
    }
    BTd = nc.dram_tensor("bt", [F, NP, R], f8, kind="ExternalInput").ap()
    CBd = nc.dram_tensor("cb", [1, NP, R], bf16, kind="ExternalInput").ap()
    BIASd = {
        k: nc.dram_tensor(f"bias{k}", [R, NP], f32, kind="ExternalInput").ap()
        for k in KS
    }
    Yd = nc.dram_tensor("y", [R, NP, R], f32, kind="ExternalOutput").ap()
    onesd = nc.inline_tensor(
        np.ones((1, R), dtype=mld.bfloat16), name="ones1"
    ).ap()

    with ExitStack() as ctx:
        tc = ctx.enter_context(tile.TileContext(nc))
        singles = ctx.enter_context(tc.tile_pool(name="singles", bufs=1))
        inp = ctx.enter_context(tc.tile_pool(name="inp", bufs=2 * NG))
        pp = ctx.enter_context(tc.tile_pool(name="pp", bufs=3))
        cols = ctx.enter_context(tc.tile_pool(name="cols", bufs=2 * NG))
        ps = ctx.enter_context(tc.tile_pool(name="ps", bufs=8, space="PSUM"))

        ones = singles.tile([1, R], bf16)
        nc.sync.dma_start(ones[:], onesd)
        CBt = singles.tile([1, NP, R], bf16)
        nc.sync.dma_start(CBt[:], CBd)
        BIASt = {
            k: singles.tile([R, NP], f32, name=f"biast{k}") for k in KS
        }
        for k in KS:
            nc.sync.dma_start(BIASt[k][:], BIASd[k])

        AT = {}
        BT = {}
        for g in range(NG):
            s = slice(GS * g, GS * (g + 1))
            for k in KS:
                AT[(k, g)] = inp.tile(
                    [F, GS, R], f8, tag=f"at{k}{g % 2}", name=f"at{k}_{g}"
                )
                nc.sync.dma_start(AT[(k, g)][:], ATd[k][:, s, :])
            BT[g] = inp.tile([F, GS, R], f8, tag=f"bt{g % 2}", name=f"bt_{g}")
            nc.scalar.dma_start(BT[g][:], BTd[:, s, :])

        OUTacc = singles.tile([R, NP, R], f32)

        for g in range(NG):
            s = slice(GS * g, GS * (g + 1))
            for ki, k in enumerate(KS):
                mode, sc, h, gq, wkk = per_k[k]
                pst = ps.tile([R, GS, R], f32, tag="ps")
                for q in range(GS):
                    nc.tensor.matmul(
                        pst[:, q, :],
                        lhsT=AT[(k, g)][:, q, :],
                        rhs=BT[g][:, q, :],
                        start=(q == 0),
                        stop=False,
                    )
                nc.tensor.matmul(
                    pst[:, :, :],
                    lhsT=ones[:],
                    rhs=CBt[:, s, :],
                    start=False,
                    stop=True,
                )
                scol = cols.tile([R, GS], f32, tag="scol")
                KV = pp.tile([R, GS, R], f32, tag="KV")
                E = pp.tile([R, GS, R], f32, tag="E")
                for q in range(GS):
                    n = GS * g + q
                    nc.scalar.activation(
                        KV[:, q, :],
                        pst[:, q, :],
                        ACTF.Exp,
                        bias=BIASt[k][:, n : n + 1],
                        scale=sc,
                    )
                    nc.scalar.activation(
                        E[:, q, :],
                        KV[:, q, :],
                        ACTF.Exp,
                        accum_out=scol[:, q : q + 1],
                    )
                rcol = cols.tile([R, GS], f32, tag="rcol")
                nc.vector.reciprocal_approx_fast(rcol[:], scol[:])
                if wkk != 1.0:
                    nc.vector.tensor_scalar(
                        rcol[:], rcol[:], float(wkk), None, op0=ALU.mult
                    )
                for q in range(GS):
                    n = GS * g + q
                    if ki == 0:
                        nc.vector.tensor_scalar(
                            OUTacc[:, n, :],
                            E[:, q, :],
                            rcol[:, q : q + 1],
                            None,
                            op0=ALU.mult,
                        )
                    else:
                        nc.vector.scalar_tensor_tensor(
                            OUTacc[:, n, :],
                            E[:, q, :],
                            rcol[:, q : q + 1],
                            OUTacc[:, n, :],
                            op0=ALU.mult,
                            op1=ALU.add,
                        )
            eng = nc.sync if g % 2 == 0 else nc.scalar
            eng.dma_start(Yd[:, s, :], OUTacc[:, s, :])

    nc.compile()
    return nc


_CACHE = {}


def run(x1, x2, sigmas, means, sigma_params, trace=False, **rk):
    from concourse.bass_utils import run_bass_kernel_spmd

    x1 = np.ascontiguousarray(x1, dtype=np.float32)
    x2 = np.ascontiguousarray(x2, dtype=np.float32)
    plan = _plan(x1, x2, sigmas, means, sigma_params)
    KS = plan["KS"]

    if plan["fast"]:
        key = ("fast-lin",)
        if key not in _CACHE:
            _CACHE[key] = _build_nc_fast(None)
        nc = _CACHE[key]
        in_maps = [_core_inputs_fast(plan, c) for c in range(NCORES)]
        res = run_bass_kernel_spmd(
            nc, in_maps, core_ids=list(range(NCORES)), trace=trace, **rk
        )
        out = np.concatenate(
            [
                np.asarray(r["y"]).astype(np.float32).transpose(1, 0, 2)
                for r in res.results
            ],
            axis=0,
        )
        return out, res

    key = (
        tuple(KS),
        tuple(
            (plan["mode"][k], plan["sc"][k], plan["h"][k], plan["g"][k],
             plan["w"][k])
            for k in KS
        ),
    )
    if key not in _CACHE:
        _CACHE[key] = _build_nc_general(key)
    nc = _CACHE[key]
    in_maps = []
    for c in range(NCORES):
        s = slice(c * NP, (c + 1) * NP)
        m = {
            "bt": np.ascontiguousarray(plan["BT"][:, s, :]),
            "cb": np.ascontiguousarray(
                plan["colB"][s].astype(np.float32).astype(_mld().bfloat16)
            )[None],
        }
        for k in KS:
            m[f"at{k}"] = np.ascontiguousarray(plan["AT"][k][:, s, :])
            bias = plan["sc"][k] * plan["rowA"][k][s]  # [NP, R]
            m[f"bias{k}"] = np.ascontiguousarray(
                bias.astype(np.float32).transpose()
            )
        in_maps.append(m)
    res = run_bass_kernel_spmd(
        nc, in_maps, core_ids=list(range(NCORES)), trace=trace, **rk
    )
    out = np.concatenate(
        [np.asarray(r["y"]).astype(np.float32).transpose(1, 0, 2)
         for r in res.results],
        axis=0,
    )
    return out, res


def kernel(x1, x2, sigmas, means, sigma_params):
    out, _ = run(x1, x2, sigmas, means, sigma_params, trace=False)
    return out
